# revision 23
# baseline (speedup 1.0000x reference)
"""Trainium2 Bass/Tile kernel for a dense transformer block.

B=2, S=2048, D=1024, H=16 heads (dh=64), FF=4096, f32 IO.

Sharding: 8 cores = (2 batches) x (4 query-slices of 512 tokens), zero
cross-core communication (K/V recomputed per core).

v3: fp8 attention + split-query software pipeline.  All attention GEMMs
(Q/K/V/O projections, AV) are fp8e4m3 DoubleRow matmuls (0.5 PE
cycles/row, weights host-scaled by 64); scores run on fp8 Q/K (the
1/sqrt(dh) folds into the softmax exp scale); exp writes fp8 directly.
The FFN stays bf16 (fp8 FFN breaches the 2e-2 gate).  LayerNorm stats /
broadcast matmuls use float32r operands (1 cycle/row vs 4 for fp32).

The queries are processed in two 256-wide halves: era A runs attention
for half 0 (plus all K/V production), era B runs attention for half 1
with the ENTIRE half-0 FFN interleaved into its pair loop (softmax exp
keeps ACT busy while the FFN matmuls fill the PE), era C finishes with
the half-1 FFN.  LN statistics accumulate via DVE adds into SBUF so the
interleaved-FFN matmuls can share a single 2-slot PSUM ring with the
reciprocal broadcasts.
"""

import os
from contextlib import ExitStack

import numpy as np
import ml_dtypes

import concourse.bass as bass
import concourse.tile as tile
from concourse import bacc, mybir
from concourse.bass_utils import run_bass_kernel_spmd

BF16 = mybir.dt.bfloat16
F32 = mybir.dt.float32
F32R = mybir.dt.float32r
F8E4 = mybir.dt.float8e4
AF = mybir.ActivationFunctionType
OP = mybir.AluOpType
PM = mybir.MatmulPerfMode

B, S, D, H, FF = 2, 2048, 1024, 16, 4096
DH = D // H            # 64
NCORES = 8
QS = S // 4            # 512 queries per core
HQ = QS // 2           # 256-query pipeline half
NK = S // 128          # 16 key chunks
ND = D // 128          # 8 feature chunks
NF = FF // 128         # 32 ff chunks
NP = H // 2            # 8 head pairs
VW = DH + 1            # 65 = head width + ones column
EPS = 1e-12
WS = 64.0              # host-side fp8 weight scale
IWS = 1.0 / WS
IWS2 = IWS * IWS


def _blocks(n, w):
    return [list(range(i, min(i + w, n))) for i in range(0, n, w)]


def _build(use_mask, use_bv, use_bf2, dbg=False):
    nc = bacc.Bacc("TRN2", target_bir_lowering=False, debug=False)

    def din(name, shape, dtype):
        return nc.dram_tensor(name, shape, dtype, kind="ExternalInput").ap()

    def dump(name, tl):
        if not dbg:
            return
        dd = nc.dram_tensor(f"dbg_{name}", list(tl.shape), tl.dtype,
                            kind="ExternalOutput").ap()
        nc.sync.dma_start(out=dd, in_=tl)

    xkT_d = din("xkT", [128, ND, S], F8E4)
    xqT_d = din("xqT", [128, ND, QS], F8E4)
    xqTf_d = din("xqTf", [128, ND, QS], BF16)     # x slice, +bo folded in
    wq_d = din("wq", [128, ND, D], F8E4)          # pre-scaled by WS on host
    wk_d = din("wk", [128, ND, D], F8E4)
    wv_d = din("wv", [128, ND, D], F8E4)
    wo_d = din("wo", [128, ND, D], F8E4)
    w1_d = din("w1", [128, 8, ND, 512], BF16)     # [p, jb, k, n]
    w2_d = din("w2", [128, 4, 2, 16, 256], BF16)  # [p, jpair, kh, k16, n]
    bq_d = din("bq", [D], F32)
    bk_d = din("bk", [D], F32)
    bf1_d = din("bf1", [FF], F32)
    g1_d = din("g1", [D], F32)
    be1_d = din("be1", [D], F32)
    g2_d = din("g2", [D], F32)
    be2_d = din("be2", [D], F32)
    bf2_d = din("bf2", [D], F32) if use_bf2 else None
    bv_d = din("bv", [D], BF16) if use_bv else None     # pre-scaled by WS
    madd_d = din("madd", [S], F32) if use_mask else None
    yT_d = nc.dram_tensor("yT", [D, QS], F32, kind="ExternalOutput").ap()

    with tile.TileContext(nc) as tc, ExitStack() as glob:
        const = glob.enter_context(tc.tile_pool(name="const", bufs=1))
        gx = glob.enter_context(tc.tile_pool(name="gx", bufs=1))
        ffx = glob.enter_context(tc.tile_pool(name="ffx", bufs=1))
        att_ps = ExitStack()
        avp = att_ps.enter_context(tc.tile_pool(name="avp", bufs=2, space="PSUM"))
        mmp = att_ps.enter_context(tc.tile_pool(name="mmp", bufs=2, space="PSUM"))
        scp = att_ps.enter_context(tc.tile_pool(name="scp", bufs=2, space="PSUM"))

        # ---------------- big tiles + front-loaded DMAs ----------------
        hT8 = gx.tile([128, ND, QS], F8E4, name="hT8")          # 64*h, attn out
        wo_sb = gx.tile([128, ND, D], F8E4, name="wo_sb")
        xqf_sb = gx.tile([128, ND, QS], BF16, name="xqf_sb")
        attbig_cm = ExitStack()
        attbig = attbig_cm.enter_context(tc.tile_pool(name="attbig", bufs=1))
        KT = attbig.tile([128, ND, S], F8E4, name="KT")         # fp8(k + bk)
        V3 = attbig.tile([128, NK, H * VW], F8E4, name="V3")    # V + ones cols
        QT = attbig.tile([128, ND, QS], F8E4, name="QT")
        att2 = ExitStack()
        recipp = att2.enter_context(tc.tile_pool(name="recipp", bufs=2))
        expp = att2.enter_context(tc.tile_pool(name="expp", bufs=6))
        lnsbA = att2.enter_context(tc.tile_pool(name="lnsbA", bufs=2))
        wstA = att2.enter_context(tc.tile_pool(name="wstA", bufs=2))
        xkp_cm = ExitStack()
        xkp = xkp_cm.enter_context(tc.tile_pool(name="xkp", bufs=1))
        xk_sb = xkp.tile([128, ND, S], F8E4, name="xk_sb")
        wk_sb = xkp.tile([128, ND, D], F8E4, name="wk_sb")
        wv_sb = xkp.tile([128, ND, D], F8E4, name="wv_sb")
        wq_sb = wstA.tile([128, ND, D], F8E4, tag="wst", name="wq_sb")
        xq_sb = wstA.tile([128, ND, QS], F8E4, tag="wst", name="xq_sb")
        nc.sync.dma_start(out=xq_sb, in_=xqT_d[:])
        nc.sync.dma_start(out=wq_sb[:, :, 0:512], in_=wq_d[:, :, 0:512])
        nc.sync.dma_start(out=wq_sb[:, :, 512:D], in_=wq_d[:, :, 512:D])
        # ---------------- constants & small params ----------------
        bq_sb = const.tile([128, ND], F32, name="bq_sb")
        nc.sync.dma_start(out=bq_sb, in_=bq_d.rearrange("(c p) -> p c", p=128))
        bk_sb = const.tile([128, ND], F32, name="bk_sb")
        nc.sync.dma_start(out=bk_sb, in_=bk_d.rearrange("(c p) -> p c", p=128))
        bf1_sb = const.tile([128, NF], F32, name="bf1_sb")
        nc.sync.dma_start(out=bf1_sb, in_=bf1_d.rearrange("(c p) -> p c", p=128))
        if use_bf2:
            bf2_sb = const.tile([128, ND], F32, name="bf2_sb")
            nc.sync.dma_start(out=bf2_sb, in_=bf2_d.rearrange("(c p) -> p c", p=128))
        if use_mask:
            madd_sb = const.tile([128, NK], F32, name="madd_sb")
            nc.sync.dma_start(out=madd_sb, in_=madd_d.rearrange("(c p) -> p c", p=128))
        if use_bv:
            bv_sb = const.tile([1, D], BF16, name="bv_sb")
            nc.sync.dma_start(out=bv_sb, in_=bv_d.rearrange("(a d) -> a d", a=1))
            ones1b = const.tile([1, 128], BF16, name="ones1b")
            nc.vector.memset(ones1b, 1.0)
        # reciprocal broadcast selectors carry the 64x for the fp8 hT scale.
        # (memset can't write f32r directly; stage in f32 and DVE-round.)
        stg = const.tile([128, 4], F32, name="stg")
        nc.vector.memset(stg[:, 2:3], 1.0 / D)
        indstg = const.tile([1, 128 + HQ], F32, name="indstg")
        nc.vector.memset(indstg, 0.0)
        nc.vector.memset(indstg[0:1, 0:64], WS)
        indE = const.tile([1, 128], F32R, name="indE")
        nc.vector.tensor_copy(out=indE, in_=indstg[0:1, 0:128])
        nc.vector.memset(indstg[0:1, 0:64], 0.0)
        nc.vector.memset(indstg[0:1, 64:128], WS)
        indO = const.tile([1, 128], F32R, name="indO")
        nc.vector.tensor_copy(out=indO, in_=indstg[0:1, 0:128])
        invD128 = const.tile([128, 1], F32R, name="invD128")
        nc.vector.tensor_copy(out=invD128, in_=stg[:, 2:3])
        ones256 = const.tile([1, HQ], F32R, name="ones256")
        nc.vector.memset(indstg[0:1, 128:128 + HQ], 1.0)
        nc.vector.tensor_copy(out=ones256, in_=indstg[0:1, 128:128 + HQ])
        eps_sb = const.tile([1, 1], F32, name="eps_sb")
        nc.vector.memset(eps_sb, EPS)
        actwarm = const.tile([1, 1], F32, name="actwarm")
        nc.scalar.activation(actwarm, eps_sb, AF.Exp)
        # startup queue: phase-A operands first, w1 stream behind
        nc.sync.dma_start(out=wk_sb[:, :, 0:256], in_=wk_d[:, :, 0:256])
        nc.sync.dma_start(out=xk_sb, in_=xkT_d[:])
        nc.sync.dma_start(out=wv_sb[:, :, 0:512], in_=wv_d[:, :, 0:512])
        nc.sync.dma_start(out=wk_sb[:, :, 256:D], in_=wk_d[:, :, 256:D])
        nc.sync.dma_start(out=wv_sb[:, :, 512:D], in_=wv_d[:, :, 512:D])
        nc.sync.dma_start(out=wo_sb, in_=wo_d[:])
        nc.sync.dma_start(out=xqf_sb, in_=xqTf_d[:])

        nc.vector.memset(
            V3.rearrange("p c (h w) -> p (c h) w", w=VW)[:, :, DH:DH + 1], 1.0)

        # ---------------- phase 0: Q projection (fp8 DoubleRow) ----------------
        for tb in _blocks(ND, 2):
            ps = {}
            for t in tb:
                ps[t] = mmp.tile([128, QS], F32, tag="mm", name=f"qtps{t}")
            for k in range(ND // 2):
                for t in tb:
                    nc.tensor.matmul(ps[t],
                                     lhsT=wq_sb[:, 2 * k:2 * k + 2, t * 128:(t + 1) * 128],
                                     rhs=xq_sb[:, 2 * k:2 * k + 2, :],
                                     start=(k == 0), stop=(k == ND // 2 - 1),
                                     perf_mode=PM.DoubleRow)
            for t in tb:
                nc.vector.tensor_scalar(out=QT[:, t, :], in0=ps[t], scalar1=IWS,
                                        scalar2=bq_sb[:, t:t + 1],
                                        op0=OP.mult, op1=OP.add)

        # ---------------- attention-era task builders ----------------
        def task_kt(t, sc4):
            def run():
                ps = mmp.tile([128, 512], F32, tag="mm", name=f"ktps{t}_{sc4}")
                for k in range(ND // 2):
                    nc.tensor.matmul(ps,
                                     lhsT=wk_sb[:, 2 * k:2 * k + 2, t * 128:(t + 1) * 128],
                                     rhs=xk_sb[:, 2 * k:2 * k + 2, sc4 * 512:(sc4 + 1) * 512],
                                     start=(k == 0), stop=(k == ND // 2 - 1),
                                     perf_mode=PM.DoubleRow)
                nc.vector.tensor_scalar(
                    out=KT[:, t, sc4 * 512:(sc4 + 1) * 512], in0=ps,
                    scalar1=IWS, scalar2=bk_sb[:, t:t + 1], op0=OP.mult, op1=OP.add)
            return run

        def task_v(nh, c):
            def run():
                ps = mmp.tile([128, 512], F32, tag="mm", name=f"vps{nh}_{c}")
                if use_bv:
                    nc.tensor.matmul(ps, lhsT=ones1b,
                                     rhs=bv_sb[:, nh * 512:(nh + 1) * 512],
                                     start=True, stop=False)
                for k in range(ND // 2):
                    nc.tensor.matmul(ps,
                                     lhsT=xk_sb[:, 2 * k:2 * k + 2, c * 128:(c + 1) * 128],
                                     rhs=wv_sb[:, 2 * k:2 * k + 2, nh * 512:(nh + 1) * 512],
                                     start=(k == 0 and not use_bv),
                                     stop=(k == ND // 2 - 1),
                                     perf_mode=PM.DoubleRow)
                out_ap = V3[:, c, :].rearrange("p (h w) -> p h w", w=VW)[:, 8 * nh:8 * nh + 8, 0:DH]
                nc.vector.tensor_scalar_mul(
                    out=out_ap, in0=ps.rearrange("p (h w) -> p h w", w=DH), scalar1=IWS)
            return run

        # ---------------- attention half (pairs over one query half) ---------
        def attention_half(half, tasks, horizon=64):
            q0 = half * HQ
            qsl = slice(q0, q0 + HQ)
            tailB_pending = []

            def emit_tailB(p, recE, recO, hTu_t):
                rbc = mmp.tile([128, HQ], F32, tag="mm", name=f"rbc{half}_{p}")
                nc.tensor.matmul(rbc, lhsT=indE, rhs=recE, start=True, stop=False)
                nc.tensor.matmul(rbc, lhsT=indO, rhs=recO, start=False, stop=True)
                nc.vector.tensor_mul(out=hT8[:, p, qsl], in0=hTu_t, in1=rbc)

            ntasks = len(tasks)
            ti = 0
            for p in range(NP):
                t = p
                av = {}
                av[0] = avp.tile([VW, HQ], F32, tag="av", name=f"av{half}_{p}e")
                av[1] = avp.tile([VW, HQ], F32, tag="av", name=f"av{half}_{p}o")
                pend = []

                def emit_av(i, et):
                    for parity in (0, 1):
                        h = 2 * p + parity
                        nc.tensor.matmul(av[parity],
                                         lhsT=V3[:, 2 * i:2 * i + 2, h * VW:(h + 1) * VW],
                                         rhs=et[:, parity, :, :],
                                         start=(i == 0), stop=(i == 7),
                                         perf_mode=PM.DoubleRow)

                for i in range(8):
                    sct = scp.tile([128, 2, 2, HQ], F32, tag="sc",
                                   name=f"sc{half}_{p}_{i}")
                    for cc in (0, 1):
                        ch = 2 * i + cc
                        for parity in (0, 1):
                            base = 64 * parity
                            nc.tensor.matmul(sct[:, parity, cc, :],
                                             lhsT=KT[base:base + 64, t, ch * 128:(ch + 1) * 128],
                                             rhs=QT[base:base + 64, t, qsl],
                                             start=True, stop=True)
                    et = expp.tile([128, 2, 2, HQ], F8E4, tag="exp",
                                   name=f"ex{half}_{p}_{i}")
                    if use_mask:
                        for cc in (0, 1):
                            ch = 2 * i + cc
                            nc.scalar.activation(et[:, :, cc, :], sct[:, :, cc, :],
                                                 AF.Exp, bias=madd_sb[:, ch:ch + 1],
                                                 scale=0.125)
                    else:
                        nc.scalar.activation(et, sct, AF.Exp, scale=0.125)
                    pend.append((i, et))
                    if len(pend) > 1:
                        emit_av(*pend.pop(0))
                    if i == 4 and tailB_pending:
                        emit_tailB(*tailB_pending.pop(0))
                    hi = (ntasks * min(p * 8 + i + 1, horizon)) // horizon
                    while ti < hi:
                        tasks[ti]()
                        ti += 1
                while pend:
                    emit_av(*pend.pop(0))

                denE = recipp.tile([1, HQ], F32, tag="den", bufs=2, name=f"denE{half}_{p}")
                denO = recipp.tile([1, HQ], F32, tag="den", bufs=2, name=f"denO{half}_{p}")
                recEf = recipp.tile([1, HQ], F32, tag="recf", bufs=2, name=f"recEf{half}_{p}")
                recOf = recipp.tile([1, HQ], F32, tag="recf", bufs=2, name=f"recOf{half}_{p}")
                recE = recipp.tile([1, HQ], F32R, tag="rec", bufs=4, name=f"recE{half}_{p}")
                recO = recipp.tile([1, HQ], F32R, tag="rec", bufs=4, name=f"recO{half}_{p}")
                hTu_t = recipp.tile([128, HQ], BF16, tag="htu", bufs=3,
                                    name=f"hTu{half}_{p}")
                if p == NP - 1:
                    nc.scalar.copy(denE, av[0][64:65, :])
                    nc.scalar.copy(denO, av[1][64:65, :])
                    nc.scalar.copy(hTu_t[0:64, :], av[0][0:64, :])
                    nc.scalar.copy(hTu_t[64:128, :], av[1][0:64, :])
                else:
                    nc.vector.tensor_copy(out=hTu_t[0:64, :], in_=av[0][0:64, :])
                    nc.vector.tensor_copy(out=hTu_t[64:128, :], in_=av[1][0:64, :])
                    nc.vector.tensor_copy(out=denE, in_=av[0][64:65, :])
                    nc.vector.tensor_copy(out=denO, in_=av[1][64:65, :])
                nc.vector.reciprocal_approx_fast(out=recEf, in_=denE)
                nc.vector.reciprocal_approx_fast(out=recOf, in_=denO)
                nc.vector.tensor_copy(out=recE, in_=recEf)
                nc.vector.tensor_copy(out=recO, in_=recOf)
                tailB_pending.append((p, recE, recO, hTu_t))
            while tailB_pending:
                emit_tailB(*tailB_pending.pop(0))
            while ti < ntasks:
                tasks[ti]()
                ti += 1

        # ---------------- FFN emission (per query half) ----------------
        # Matmul psums flow through the mm ring of the given pool; LN stats
        # accumulate on DVE into SBUF (no long-lived psum).
        def make_ffn_tasks(half, mm_pool, lnsb, wst, resid, h1f, h1bf, gTl):
            q0 = half * HQ
            qsl = slice(q0, q0 + HQ)
            tg = f"h{half}"
            tasks = []
            st1 = {}
            st2 = {}

            def ln_accum(st, tagn, c, src_f32r, src_name):
                stp = mm_pool.tile([1, HQ], F32, tag="mm",
                                   name=f"st{tagn}_{src_name}_{c}")
                nc.tensor.matmul(stp, lhsT=invD128, rhs=src_f32r,
                                 start=True, stop=True)
                if c == 0:
                    acc = lnsb.tile([1, HQ], F32, tag=f"acc_{src_name}", bufs=2,
                                    name=f"acc{tagn}_{src_name}")
                    st[src_name] = acc
                    nc.vector.tensor_copy(out=acc, in_=stp)
                else:
                    nc.vector.tensor_add(out=st[src_name], in0=st[src_name], in1=stp)

            def ln_chunk(st, tagn, c):
                sq = lnsb.tile([128, HQ], F32R, tag="sq", bufs=2, name=f"sq{tagn}_{c}")
                nc.vector.tensor_mul(out=sq, in0=resid[:, c, :], in1=resid[:, c, :])
                ln_accum(st, tagn, c, resid[:, c, :], "s")
                ln_accum(st, tagn, c, sq, "q")

            def ln_finish(st, g_sb, be_sb, writer, tagn):
                u = st["s"]
                var = lnsb.tile([1, HQ], F32, tag="var", bufs=2, name=f"var_{tagn}")
                std = lnsb.tile([1, HQ], F32, tag="std", bufs=2, name=f"std_{tagn}")
                avecf = lnsb.tile([1, HQ], F32, tag="avecf", bufs=2, name=f"avecf_{tagn}")
                avec = lnsb.tile([1, HQ], F32R, tag="avec", bufs=2, name=f"avec_{tagn}")
                cvec = lnsb.tile([1, HQ], F32R, tag="cvec", bufs=2, name=f"cvec_{tagn}")
                nc.vector.scalar_tensor_tensor(out=var, in0=u, scalar=-1.0, in1=u,
                                               op0=OP.mult, op1=OP.mult)
                nc.vector.tensor_add(out=var, in0=st["q"], in1=var)
                nc.scalar.activation(std, var, AF.Sqrt, bias=eps_sb, scale=1.0)
                nc.vector.reciprocal_approx_fast(out=avecf, in_=std)
                nc.vector.tensor_copy(out=avec, in_=avecf)
                nc.vector.scalar_tensor_tensor(out=cvec, in0=u, scalar=-1.0,
                                               in1=avecf, op0=OP.mult, op1=OP.mult)

                def apply_chunk(c):
                    def run():
                        abc = mm_pool.tile([128, HQ], F32, tag="mm",
                                           name=f"abc{tagn}_{c}")
                        nc.tensor.matmul(abc, lhsT=g_sb[0:1, c * 128:(c + 1) * 128],
                                         rhs=avec, start=True, stop=True)
                        cbc = mm_pool.tile([128, HQ], F32, tag="mm",
                                           name=f"cbc{tagn}_{c}")
                        nc.tensor.matmul(cbc, lhsT=g_sb[0:1, c * 128:(c + 1) * 128],
                                         rhs=cvec, start=True, stop=False)
                        nc.tensor.matmul(cbc, lhsT=be_sb[0:1, c * 128:(c + 1) * 128],
                                         rhs=ones256, start=False, stop=True)
                        tmp = lnsb.tile([128, HQ], F32, tag="tmp", bufs=3,
                                        name=f"lnt{tagn}_{c}")
                        nc.vector.tensor_mul(out=tmp, in0=resid[:, c, :], in1=abc)
                        writer(c, tmp, cbc)
                    return run
                return [apply_chunk(c) for c in range(ND)]

            # --- o-projection + resid1 + LN1 stats, blocks of 2 ---
            def oproj_block(jb):
                def run():
                    ps = {j: mm_pool.tile([128, HQ], F32, tag="mm",
                                          name=f"op{tg}_{j}") for j in jb}
                    for k in range(ND // 2):
                        for j in jb:
                            nc.tensor.matmul(ps[j],
                                             lhsT=wo_sb[:, 2 * k:2 * k + 2, j * 128:(j + 1) * 128],
                                             rhs=hT8[:, 2 * k:2 * k + 2, qsl],
                                             start=(k == 0), stop=(k == ND // 2 - 1),
                                             perf_mode=PM.DoubleRow)
                    for j in jb:
                        nc.vector.scalar_tensor_tensor(
                            out=resid[:, j, :], in0=ps[j], scalar=IWS2,
                            in1=xqf_sb[:, j, qsl], op0=OP.mult, op1=OP.add)
                        ln_chunk(st1, f"a{tg}", j)
                return run
            for jb in _blocks(ND, 2):
                tasks.append(oproj_block(jb))

            def ln1_writer(c, tmp, cbc):
                nc.vector.tensor_add(out=h1f[:, c, :], in0=tmp, in1=cbc)
                nc.vector.tensor_copy(out=h1bf[:, c, :], in_=h1f[:, c, :])

            holder = {}

            def ln1_fin():
                holder["ap1"] = ln_finish(st1, g1_sb, be1_sb, ln1_writer, f"a{tg}")
            tasks.append(ln1_fin)

            def ln1_apply(i):
                def run():
                    holder["ap1"][2 * i]()
                    holder["ap1"][2 * i + 1]()
                return run
            for i in range(ND // 2):
                tasks.append(ln1_apply(i))

            # --- fc1 + gelu, one out-chunk per task; w1 streamed 2 ahead ---
            w1tiles = {}

            def w1_prefetch(jb):
                def run():
                    w1tiles[jb] = wst.tile([128, ND, 512], BF16, tag="wst",
                                           name=f"w1t{tg}_{jb}")
                    nc.gpsimd.dma_start(out=w1tiles[jb], in_=w1_d[:, jb, :, :])
                return run

            def fc1_chunk(jg):
                def run():
                    jb, j = jg // 4, jg % 4
                    w1t = w1tiles[jb]
                    ps = mm_pool.tile([128, HQ], F32, tag="mm", name=f"f1{tg}_{jg}")
                    for k in range(ND):
                        nc.tensor.matmul(ps, lhsT=w1t[:, k, j * 128:(j + 1) * 128],
                                         rhs=h1bf[:, k, :],
                                         start=(k == 0), stop=(k == ND - 1))
                    nc.scalar.activation(gTl[:, jg, :], ps, AF.Gelu,
                                         bias=bf1_sb[:, jg:jg + 1], scale=1.0)
                    if j == 3:
                        del w1tiles[jb]
                return run
            tasks.append(w1_prefetch(0))
            tasks.append(w1_prefetch(1))
            for jg in range(NF):
                tasks.append(fc1_chunk(jg))
                if jg % 4 == 0 and jg // 4 + 2 < 8:
                    tasks.append(w1_prefetch(jg // 4 + 2))

            # --- fc2 + resid2 + LN2 stats; w2 streamed as per-chunk-pair
            # blocks so each task fully drains its psums (2-slot ring safe) ---
            w2tiles = {}

            def w2_prefetch(jp):
                def run():
                    for kh in (0, 1):
                        w2tiles[(jp, kh)] = wst.tile([128, 16, HQ], BF16, tag="wst",
                                                     name=f"w2t{tg}_{jp}_{kh}")
                        nc.gpsimd.dma_start(out=w2tiles[(jp, kh)],
                                            in_=w2_d[:, jp, kh, :, :])
                return run

            def fc2_pair(jp):
                def run():
                    if jp + 1 < 4:
                        w2_prefetch(jp + 1)()
                    ps = {j: mm_pool.tile([128, HQ], F32, tag="mm",
                                          name=f"f2{tg}_{jp}_{j}")
                          for j in (0, 1)}
                    for kh in (0, 1):
                        w2t = w2tiles.pop((jp, kh))
                        for k16 in range(16):
                            kk = kh * 16 + k16
                            for j in (0, 1):
                                nc.tensor.matmul(
                                    ps[j], lhsT=w2t[:, k16, j * 128:(j + 1) * 128],
                                    rhs=gTl[:, kk, :],
                                    start=(kk == 0), stop=(kk == NF - 1))
                    for j in (0, 1):
                        jg = 2 * jp + j
                        if use_bf2:
                            tmp2 = lnsb.tile([128, HQ], F32, tag="tmp", bufs=3,
                                             name=f"f2t{tg}_{jg}")
                            nc.vector.tensor_scalar(out=tmp2, in0=ps[j], scalar1=1.0,
                                                    scalar2=bf2_sb[:, jg:jg + 1],
                                                    op0=OP.mult, op1=OP.add)
                            nc.vector.tensor_add(out=resid[:, jg, :], in0=tmp2,
                                                 in1=h1f[:, jg, :])
                        else:
                            nc.vector.scalar_tensor_tensor(
                                out=resid[:, jg, :], in0=ps[j], scalar=1.0,
                                in1=h1f[:, jg, :], op0=OP.mult, op1=OP.add)
                        ln_chunk(st2, f"b{tg}", jg)
                return run

            tasks.append(w2_prefetch(0))
            for jp in range(4):
                tasks.append(fc2_pair(jp))

            def ln2_writer(c, tmp, cbc):
                och = lnsb.tile([128, HQ], F32, tag="out", bufs=2,
                                name=f"och{tg}_{c}")
                nc.vector.tensor_add(out=och, in0=tmp, in1=cbc)
                nc.sync.dma_start(out=yT_d[c * 128:(c + 1) * 128, qsl], in_=och)

            def ln2_fin():
                holder["ap2"] = ln_finish(st2, g2_sb, be2_sb, ln2_writer, f"b{tg}")
            tasks.append(ln2_fin)

            def ln2_apply(i):
                def run():
                    holder["ap2"][2 * i]()
                    holder["ap2"][2 * i + 1]()
                return run
            for i in range(ND // 2):
                tasks.append(ln2_apply(i))
            return tasks

        # LN gamma/beta as f32r operands
        g1_sb = ffx.tile([1, D], F32R, name="g1_sb")
        be1_sb = ffx.tile([1, D], F32R, name="be1_sb")
        g2_sb = ffx.tile([1, D], F32R, name="g2_sb")
        be2_sb = ffx.tile([1, D], F32R, name="be2_sb")
        for dd, dst in ((g1_d, g1_sb), (be1_d, be1_sb), (g2_d, g2_sb),
                        (be2_d, be2_sb)):
            gbe_stage = ffx.tile([1, D], F32, tag="gbes", bufs=1, name="gbe_stage")
            nc.sync.dma_start(out=gbe_stage, in_=dd.rearrange("(a d) -> a d", a=1))
            nc.vector.tensor_copy(out=dst, in_=gbe_stage)

        residA = ffx.tile([128, ND, HQ], F32R, name="residA")
        h1fA = ffx.tile([128, ND, HQ], F32, name="h1fA")
        h1bfA = ffx.tile([128, ND, HQ], BF16, name="h1bfA")
        gTA = ffx.tile([128, NF, HQ], BF16, name="gTA")

        # ---------------- era A: attention half 0 + K/V production ----------
        kv_tasks = []
        for t in (0, 1):
            for sc4 in range(4):
                kv_tasks.append(task_kt(t, sc4))
        for c in range(NK):
            kv_tasks.append(task_v(0, c))
        for p in range(6):
            for sc4 in range(4):
                kv_tasks.append(task_kt(p + 2, sc4))
            if p < 4:
                for c in range(4 * p, 4 * p + 4):
                    kv_tasks.append(task_v(1, c))
        upfront, rest = kv_tasks[:24], kv_tasks[24:]
        for task in upfront:
            task()
        attention_half(0, rest, horizon=32)
        xkp_cm.close()

        # ---------------- era B: attention half 1 + FFN half 0 --------------
        ffnA = make_ffn_tasks(0, mmp, lnsbA, wstA, residA, h1fA, h1bfA, gTA)
        nc.scalar.activation(actwarm, eps_sb, AF.Sqrt)
        attention_half(1, ffnA)

        dump("KT", KT)
        dump("V3", V3)
        dump("QT", QT)
        dump("hT8", hT8)
        dump("h1fA", h1fA)
        dump("gTA", gTA)
        dump("residA", residA)
        # ---------------- era C: FFN half 1 ----------------
        att2.close()
        attbig_cm.close()
        att_ps.close()
        ffn_cm = ExitStack()
        ffn2 = ffn_cm.enter_context(tc.tile_pool(name="ffn2", bufs=1))
        pmm = ffn_cm.enter_context(tc.tile_pool(name="pmm", bufs=4, space="PSUM"))
        lnsbB = ffn_cm.enter_context(tc.tile_pool(name="lnsbB", bufs=2))
        wstB = ffn_cm.enter_context(tc.tile_pool(name="wstB", bufs=2))
        residB = ffn2.tile([128, ND, HQ], F32R, name="residB")
        h1fB = ffn2.tile([128, ND, HQ], F32, name="h1fB")
        h1bfB = ffn2.tile([128, ND, HQ], BF16, name="h1bfB")
        gTB = ffn2.tile([128, NF, HQ], BF16, name="gTB")
        ffnB = make_ffn_tasks(1, pmm, lnsbB, wstB, residB, h1fB, h1bfB, gTB)
        for task in ffnB:
            task()
        dump("h1fB", h1fB)
        dump("gTB", gTB)
        ffn_cm.close()

    nc.compile()
    return nc


_CACHE = {}


def _get_built(use_mask, use_bv, use_bf2=False, dbg=False):
    key = (use_mask, use_bv, use_bf2, dbg)
    if key not in _CACHE:
        _CACHE[key] = _build(use_mask, use_bv, use_bf2, dbg)
    return _CACHE[key]


def kernel(x, mask, wq, bq, wk, bk, wv, bv, wo, bo, g1, be1, w1, bf1, w2, bf2, g2, be2):
    bf = ml_dtypes.bfloat16
    e4 = ml_dtypes.float8_e4m3
    f4 = np.float32
    x = np.asarray(x, f4)
    madd_full = (-10000.0 * (1.0 - np.asarray(mask).astype(f4)))  # [B, S]
    use_mask = bool((madd_full != 0.0).any())
    use_bv = bool(np.any(np.asarray(bv) != 0))
    use_bf2 = bool(np.any(np.asarray(bf2) != 0))
    nc = _get_built(use_mask, use_bv, use_bf2)

    def tile_w(w, dt, scale=1.0):
        # [D, N] -> [128, D/128, N]
        w = (np.asarray(w, f4) * scale).astype(dt)
        return np.ascontiguousarray(w.reshape(-1, 128, w.shape[1]).transpose(1, 0, 2))

    w1h = np.asarray(w1, f4).astype(bf).reshape(ND, 128, 8, 512).transpose(1, 2, 0, 3)
    w2h = np.asarray(w2, f4).astype(bf).reshape(2, 16, 128, 4, 256).transpose(2, 3, 0, 1, 4)
    shared = {
        "wq": tile_w(wq, e4, WS),
        "wk": tile_w(wk, e4, WS),
        "wv": tile_w(wv, e4, WS),
        "wo": tile_w(wo, e4, WS),
        "w1": np.ascontiguousarray(w1h),
        "w2": np.ascontiguousarray(w2h),
        "bq": np.asarray(bq, f4), "bk": np.asarray(bk, f4),
        "bf1": np.asarray(bf1, f4),
        "g1": np.asarray(g1, f4),
        "be1": np.asarray(be1, f4), "g2": np.asarray(g2, f4),
        "be2": np.asarray(be2, f4),
    }
    if use_bv:
        shared["bv"] = (np.asarray(bv, f4) * WS).astype(bf)
    if use_bf2:
        shared["bf2"] = np.asarray(bf2, f4)

    # [D, S] -> [128, ND, S] pre-tiled transposes
    bo_f = np.asarray(bo, f4)
    xTt = {b: np.ascontiguousarray(
        x[b].T.reshape(ND, 128, S).transpose(1, 0, 2)) for b in range(B)}
    xTt_8 = {b: xTt[b].astype(e4) for b in range(B)}
    bo_t = bo_f.reshape(ND, 128).T[:, :, None]          # [128, ND, 1]
    in_maps = []
    for c in range(NCORES):
        b, q0 = c // 4, (c % 4) * QS
        m = dict(shared)
        m["xkT"] = xTt_8[b]
        m["xqT"] = np.ascontiguousarray(xTt_8[b][:, :, q0:q0 + QS])
        m["xqTf"] = np.ascontiguousarray(xTt[b][:, :, q0:q0 + QS] + bo_t).astype(bf)
        if use_mask:
            m["madd"] = np.ascontiguousarray(madd_full[b])
        in_maps.append(m)

    res = run_bass_kernel_spmd(nc, in_maps, core_ids=list(range(NCORES)))
    kernel.last_result = res
    if res.exec_time_ns is not None:
        print(f"HW exec time: {res.exec_time_ns} ns")

    y = np.empty((B, S, D), np.float32)
    for c in range(NCORES):
        b, q0 = c // 4, (c % 4) * QS
        y[b, q0:q0 + QS, :] = np.asarray(res.results[c]["yT"], np.float32).T
    return y


# revision 26
# speedup vs baseline: 1.0086x; 1.0086x over previous
"""Trainium2 Bass/Tile kernel for a dense transformer block.

B=2, S=2048, D=1024, H=16 heads (dh=64), FF=4096, f32 IO.

Sharding: 8 cores = (2 batches) x (4 query-slices of 512 tokens), zero
cross-core communication (K/V recomputed per core).

v3: fp8 attention + split-query software pipeline.  All attention GEMMs
(Q/K/V/O projections, AV) are fp8e4m3 DoubleRow matmuls (0.5 PE
cycles/row, weights host-scaled by 64); scores run on fp8 Q/K (the
1/sqrt(dh) folds into the softmax exp scale); exp writes fp8 directly.
The FFN stays bf16 (fp8 FFN breaches the 2e-2 gate).  LayerNorm stats /
broadcast matmuls use float32r operands (1 cycle/row vs 4 for fp32).

The queries are processed in two 256-wide halves: era A runs attention
for half 0 (plus all K/V production), era B runs attention for half 1
with the ENTIRE half-0 FFN interleaved into its pair loop (softmax exp
keeps ACT busy while the FFN matmuls fill the PE), era C finishes with
the half-1 FFN.  LN statistics accumulate via DVE adds into SBUF so the
interleaved-FFN matmuls can share a single 2-slot PSUM ring with the
reciprocal broadcasts.
"""

import os
from contextlib import ExitStack

import numpy as np
import ml_dtypes

import concourse.bass as bass
import concourse.tile as tile
from concourse import bacc, mybir
from concourse.bass_utils import run_bass_kernel_spmd

BF16 = mybir.dt.bfloat16
F32 = mybir.dt.float32
F32R = mybir.dt.float32r
F8E4 = mybir.dt.float8e4
AF = mybir.ActivationFunctionType
OP = mybir.AluOpType
PM = mybir.MatmulPerfMode

B, S, D, H, FF = 2, 2048, 1024, 16, 4096
DH = D // H            # 64
NCORES = 8
QS = S // 4            # 512 queries per core
HQ = QS // 2           # 256-query pipeline half
NK = S // 128          # 16 key chunks
ND = D // 128          # 8 feature chunks
NF = FF // 128         # 32 ff chunks
NP = H // 2            # 8 head pairs
VW = DH + 1            # 65 = head width + ones column
EPS = 1e-12
WS = 64.0              # host-side fp8 weight scale
IWS = 1.0 / WS
IWS2 = IWS * IWS


def _blocks(n, w):
    return [list(range(i, min(i + w, n))) for i in range(0, n, w)]


def _build(use_mask, use_bv, use_bf2, dbg=False):
    nc = bacc.Bacc("TRN2", target_bir_lowering=False, debug=False)

    def din(name, shape, dtype):
        return nc.dram_tensor(name, shape, dtype, kind="ExternalInput").ap()

    def dump(name, tl):
        if not dbg:
            return
        dd = nc.dram_tensor(f"dbg_{name}", list(tl.shape), tl.dtype,
                            kind="ExternalOutput").ap()
        nc.sync.dma_start(out=dd, in_=tl)

    xkT_d = din("xkT", [128, ND, S], F8E4)
    xqT_d = din("xqT", [128, ND, QS], F8E4)
    xqTf_d = din("xqTf", [128, ND, QS], BF16)     # x slice, +bo folded in
    wq_d = din("wq", [128, ND, D], F8E4)          # pre-scaled by WS on host
    wk_d = din("wk", [128, ND, D], F8E4)
    wv_d = din("wv", [128, ND, D], F8E4)
    wo_d = din("wo", [128, ND, D], F8E4)
    w1_d = din("w1", [128, 8, ND, 512], BF16)     # [p, jb, k, n]
    w2_d = din("w2", [128, 4, 2, 16, 256], BF16)  # [p, jpair, kh, k16, n]
    bq_d = din("bq", [D], F32)
    bk_d = din("bk", [D], F32)
    bf1_d = din("bf1", [FF], F32)
    g1_d = din("g1", [D], F32)
    be1_d = din("be1", [D], F32)
    g2_d = din("g2", [D], F32)
    be2_d = din("be2", [D], F32)
    bf2_d = din("bf2", [D], F32) if use_bf2 else None
    bv_d = din("bv", [D], BF16) if use_bv else None     # pre-scaled by WS
    madd_d = din("madd", [S], F32) if use_mask else None
    yT_d = nc.dram_tensor("yT", [D, QS], F32, kind="ExternalOutput").ap()

    with tile.TileContext(nc) as tc, ExitStack() as glob:
        const = glob.enter_context(tc.tile_pool(name="const", bufs=1))
        gx = glob.enter_context(tc.tile_pool(name="gx", bufs=1))
        ffx = glob.enter_context(tc.tile_pool(name="ffx", bufs=1))
        att_ps = ExitStack()
        avp = att_ps.enter_context(tc.tile_pool(name="avp", bufs=2, space="PSUM"))
        mmp = att_ps.enter_context(tc.tile_pool(name="mmp", bufs=2, space="PSUM"))
        scp = att_ps.enter_context(tc.tile_pool(name="scp", bufs=2, space="PSUM"))

        # ---------------- big tiles + front-loaded DMAs ----------------
        hT8 = gx.tile([128, ND, QS], F8E4, name="hT8")          # 64*h, attn out
        wo_sb = gx.tile([128, ND, D], F8E4, name="wo_sb")
        xqf_sb = gx.tile([128, ND, QS], BF16, name="xqf_sb")
        attbig_cm = ExitStack()
        attbig = attbig_cm.enter_context(tc.tile_pool(name="attbig", bufs=1))
        KT = attbig.tile([128, ND, S], F8E4, name="KT")         # fp8(k + bk)
        V3 = attbig.tile([128, NK, H * VW], F8E4, name="V3")    # V + ones cols
        QT = attbig.tile([128, ND, QS], F8E4, name="QT")
        att2 = ExitStack()
        recipp = att2.enter_context(tc.tile_pool(name="recipp", bufs=2))
        expp = att2.enter_context(tc.tile_pool(name="expp", bufs=6))
        lnsbA = att2.enter_context(tc.tile_pool(name="lnsbA", bufs=2))
        wstA = att2.enter_context(tc.tile_pool(name="wstA", bufs=2))
        xkp_cm = ExitStack()
        xkp = xkp_cm.enter_context(tc.tile_pool(name="xkp", bufs=1))
        xk_sb = xkp.tile([128, ND, S], F8E4, name="xk_sb")
        wk_sb = xkp.tile([128, ND, D], F8E4, name="wk_sb")
        wv_sb = xkp.tile([128, ND, D], F8E4, name="wv_sb")
        wq_sb = wstA.tile([128, ND, D], F8E4, tag="wst", name="wq_sb")
        xq_sb = wstA.tile([128, ND, QS], F8E4, tag="wst", name="xq_sb")
        nc.sync.dma_start(out=xq_sb, in_=xqT_d[:])
        nc.sync.dma_start(out=wq_sb[:, :, 0:512], in_=wq_d[:, :, 0:512])
        nc.sync.dma_start(out=wq_sb[:, :, 512:D], in_=wq_d[:, :, 512:D])
        # ---------------- constants & small params ----------------
        bq_sb = const.tile([128, ND], F32, name="bq_sb")
        nc.sync.dma_start(out=bq_sb, in_=bq_d.rearrange("(c p) -> p c", p=128))
        bk_sb = const.tile([128, ND], F32, name="bk_sb")
        nc.sync.dma_start(out=bk_sb, in_=bk_d.rearrange("(c p) -> p c", p=128))
        bf1_sb = const.tile([128, NF], F32, name="bf1_sb")
        nc.sync.dma_start(out=bf1_sb, in_=bf1_d.rearrange("(c p) -> p c", p=128))
        if use_bf2:
            bf2_sb = const.tile([128, ND], F32, name="bf2_sb")
            nc.sync.dma_start(out=bf2_sb, in_=bf2_d.rearrange("(c p) -> p c", p=128))
        if use_mask:
            madd_sb = const.tile([128, NK], F32, name="madd_sb")
            nc.sync.dma_start(out=madd_sb, in_=madd_d.rearrange("(c p) -> p c", p=128))
        if use_bv:
            bv_sb = const.tile([1, D], BF16, name="bv_sb")
            nc.sync.dma_start(out=bv_sb, in_=bv_d.rearrange("(a d) -> a d", a=1))
            ones1b = const.tile([1, 128], BF16, name="ones1b")
            nc.vector.memset(ones1b, 1.0)
        # reciprocal broadcast selectors carry the 64x for the fp8 hT scale.
        # (memset can't write f32r directly; stage in f32 and DVE-round.)
        stg = const.tile([128, 4], F32, name="stg")
        nc.vector.memset(stg[:, 2:3], 1.0 / D)
        indstg = const.tile([1, 128 + HQ], F32, name="indstg")
        nc.vector.memset(indstg, 0.0)
        nc.vector.memset(indstg[0:1, 0:64], WS)
        indE = const.tile([1, 128], F32R, name="indE")
        nc.vector.tensor_copy(out=indE, in_=indstg[0:1, 0:128])
        nc.vector.memset(indstg[0:1, 0:64], 0.0)
        nc.vector.memset(indstg[0:1, 64:128], WS)
        indO = const.tile([1, 128], F32R, name="indO")
        nc.vector.tensor_copy(out=indO, in_=indstg[0:1, 0:128])
        invD128 = const.tile([128, 1], F32R, name="invD128")
        nc.vector.tensor_copy(out=invD128, in_=stg[:, 2:3])
        ones256 = const.tile([1, HQ], F32R, name="ones256")
        nc.vector.memset(indstg[0:1, 128:128 + HQ], 1.0)
        nc.vector.tensor_copy(out=ones256, in_=indstg[0:1, 128:128 + HQ])
        eps_sb = const.tile([1, 1], F32, name="eps_sb")
        nc.vector.memset(eps_sb, EPS)
        actwarm = const.tile([1, 1], F32, name="actwarm")
        nc.scalar.activation(actwarm, eps_sb, AF.Exp)
        # startup queue: phase-A operands first, w1 stream behind
        nc.sync.dma_start(out=wk_sb[:, :, 0:256], in_=wk_d[:, :, 0:256])
        nc.sync.dma_start(out=xk_sb, in_=xkT_d[:])
        nc.sync.dma_start(out=wv_sb[:, :, 0:512], in_=wv_d[:, :, 0:512])
        nc.sync.dma_start(out=wk_sb[:, :, 256:D], in_=wk_d[:, :, 256:D])
        nc.sync.dma_start(out=wv_sb[:, :, 512:D], in_=wv_d[:, :, 512:D])
        nc.sync.dma_start(out=wo_sb, in_=wo_d[:])
        nc.sync.dma_start(out=xqf_sb, in_=xqTf_d[:])

        nc.vector.memset(
            V3.rearrange("p c (h w) -> p (c h) w", w=VW)[:, :, DH:DH + 1], 1.0)

        # ---------------- phase 0: Q projection (fp8 DoubleRow) ----------------
        for tb in _blocks(ND, 2):
            ps = {}
            for t in tb:
                ps[t] = mmp.tile([128, QS], F32, tag="mm", name=f"qtps{t}")
            for k in range(ND // 2):
                for t in tb:
                    nc.tensor.matmul(ps[t],
                                     lhsT=wq_sb[:, 2 * k:2 * k + 2, t * 128:(t + 1) * 128],
                                     rhs=xq_sb[:, 2 * k:2 * k + 2, :],
                                     start=(k == 0), stop=(k == ND // 2 - 1),
                                     perf_mode=PM.DoubleRow)
            for t in tb:
                nc.vector.tensor_scalar(out=QT[:, t, :], in0=ps[t], scalar1=IWS,
                                        scalar2=bq_sb[:, t:t + 1],
                                        op0=OP.mult, op1=OP.add)

        # ---------------- attention-era task builders ----------------
        def task_kt(t, sc4):
            def run():
                ps = mmp.tile([128, 512], F32, tag="mm", name=f"ktps{t}_{sc4}")
                for k in range(ND // 2):
                    nc.tensor.matmul(ps,
                                     lhsT=wk_sb[:, 2 * k:2 * k + 2, t * 128:(t + 1) * 128],
                                     rhs=xk_sb[:, 2 * k:2 * k + 2, sc4 * 512:(sc4 + 1) * 512],
                                     start=(k == 0), stop=(k == ND // 2 - 1),
                                     perf_mode=PM.DoubleRow)
                nc.vector.tensor_scalar(
                    out=KT[:, t, sc4 * 512:(sc4 + 1) * 512], in0=ps,
                    scalar1=IWS, scalar2=bk_sb[:, t:t + 1], op0=OP.mult, op1=OP.add)
            return run

        def task_v(nh, c):
            def run():
                ps = mmp.tile([128, 512], F32, tag="mm", name=f"vps{nh}_{c}")
                if use_bv:
                    nc.tensor.matmul(ps, lhsT=ones1b,
                                     rhs=bv_sb[:, nh * 512:(nh + 1) * 512],
                                     start=True, stop=False)
                for k in range(ND // 2):
                    nc.tensor.matmul(ps,
                                     lhsT=xk_sb[:, 2 * k:2 * k + 2, c * 128:(c + 1) * 128],
                                     rhs=wv_sb[:, 2 * k:2 * k + 2, nh * 512:(nh + 1) * 512],
                                     start=(k == 0 and not use_bv),
                                     stop=(k == ND // 2 - 1),
                                     perf_mode=PM.DoubleRow)
                out_ap = V3[:, c, :].rearrange("p (h w) -> p h w", w=VW)[:, 8 * nh:8 * nh + 8, 0:DH]
                nc.vector.tensor_scalar_mul(
                    out=out_ap, in0=ps.rearrange("p (h w) -> p h w", w=DH), scalar1=IWS)
            return run

        # ---------------- attention half (pairs over one query half) ---------
        def attention_half(half, tasks, horizon=64, bucketed=False):
            q0 = half * HQ
            qsl = slice(q0, q0 + HQ)
            tailB_pending = []

            def emit_tailB(p, recE, recO, hTu_t):
                rbc = mmp.tile([128, HQ], F32, tag="mm", name=f"rbc{half}_{p}")
                nc.tensor.matmul(rbc, lhsT=indE, rhs=recE, start=True, stop=False)
                nc.tensor.matmul(rbc, lhsT=indO, rhs=recO, start=False, stop=True)
                nc.vector.tensor_mul(out=hT8[:, p, qsl], in0=hTu_t, in1=rbc)

            ntasks = 0 if bucketed else len(tasks)
            ti = 0
            for p in range(NP):
                if bucketed:
                    ptasks = tasks[p]
                    nt = len(ptasks)
                t = p
                av = {}
                av[0] = avp.tile([VW, HQ], F32, tag="av", name=f"av{half}_{p}e")
                av[1] = avp.tile([VW, HQ], F32, tag="av", name=f"av{half}_{p}o")
                pend = []

                def emit_av(i, et):
                    for parity in (0, 1):
                        h = 2 * p + parity
                        nc.tensor.matmul(av[parity],
                                         lhsT=V3[:, 2 * i:2 * i + 2, h * VW:(h + 1) * VW],
                                         rhs=et[:, parity, :, :],
                                         start=(i == 0), stop=(i == 7),
                                         perf_mode=PM.DoubleRow)

                for i in range(8):
                    sct = scp.tile([128, 2, 2, HQ], F32, tag="sc",
                                   name=f"sc{half}_{p}_{i}")
                    for cc in (0, 1):
                        ch = 2 * i + cc
                        for parity in (0, 1):
                            base = 64 * parity
                            nc.tensor.matmul(sct[:, parity, cc, :],
                                             lhsT=KT[base:base + 64, t, ch * 128:(ch + 1) * 128],
                                             rhs=QT[base:base + 64, t, qsl],
                                             start=True, stop=True)
                    et = expp.tile([128, 2, 2, HQ], F8E4, tag="exp",
                                   name=f"ex{half}_{p}_{i}")
                    if use_mask:
                        for cc in (0, 1):
                            ch = 2 * i + cc
                            nc.scalar.activation(et[:, :, cc, :], sct[:, :, cc, :],
                                                 AF.Exp, bias=madd_sb[:, ch:ch + 1],
                                                 scale=0.125)
                    else:
                        nc.scalar.activation(et, sct, AF.Exp, scale=0.125)
                    pend.append((i, et))
                    if len(pend) > 1:
                        emit_av(*pend.pop(0))
                    if i == 4 and tailB_pending:
                        emit_tailB(*tailB_pending.pop(0))
                    if bucketed:
                        for task in ptasks[(nt * i) // 8:(nt * (i + 1)) // 8]:
                            task()
                    else:
                        hi = (ntasks * min(p * 8 + i + 1, horizon)) // horizon
                        while ti < hi:
                            tasks[ti]()
                            ti += 1
                while pend:
                    emit_av(*pend.pop(0))

                denE = recipp.tile([1, HQ], F32, tag="den", bufs=2, name=f"denE{half}_{p}")
                denO = recipp.tile([1, HQ], F32, tag="den", bufs=2, name=f"denO{half}_{p}")
                recEf = recipp.tile([1, HQ], F32, tag="recf", bufs=2, name=f"recEf{half}_{p}")
                recOf = recipp.tile([1, HQ], F32, tag="recf", bufs=2, name=f"recOf{half}_{p}")
                recE = recipp.tile([1, HQ], F32R, tag="rec", bufs=4, name=f"recE{half}_{p}")
                recO = recipp.tile([1, HQ], F32R, tag="rec", bufs=4, name=f"recO{half}_{p}")
                hTu_t = recipp.tile([128, HQ], BF16, tag="htu", bufs=3,
                                    name=f"hTu{half}_{p}")
                if p == NP - 1:
                    nc.scalar.copy(denE, av[0][64:65, :])
                    nc.scalar.copy(denO, av[1][64:65, :])
                    nc.scalar.copy(hTu_t[0:64, :], av[0][0:64, :])
                    nc.scalar.copy(hTu_t[64:128, :], av[1][0:64, :])
                else:
                    nc.vector.tensor_copy(out=hTu_t[0:64, :], in_=av[0][0:64, :])
                    nc.vector.tensor_copy(out=hTu_t[64:128, :], in_=av[1][0:64, :])
                    nc.vector.tensor_copy(out=denE, in_=av[0][64:65, :])
                    nc.vector.tensor_copy(out=denO, in_=av[1][64:65, :])
                nc.vector.reciprocal_approx_fast(out=recEf, in_=denE)
                nc.vector.reciprocal_approx_fast(out=recOf, in_=denO)
                nc.vector.tensor_copy(out=recE, in_=recEf)
                nc.vector.tensor_copy(out=recO, in_=recOf)
                tailB_pending.append((p, recE, recO, hTu_t))
            while tailB_pending:
                emit_tailB(*tailB_pending.pop(0))
            if not bucketed:
                while ti < ntasks:
                    tasks[ti]()
                    ti += 1

        # ---------------- FFN emission (per query half) ----------------
        # Matmul psums flow through the mm ring of the given pool; LN stats
        # accumulate on DVE into SBUF (no long-lived psum).
        def make_ffn_tasks(half, mm_pool, fc_pool, lnsb, wst, resid, h1f, h1bf, gTl):
            q0 = half * HQ
            qsl = slice(q0, q0 + HQ)
            tg = f"h{half}"
            tasks = []
            st1 = {}
            st2 = {}

            def ln_accum(st, tagn, c, src_f32r, src_name):
                stp = mm_pool.tile([1, HQ], F32, tag="mm",
                                   name=f"st{tagn}_{src_name}_{c}")
                nc.tensor.matmul(stp, lhsT=invD128, rhs=src_f32r,
                                 start=True, stop=True)
                if c == 0:
                    acc = lnsb.tile([1, HQ], F32, tag=f"acc_{src_name}", bufs=2,
                                    name=f"acc{tagn}_{src_name}")
                    st[src_name] = acc
                    nc.vector.tensor_copy(out=acc, in_=stp)
                else:
                    nc.vector.tensor_add(out=st[src_name], in0=st[src_name], in1=stp)

            def ln_chunk(st, tagn, c):
                sq = lnsb.tile([128, HQ], F32R, tag="sq", bufs=2, name=f"sq{tagn}_{c}")
                nc.vector.tensor_mul(out=sq, in0=resid[:, c, :], in1=resid[:, c, :])
                ln_accum(st, tagn, c, resid[:, c, :], "s")
                ln_accum(st, tagn, c, sq, "q")

            def ln_finish(st, g_sb, be_sb, writer, tagn):
                u = st["s"]
                var = lnsb.tile([1, HQ], F32, tag="var", bufs=2, name=f"var_{tagn}")
                std = lnsb.tile([1, HQ], F32, tag="std", bufs=2, name=f"std_{tagn}")
                avecf = lnsb.tile([1, HQ], F32, tag="avecf", bufs=2, name=f"avecf_{tagn}")
                avec = lnsb.tile([1, HQ], F32R, tag="avec", bufs=2, name=f"avec_{tagn}")
                cvec = lnsb.tile([1, HQ], F32R, tag="cvec", bufs=2, name=f"cvec_{tagn}")
                nc.vector.scalar_tensor_tensor(out=var, in0=u, scalar=-1.0, in1=u,
                                               op0=OP.mult, op1=OP.mult)
                nc.vector.tensor_add(out=var, in0=st["q"], in1=var)
                nc.scalar.activation(std, var, AF.Sqrt, bias=eps_sb, scale=1.0)
                nc.vector.reciprocal_approx_fast(out=avecf, in_=std)
                nc.vector.tensor_copy(out=avec, in_=avecf)
                nc.vector.scalar_tensor_tensor(out=cvec, in0=u, scalar=-1.0,
                                               in1=avecf, op0=OP.mult, op1=OP.mult)

                def apply_chunk(c):
                    def run():
                        abc = mm_pool.tile([128, HQ], F32, tag="mm",
                                           name=f"abc{tagn}_{c}")
                        nc.tensor.matmul(abc, lhsT=g_sb[0:1, c * 128:(c + 1) * 128],
                                         rhs=avec, start=True, stop=True)
                        cbc = mm_pool.tile([128, HQ], F32, tag="mm",
                                           name=f"cbc{tagn}_{c}")
                        nc.tensor.matmul(cbc, lhsT=g_sb[0:1, c * 128:(c + 1) * 128],
                                         rhs=cvec, start=True, stop=False)
                        nc.tensor.matmul(cbc, lhsT=be_sb[0:1, c * 128:(c + 1) * 128],
                                         rhs=ones256, start=False, stop=True)
                        tmp = lnsb.tile([128, HQ], F32, tag="tmp", bufs=3,
                                        name=f"lnt{tagn}_{c}")
                        nc.vector.tensor_mul(out=tmp, in0=resid[:, c, :], in1=abc)
                        writer(c, tmp, cbc)
                    return run
                return [apply_chunk(c) for c in range(ND)]

            # --- o-projection + resid1 + LN1 stats, blocks of 2 ---
            def oproj_block(jb):
                def run():
                    ps = {j: mm_pool.tile([128, HQ], F32, tag="mm",
                                          name=f"op{tg}_{j}") for j in jb}
                    for k in range(ND // 2):
                        for j in jb:
                            nc.tensor.matmul(ps[j],
                                             lhsT=wo_sb[:, 2 * k:2 * k + 2, j * 128:(j + 1) * 128],
                                             rhs=hT8[:, 2 * k:2 * k + 2, qsl],
                                             start=(k == 0), stop=(k == ND // 2 - 1),
                                             perf_mode=PM.DoubleRow)
                    for j in jb:
                        nc.vector.scalar_tensor_tensor(
                            out=resid[:, j, :], in0=ps[j], scalar=IWS2,
                            in1=xqf_sb[:, j, qsl], op0=OP.mult, op1=OP.add)
                        ln_chunk(st1, f"a{tg}", j)
                return run
            for jb in _blocks(ND, 2):
                tasks.append(oproj_block(jb))

            def ln1_writer(c, tmp, cbc):
                nc.vector.tensor_add(out=h1f[:, c, :], in0=tmp, in1=cbc)
                nc.vector.tensor_copy(out=h1bf[:, c, :], in_=h1f[:, c, :])

            holder = {}

            def ln1_fin():
                holder["ap1"] = ln_finish(st1, g1_sb, be1_sb, ln1_writer, f"a{tg}")
            tasks.append(ln1_fin)

            def ln1_apply(i):
                def run():
                    holder["ap1"][2 * i]()
                    holder["ap1"][2 * i + 1]()
                return run
            for i in range(ND // 2):
                tasks.append(ln1_apply(i))

            # --- fc1 + gelu, one out-chunk per task; w1 streamed 2 ahead ---
            w1tiles = {}

            def w1_prefetch(jb):
                def run():
                    w1tiles[jb] = wst.tile([128, ND, 512], BF16, tag="wst",
                                           name=f"w1t{tg}_{jb}")
                    nc.gpsimd.dma_start(out=w1tiles[jb], in_=w1_d[:, jb, :, :])
                return run

            def fc1_chunk(jg):
                def run():
                    jb, j = jg // 4, jg % 4
                    w1t = w1tiles[jb]
                    ps = fc_pool.tile([128, HQ], F32, tag="mm", name=f"f1{tg}_{jg}")
                    for k in range(ND):
                        nc.tensor.matmul(ps, lhsT=w1t[:, k, j * 128:(j + 1) * 128],
                                         rhs=h1bf[:, k, :],
                                         start=(k == 0), stop=(k == ND - 1))
                    nc.scalar.activation(gTl[:, jg, :], ps, AF.Gelu,
                                         bias=bf1_sb[:, jg:jg + 1], scale=1.0)
                    if j == 3:
                        del w1tiles[jb]
                return run
            tasks.append(w1_prefetch(0))
            tasks.append(w1_prefetch(1))
            for jg in range(NF):
                tasks.append(fc1_chunk(jg))
                if jg % 4 == 0 and jg // 4 + 2 < 8:
                    tasks.append(w1_prefetch(jg // 4 + 2))

            # --- fc2 + resid2 + LN2 stats; w2 streamed as per-chunk-pair
            # blocks so each task fully drains its psums (2-slot ring safe) ---
            w2tiles = {}

            def w2_prefetch(jp):
                def run():
                    for kh in (0, 1):
                        w2tiles[(jp, kh)] = wst.tile([128, 16, HQ], BF16, tag="wst",
                                                     name=f"w2t{tg}_{jp}_{kh}")
                        nc.gpsimd.dma_start(out=w2tiles[(jp, kh)],
                                            in_=w2_d[:, jp, kh, :, :])
                return run

            def fc2_pair(jp):
                def run():
                    if jp + 1 < 4:
                        w2_prefetch(jp + 1)()
                    ps = {j: mm_pool.tile([128, HQ], F32, tag="mm",
                                          name=f"f2{tg}_{jp}_{j}")
                          for j in (0, 1)}
                    for kh in (0, 1):
                        w2t = w2tiles.pop((jp, kh))
                        for k16 in range(16):
                            kk = kh * 16 + k16
                            for j in (0, 1):
                                nc.tensor.matmul(
                                    ps[j], lhsT=w2t[:, k16, j * 128:(j + 1) * 128],
                                    rhs=gTl[:, kk, :],
                                    start=(kk == 0), stop=(kk == NF - 1))
                    for j in (0, 1):
                        jg = 2 * jp + j
                        if use_bf2:
                            tmp2 = lnsb.tile([128, HQ], F32, tag="tmp", bufs=3,
                                             name=f"f2t{tg}_{jg}")
                            nc.vector.tensor_scalar(out=tmp2, in0=ps[j], scalar1=1.0,
                                                    scalar2=bf2_sb[:, jg:jg + 1],
                                                    op0=OP.mult, op1=OP.add)
                            nc.vector.tensor_add(out=resid[:, jg, :], in0=tmp2,
                                                 in1=h1f[:, jg, :])
                        else:
                            nc.vector.scalar_tensor_tensor(
                                out=resid[:, jg, :], in0=ps[j], scalar=1.0,
                                in1=h1f[:, jg, :], op0=OP.mult, op1=OP.add)
                        ln_chunk(st2, f"b{tg}", jg)
                return run

            tasks.append(w2_prefetch(0))
            for jp in range(4):
                tasks.append(fc2_pair(jp))

            def ln2_writer(c, tmp, cbc):
                och = lnsb.tile([128, HQ], F32, tag="out", bufs=2,
                                name=f"och{tg}_{c}")
                nc.vector.tensor_add(out=och, in0=tmp, in1=cbc)
                nc.sync.dma_start(out=yT_d[c * 128:(c + 1) * 128, qsl], in_=och)

            def ln2_fin():
                holder["ap2"] = ln_finish(st2, g2_sb, be2_sb, ln2_writer, f"b{tg}")
            tasks.append(ln2_fin)

            def ln2_apply(i):
                def run():
                    holder["ap2"][2 * i]()
                    holder["ap2"][2 * i + 1]()
                return run
            for i in range(ND // 2):
                tasks.append(ln2_apply(i))
            return tasks

        # LN gamma/beta as f32r operands
        g1_sb = ffx.tile([1, D], F32R, name="g1_sb")
        be1_sb = ffx.tile([1, D], F32R, name="be1_sb")
        g2_sb = ffx.tile([1, D], F32R, name="g2_sb")
        be2_sb = ffx.tile([1, D], F32R, name="be2_sb")
        for dd, dst in ((g1_d, g1_sb), (be1_d, be1_sb), (g2_d, g2_sb),
                        (be2_d, be2_sb)):
            gbe_stage = ffx.tile([1, D], F32, tag="gbes", bufs=1, name="gbe_stage")
            nc.sync.dma_start(out=gbe_stage, in_=dd.rearrange("(a d) -> a d", a=1))
            nc.vector.tensor_copy(out=dst, in_=gbe_stage)

        residA = ffx.tile([128, ND, HQ], F32R, name="residA")
        h1fA = ffx.tile([128, ND, HQ], F32, name="h1fA")
        h1bfA = ffx.tile([128, ND, HQ], BF16, name="h1bfA")
        gTA = ffx.tile([128, NF, HQ], BF16, name="gTA")

        # ---------------- era A: attention half 0 + K/V production ----------
        # bucket tasks per pair (kt for pair p+2, v-half-1 chunks for pairs<4)
        # so every tile is emitted before its consuming pair
        for t in (0, 1):
            for sc4 in range(4):
                task_kt(t, sc4)()
        for c in range(NK):
            task_v(0, c)()
        kv_sched = []
        for p in range(NP):
            ts = []
            if p < 6:
                for sc4 in range(4):
                    ts.append(task_kt(p + 2, sc4))
            if p < 4:
                for c in range(4 * p, 4 * p + 4):
                    ts.append(task_v(1, c))
            kv_sched.append(ts)
        attention_half(0, kv_sched, bucketed=True)
        xkp_cm.close()

        # ---------------- era B: attention half 1 + FFN half 0 --------------
        ffnA = make_ffn_tasks(0, mmp, mmp, lnsbA, wstA, residA, h1fA, h1bfA, gTA)
        nc.scalar.activation(actwarm, eps_sb, AF.Sqrt)
        attention_half(1, ffnA)

        dump("KT", KT)
        dump("V3", V3)
        dump("QT", QT)
        dump("hT8", hT8)
        dump("h1fA", h1fA)
        dump("gTA", gTA)
        dump("residA", residA)
        # ---------------- era C: FFN half 1 ----------------
        att2.close()
        attbig_cm.close()
        att_ps.close()
        ffn_cm = ExitStack()
        ffn2 = ffn_cm.enter_context(tc.tile_pool(name="ffn2", bufs=1))
        pmm = ffn_cm.enter_context(tc.tile_pool(name="pmm", bufs=4, space="PSUM"))
        pmm2 = ffn_cm.enter_context(tc.tile_pool(name="pmm2", bufs=4, space="PSUM"))
        lnsbB = ffn_cm.enter_context(tc.tile_pool(name="lnsbB", bufs=2))
        wstB = ffn_cm.enter_context(tc.tile_pool(name="wstB", bufs=2))
        residB = ffn2.tile([128, ND, HQ], F32R, name="residB")
        h1fB = ffn2.tile([128, ND, HQ], F32, name="h1fB")
        h1bfB = ffn2.tile([128, ND, HQ], BF16, name="h1bfB")
        gTB = ffn2.tile([128, NF, HQ], BF16, name="gTB")
        ffnB = make_ffn_tasks(1, pmm, pmm2, lnsbB, wstB, residB, h1fB, h1bfB, gTB)
        for task in ffnB:
            task()
        dump("h1fB", h1fB)
        dump("gTB", gTB)
        ffn_cm.close()

    nc.compile()
    return nc


_CACHE = {}


def _get_built(use_mask, use_bv, use_bf2=False, dbg=False):
    key = (use_mask, use_bv, use_bf2, dbg)
    if key not in _CACHE:
        _CACHE[key] = _build(use_mask, use_bv, use_bf2, dbg)
    return _CACHE[key]


def kernel(x, mask, wq, bq, wk, bk, wv, bv, wo, bo, g1, be1, w1, bf1, w2, bf2, g2, be2):
    bf = ml_dtypes.bfloat16
    e4 = ml_dtypes.float8_e4m3
    f4 = np.float32
    x = np.asarray(x, f4)
    madd_full = (-10000.0 * (1.0 - np.asarray(mask).astype(f4)))  # [B, S]
    use_mask = bool((madd_full != 0.0).any())
    use_bv = bool(np.any(np.asarray(bv) != 0))
    use_bf2 = bool(np.any(np.asarray(bf2) != 0))
    nc = _get_built(use_mask, use_bv, use_bf2)

    def tile_w(w, dt, scale=1.0):
        # [D, N] -> [128, D/128, N]
        w = (np.asarray(w, f4) * scale).astype(dt)
        return np.ascontiguousarray(w.reshape(-1, 128, w.shape[1]).transpose(1, 0, 2))

    w1h = np.asarray(w1, f4).astype(bf).reshape(ND, 128, 8, 512).transpose(1, 2, 0, 3)
    w2h = np.asarray(w2, f4).astype(bf).reshape(2, 16, 128, 4, 256).transpose(2, 3, 0, 1, 4)
    shared = {
        "wq": tile_w(wq, e4, WS),
        "wk": tile_w(wk, e4, WS),
        "wv": tile_w(wv, e4, WS),
        "wo": tile_w(wo, e4, WS),
        "w1": np.ascontiguousarray(w1h),
        "w2": np.ascontiguousarray(w2h),
        "bq": np.asarray(bq, f4), "bk": np.asarray(bk, f4),
        "bf1": np.asarray(bf1, f4),
        "g1": np.asarray(g1, f4),
        "be1": np.asarray(be1, f4), "g2": np.asarray(g2, f4),
        "be2": np.asarray(be2, f4),
    }
    if use_bv:
        shared["bv"] = (np.asarray(bv, f4) * WS).astype(bf)
    if use_bf2:
        shared["bf2"] = np.asarray(bf2, f4)

    # [D, S] -> [128, ND, S] pre-tiled transposes
    bo_f = np.asarray(bo, f4)
    xTt = {b: np.ascontiguousarray(
        x[b].T.reshape(ND, 128, S).transpose(1, 0, 2)) for b in range(B)}
    xTt_8 = {b: xTt[b].astype(e4) for b in range(B)}
    bo_t = bo_f.reshape(ND, 128).T[:, :, None]          # [128, ND, 1]
    in_maps = []
    for c in range(NCORES):
        b, q0 = c // 4, (c % 4) * QS
        m = dict(shared)
        m["xkT"] = xTt_8[b]
        m["xqT"] = np.ascontiguousarray(xTt_8[b][:, :, q0:q0 + QS])
        m["xqTf"] = np.ascontiguousarray(xTt[b][:, :, q0:q0 + QS] + bo_t).astype(bf)
        if use_mask:
            m["madd"] = np.ascontiguousarray(madd_full[b])
        in_maps.append(m)

    res = run_bass_kernel_spmd(nc, in_maps, core_ids=list(range(NCORES)))
    kernel.last_result = res
    if res.exec_time_ns is not None:
        print(f"HW exec time: {res.exec_time_ns} ns")

    y = np.empty((B, S, D), np.float32)
    for c in range(NCORES):
        b, q0 = c // 4, (c % 4) * QS
        y[b, q0:q0 + QS, :] = np.asarray(res.results[c]["yT"], np.float32).T
    return y


# revision 28
# speedup vs baseline: 1.1653x; 1.1554x over previous
"""Trainium2 Bass/Tile kernel for a dense transformer block.

B=2, S=2048, D=1024, H=16 heads (dh=64), FF=4096, f32 IO.

Sharding: 8 cores = (2 batches) x (4 query-slices of 512 tokens), zero
cross-core communication (K/V recomputed per core).

v3: fp8 attention + split-query software pipeline.  All attention GEMMs
(Q/K/V/O projections, AV) are fp8e4m3 DoubleRow matmuls (0.5 PE
cycles/row, weights host-scaled by 64); scores run on fp8 Q/K (the
1/sqrt(dh) folds into the softmax exp scale); exp writes fp8 directly.
The FFN stays bf16 (fp8 FFN breaches the 2e-2 gate).  LayerNorm stats /
broadcast matmuls use float32r operands (1 cycle/row vs 4 for fp32).

The queries are processed in two 256-wide halves: era A runs attention
for half 0 (plus all K/V production), era B runs attention for half 1
with the ENTIRE half-0 FFN interleaved into its pair loop (softmax exp
keeps ACT busy while the FFN matmuls fill the PE), era C finishes with
the half-1 FFN.  LN statistics accumulate via DVE adds into SBUF so the
interleaved-FFN matmuls can share a single 2-slot PSUM ring with the
reciprocal broadcasts.
"""

import os
from contextlib import ExitStack

import numpy as np
import ml_dtypes

import concourse.bass as bass
import concourse.tile as tile
from concourse import bacc, mybir
from concourse.bass_utils import run_bass_kernel_spmd

BF16 = mybir.dt.bfloat16
F32 = mybir.dt.float32
F32R = mybir.dt.float32r
F8E4 = mybir.dt.float8e4
AF = mybir.ActivationFunctionType
OP = mybir.AluOpType
PM = mybir.MatmulPerfMode

B, S, D, H, FF = 2, 2048, 1024, 16, 4096
DH = D // H            # 64
NCORES = 8
QS = S // 4            # 512 queries per core
HQ = QS // 2           # 256-query pipeline half
NK = S // 128          # 16 key chunks
ND = D // 128          # 8 feature chunks
NF = FF // 128         # 32 ff chunks
NP = H // 2            # 8 head pairs
VW = DH + 1            # 65 = head width + ones column
EPS = 1e-12
WS = 64.0              # host-side fp8 weight scale
IWS = 1.0 / WS
IWS2 = IWS * IWS


def _blocks(n, w):
    return [list(range(i, min(i + w, n))) for i in range(0, n, w)]


def _build(use_mask, use_bv, use_bf2, dbg=False):
    nc = bacc.Bacc("TRN2", target_bir_lowering=False, debug=False)

    def din(name, shape, dtype):
        return nc.dram_tensor(name, shape, dtype, kind="ExternalInput").ap()

    def dump(name, tl):
        if not dbg:
            return
        dd = nc.dram_tensor(f"dbg_{name}", list(tl.shape), tl.dtype,
                            kind="ExternalOutput").ap()
        nc.sync.dma_start(out=dd, in_=tl)

    xkT_d = din("xkT", [128, ND, S], F8E4)
    xqT_d = din("xqT", [128, ND, QS], F8E4)
    xqTf_d = din("xqTf", [128, ND, QS], BF16)     # x slice, +bo folded in
    wq_d = din("wq", [128, ND, D], F8E4)          # pre-scaled by WS on host
    wk_d = din("wk", [128, ND, D], F8E4)
    wv_d = din("wv", [128, ND, D], F8E4)
    wo_d = din("wo", [128, ND, D], F8E4)
    w1_d = din("w1", [128, 8, ND, 512], BF16)     # [p, jb, k, n]
    w2_d = din("w2", [128, 4, 2, 16, 256], BF16)  # [p, jpair, kh, k16, n]
    bq_d = din("bq", [D], F32)
    bk_d = din("bk", [D], F32)
    bf1_d = din("bf1", [FF], F32)
    g1_d = din("g1", [D], F32)
    be1_d = din("be1", [D], F32)
    g2_d = din("g2", [D], F32)
    be2_d = din("be2", [D], F32)
    bf2_d = din("bf2", [D], F32) if use_bf2 else None
    bv_d = din("bv", [D], BF16) if use_bv else None     # pre-scaled by WS
    madd_d = din("madd", [S], F32) if use_mask else None
    yT_d = nc.dram_tensor("yT", [D, QS], F32, kind="ExternalOutput").ap()

    with tile.TileContext(nc) as tc, ExitStack() as glob:
        const = glob.enter_context(tc.tile_pool(name="const", bufs=1))
        gx = glob.enter_context(tc.tile_pool(name="gx", bufs=1))
        ffx = glob.enter_context(tc.tile_pool(name="ffx", bufs=1))
        att_ps = ExitStack()
        avp = att_ps.enter_context(tc.tile_pool(name="avp", bufs=2, space="PSUM"))
        mmp = att_ps.enter_context(tc.tile_pool(name="mmp", bufs=2, space="PSUM"))
        scp = att_ps.enter_context(tc.tile_pool(name="scp", bufs=2, space="PSUM"))

        # ---------------- big tiles + front-loaded DMAs ----------------
        hT8 = gx.tile([128, ND, QS], F8E4, name="hT8")          # 64*h, attn out
        wo_sb = gx.tile([128, ND, D], F8E4, name="wo_sb")
        xqf_sb = gx.tile([128, ND, QS], BF16, name="xqf_sb")
        attbig_cm = ExitStack()
        attbig = attbig_cm.enter_context(tc.tile_pool(name="attbig", bufs=1))
        KT = attbig.tile([128, ND, S], F8E4, name="KT")         # fp8(k + bk)
        V3 = attbig.tile([128, NK, H * VW], F8E4, name="V3")    # V + ones cols
        QT = attbig.tile([128, ND, QS], F8E4, name="QT")
        att2 = ExitStack()
        recipp = att2.enter_context(tc.tile_pool(name="recipp", bufs=2))
        expp = att2.enter_context(tc.tile_pool(name="expp", bufs=6))
        lnsbA = att2.enter_context(tc.tile_pool(name="lnsbA", bufs=2))
        wstA = att2.enter_context(tc.tile_pool(name="wstA", bufs=2))
        xkp_cm = ExitStack()
        xkp = xkp_cm.enter_context(tc.tile_pool(name="xkp", bufs=1))
        xk_sb = xkp.tile([128, ND, S], F8E4, name="xk_sb")
        wk_sb = xkp.tile([128, ND, D], F8E4, name="wk_sb")
        wv_sb = xkp.tile([128, ND, D], F8E4, name="wv_sb")
        wq_sb = wstA.tile([128, ND, D], F8E4, tag="wst", name="wq_sb")
        xq_sb = wstA.tile([128, ND, QS], F8E4, tag="wst", name="xq_sb")
        nc.sync.dma_start(out=xq_sb, in_=xqT_d[:])
        nc.sync.dma_start(out=wq_sb[:, :, 0:512], in_=wq_d[:, :, 0:512])
        nc.sync.dma_start(out=wq_sb[:, :, 512:D], in_=wq_d[:, :, 512:D])
        # ---------------- constants & small params ----------------
        bq_sb = const.tile([128, ND], F32, name="bq_sb")
        nc.sync.dma_start(out=bq_sb, in_=bq_d.rearrange("(c p) -> p c", p=128))
        bk_sb = const.tile([128, ND], F32, name="bk_sb")
        nc.sync.dma_start(out=bk_sb, in_=bk_d.rearrange("(c p) -> p c", p=128))
        bf1_sb = const.tile([128, NF], F32, name="bf1_sb")
        nc.sync.dma_start(out=bf1_sb, in_=bf1_d.rearrange("(c p) -> p c", p=128))
        if use_bf2:
            bf2_sb = const.tile([128, ND], F32, name="bf2_sb")
            nc.sync.dma_start(out=bf2_sb, in_=bf2_d.rearrange("(c p) -> p c", p=128))
        if use_mask:
            madd_sb = const.tile([128, NK], F32, name="madd_sb")
            nc.sync.dma_start(out=madd_sb, in_=madd_d.rearrange("(c p) -> p c", p=128))
        if use_bv:
            bv_sb = const.tile([1, D], BF16, name="bv_sb")
            nc.sync.dma_start(out=bv_sb, in_=bv_d.rearrange("(a d) -> a d", a=1))
            ones1b = const.tile([1, 128], BF16, name="ones1b")
            nc.vector.memset(ones1b, 1.0)
        # reciprocal broadcast selectors carry the 64x for the fp8 hT scale.
        # (memset can't write f32r directly; stage in f32 and DVE-round.)
        stg = const.tile([128, 4], F32, name="stg")
        nc.vector.memset(stg[:, 2:3], 1.0 / D)
        indstg = const.tile([1, 128 + HQ], F32, name="indstg")
        nc.vector.memset(indstg, 0.0)
        nc.vector.memset(indstg[0:1, 0:64], WS)
        indE = const.tile([1, 128], F32R, name="indE")
        nc.vector.tensor_copy(out=indE, in_=indstg[0:1, 0:128])
        nc.vector.memset(indstg[0:1, 0:64], 0.0)
        nc.vector.memset(indstg[0:1, 64:128], WS)
        indO = const.tile([1, 128], F32R, name="indO")
        nc.vector.tensor_copy(out=indO, in_=indstg[0:1, 0:128])
        invD128 = const.tile([128, 1], F32R, name="invD128")
        nc.vector.tensor_copy(out=invD128, in_=stg[:, 2:3])
        ones256 = const.tile([1, HQ], F32R, name="ones256")
        nc.vector.memset(indstg[0:1, 128:128 + HQ], 1.0)
        nc.vector.tensor_copy(out=ones256, in_=indstg[0:1, 128:128 + HQ])
        eps_sb = const.tile([1, 1], F32, name="eps_sb")
        nc.vector.memset(eps_sb, EPS)
        actwarm = const.tile([1, 1], F32, name="actwarm")
        nc.scalar.activation(actwarm, eps_sb, AF.Exp)
        # startup queue: phase-A operands first, w1 stream behind
        nc.sync.dma_start(out=wk_sb[:, :, 0:256], in_=wk_d[:, :, 0:256])
        nc.sync.dma_start(out=xk_sb, in_=xkT_d[:])
        nc.sync.dma_start(out=wv_sb[:, :, 0:512], in_=wv_d[:, :, 0:512])
        nc.sync.dma_start(out=wk_sb[:, :, 256:D], in_=wk_d[:, :, 256:D])
        nc.sync.dma_start(out=wv_sb[:, :, 512:D], in_=wv_d[:, :, 512:D])
        nc.sync.dma_start(out=wo_sb, in_=wo_d[:])
        nc.sync.dma_start(out=xqf_sb, in_=xqTf_d[:])

        nc.vector.memset(
            V3.rearrange("p c (h w) -> p (c h) w", w=VW)[:, :, DH:DH + 1], 1.0)

        # ---------------- phase 0: Q projection (fp8 DoubleRow) ----------------
        for tb in _blocks(ND, 2):
            ps = {}
            for t in tb:
                ps[t] = mmp.tile([128, QS], F32, tag="mm", name=f"qtps{t}")
            for k in range(ND // 2):
                for t in tb:
                    nc.tensor.matmul(ps[t],
                                     lhsT=wq_sb[:, 2 * k:2 * k + 2, t * 128:(t + 1) * 128],
                                     rhs=xq_sb[:, 2 * k:2 * k + 2, :],
                                     start=(k == 0), stop=(k == ND // 2 - 1),
                                     perf_mode=PM.DoubleRow)
            for t in tb:
                nc.vector.tensor_scalar(out=QT[:, t, :], in0=ps[t], scalar1=IWS,
                                        scalar2=bq_sb[:, t:t + 1],
                                        op0=OP.mult, op1=OP.add)

        # ---------------- attention-era task builders ----------------
        def task_kt(t, sc4):
            def run():
                ps = mmp.tile([128, 512], F32, tag="mm", name=f"ktps{t}_{sc4}")
                for k in range(ND // 2):
                    nc.tensor.matmul(ps,
                                     lhsT=wk_sb[:, 2 * k:2 * k + 2, t * 128:(t + 1) * 128],
                                     rhs=xk_sb[:, 2 * k:2 * k + 2, sc4 * 512:(sc4 + 1) * 512],
                                     start=(k == 0), stop=(k == ND // 2 - 1),
                                     perf_mode=PM.DoubleRow)
                nc.vector.tensor_scalar(
                    out=KT[:, t, sc4 * 512:(sc4 + 1) * 512], in0=ps,
                    scalar1=IWS, scalar2=bk_sb[:, t:t + 1], op0=OP.mult, op1=OP.add)
            return run

        def task_v(nh, c):
            def run():
                ps = mmp.tile([128, 512], F32, tag="mm", name=f"vps{nh}_{c}")
                if use_bv:
                    nc.tensor.matmul(ps, lhsT=ones1b,
                                     rhs=bv_sb[:, nh * 512:(nh + 1) * 512],
                                     start=True, stop=False)
                for k in range(ND // 2):
                    nc.tensor.matmul(ps,
                                     lhsT=xk_sb[:, 2 * k:2 * k + 2, c * 128:(c + 1) * 128],
                                     rhs=wv_sb[:, 2 * k:2 * k + 2, nh * 512:(nh + 1) * 512],
                                     start=(k == 0 and not use_bv),
                                     stop=(k == ND // 2 - 1),
                                     perf_mode=PM.DoubleRow)
                out_ap = V3[:, c, :].rearrange("p (h w) -> p h w", w=VW)[:, 8 * nh:8 * nh + 8, 0:DH]
                nc.vector.tensor_scalar_mul(
                    out=out_ap, in0=ps.rearrange("p (h w) -> p h w", w=DH), scalar1=IWS)
            return run

        # ---------------- attention half (pairs over one query half) ---------
        def attention_half(half, tasks, horizon=64, bucketed=False):
            q0 = half * HQ
            qsl = slice(q0, q0 + HQ)
            tailB_pending = []

            def emit_tailB(p, recE, recO, hTu_t):
                rbc = mmp.tile([128, HQ], F32, tag="mm", name=f"rbc{half}_{p}")
                nc.tensor.matmul(rbc, lhsT=indE, rhs=recE, start=True, stop=False)
                nc.tensor.matmul(rbc, lhsT=indO, rhs=recO, start=False, stop=True)
                nc.vector.tensor_mul(out=hT8[:, p, qsl], in0=hTu_t, in1=rbc)

            ntasks = 0 if bucketed else len(tasks)
            ti = 0
            for p in range(NP):
                if bucketed:
                    ptasks = tasks[p]
                    nt = len(ptasks)
                t = p
                av = {}
                av[0] = avp.tile([VW, HQ], F32, tag="av", name=f"av{half}_{p}e")
                av[1] = avp.tile([VW, HQ], F32, tag="av", name=f"av{half}_{p}o")
                pend = []

                def emit_av(i, et):
                    for parity in (0, 1):
                        h = 2 * p + parity
                        nc.tensor.matmul(av[parity],
                                         lhsT=V3[:, 2 * i:2 * i + 2, h * VW:(h + 1) * VW],
                                         rhs=et[:, parity, :, :],
                                         start=(i == 0), stop=(i == 7),
                                         perf_mode=PM.DoubleRow)

                for i in range(8):
                    sct = scp.tile([128, 2, 2, HQ], F32, tag="sc",
                                   name=f"sc{half}_{p}_{i}")
                    for cc in (0, 1):
                        ch = 2 * i + cc
                        for parity in (0, 1):
                            base = 64 * parity
                            nc.tensor.matmul(sct[:, parity, cc, :],
                                             lhsT=KT[base:base + 64, t, ch * 128:(ch + 1) * 128],
                                             rhs=QT[base:base + 64, t, qsl],
                                             start=True, stop=True)
                    et = expp.tile([128, 2, 2, HQ], F8E4, tag="exp",
                                   name=f"ex{half}_{p}_{i}")
                    if use_mask:
                        for cc in (0, 1):
                            ch = 2 * i + cc
                            nc.scalar.activation(et[:, :, cc, :], sct[:, :, cc, :],
                                                 AF.Exp, bias=madd_sb[:, ch:ch + 1],
                                                 scale=0.125)
                    else:
                        nc.scalar.activation(et, sct, AF.Exp, scale=0.125)
                    pend.append((i, et))
                    if len(pend) > 1:
                        emit_av(*pend.pop(0))
                    if i == 4 and tailB_pending:
                        emit_tailB(*tailB_pending.pop(0))
                    if bucketed:
                        for task in ptasks[(nt * i) // 8:(nt * (i + 1)) // 8]:
                            task()
                    else:
                        hi = (ntasks * min(p * 8 + i + 1, horizon)) // horizon
                        while ti < hi:
                            tasks[ti]()
                            ti += 1
                while pend:
                    emit_av(*pend.pop(0))

                denE = recipp.tile([1, HQ], F32, tag="den", bufs=2, name=f"denE{half}_{p}")
                denO = recipp.tile([1, HQ], F32, tag="den", bufs=2, name=f"denO{half}_{p}")
                recEf = recipp.tile([1, HQ], F32, tag="recf", bufs=2, name=f"recEf{half}_{p}")
                recOf = recipp.tile([1, HQ], F32, tag="recf", bufs=2, name=f"recOf{half}_{p}")
                recE = recipp.tile([1, HQ], F32R, tag="rec", bufs=4, name=f"recE{half}_{p}")
                recO = recipp.tile([1, HQ], F32R, tag="rec", bufs=4, name=f"recO{half}_{p}")
                hTu_t = recipp.tile([128, HQ], BF16, tag="htu", bufs=3,
                                    name=f"hTu{half}_{p}")
                if p == NP - 1:
                    nc.scalar.copy(denE, av[0][64:65, :])
                    nc.scalar.copy(denO, av[1][64:65, :])
                    nc.scalar.copy(hTu_t[0:64, :], av[0][0:64, :])
                    nc.scalar.copy(hTu_t[64:128, :], av[1][0:64, :])
                else:
                    nc.vector.tensor_copy(out=hTu_t[0:64, :], in_=av[0][0:64, :])
                    nc.vector.tensor_copy(out=hTu_t[64:128, :], in_=av[1][0:64, :])
                    nc.vector.tensor_copy(out=denE, in_=av[0][64:65, :])
                    nc.vector.tensor_copy(out=denO, in_=av[1][64:65, :])
                nc.vector.reciprocal_approx_fast(out=recEf, in_=denE)
                nc.vector.reciprocal_approx_fast(out=recOf, in_=denO)
                nc.vector.tensor_copy(out=recE, in_=recEf)
                nc.vector.tensor_copy(out=recO, in_=recOf)
                tailB_pending.append((p, recE, recO, hTu_t))
            while tailB_pending:
                emit_tailB(*tailB_pending.pop(0))
            if not bucketed:
                while ti < ntasks:
                    tasks[ti]()
                    ti += 1

        # ---------------- FFN emission (per query half) ----------------
        # Matmul psums flow through the mm ring of the given pool; LN stats
        # accumulate on DVE into SBUF (no long-lived psum).
        def make_ffn_tasks(half, mm_pool, fc_pool, lnsb, wst, resid, h1f, h1bf, gTl,
                           pb, defer_gelu=False):
            # pb: late-bound pools for the post phase (may be filled after
            # this builder runs): pb["mm"], pb["lnsb"], pb["wst"]
            q0 = half * HQ
            qsl = slice(q0, q0 + HQ)
            tg = f"h{half}"
            tasks = []
            st1 = {}
            st2 = {}

            def ln_accum(st, tagn, c, src_f32r, src_name, mmP, lnP):
                stp = mmP.tile([1, HQ], F32, tag="mm",
                               name=f"st{tagn}_{src_name}_{c}")
                nc.tensor.matmul(stp, lhsT=invD128, rhs=src_f32r,
                                 start=True, stop=True)
                if c == 0:
                    acc = lnP.tile([1, HQ], F32, tag=f"acc_{src_name}", bufs=2,
                                   name=f"acc{tagn}_{src_name}")
                    st[src_name] = acc
                    nc.vector.tensor_copy(out=acc, in_=stp)
                else:
                    nc.vector.tensor_add(out=st[src_name], in0=st[src_name], in1=stp)

            def ln_chunk(st, tagn, c, mmP, lnP):
                sq = lnP.tile([128, HQ], F32R, tag="sq", bufs=2, name=f"sq{tagn}_{c}")
                nc.vector.tensor_mul(out=sq, in0=resid[:, c, :], in1=resid[:, c, :])
                ln_accum(st, tagn, c, resid[:, c, :], "s", mmP, lnP)
                ln_accum(st, tagn, c, sq, "q", mmP, lnP)

            def ln_finish(st, g_sb, be_sb, writer, tagn, mmP, lnP):
                u = st["s"]
                var = lnP.tile([1, HQ], F32, tag="var", bufs=2, name=f"var_{tagn}")
                std = lnP.tile([1, HQ], F32, tag="std", bufs=2, name=f"std_{tagn}")
                avecf = lnP.tile([1, HQ], F32, tag="avecf", bufs=2, name=f"avecf_{tagn}")
                avec = lnP.tile([1, HQ], F32R, tag="avec", bufs=2, name=f"avec_{tagn}")
                cvec = lnP.tile([1, HQ], F32R, tag="cvec", bufs=2, name=f"cvec_{tagn}")
                nc.vector.scalar_tensor_tensor(out=var, in0=u, scalar=-1.0, in1=u,
                                               op0=OP.mult, op1=OP.mult)
                nc.vector.tensor_add(out=var, in0=st["q"], in1=var)
                nc.scalar.activation(std, var, AF.Sqrt, bias=eps_sb, scale=1.0)
                nc.vector.reciprocal_approx_fast(out=avecf, in_=std)
                nc.vector.tensor_copy(out=avec, in_=avecf)
                nc.vector.scalar_tensor_tensor(out=cvec, in0=u, scalar=-1.0,
                                               in1=avecf, op0=OP.mult, op1=OP.mult)

                def apply_chunk(c):
                    def run():
                        abc = mmP.tile([128, HQ], F32, tag="mm",
                                       name=f"abc{tagn}_{c}")
                        nc.tensor.matmul(abc, lhsT=g_sb[0:1, c * 128:(c + 1) * 128],
                                         rhs=avec, start=True, stop=True)
                        cbc = mmP.tile([128, HQ], F32, tag="mm",
                                       name=f"cbc{tagn}_{c}")
                        nc.tensor.matmul(cbc, lhsT=g_sb[0:1, c * 128:(c + 1) * 128],
                                         rhs=cvec, start=True, stop=False)
                        nc.tensor.matmul(cbc, lhsT=be_sb[0:1, c * 128:(c + 1) * 128],
                                         rhs=ones256, start=False, stop=True)
                        tmp = lnP.tile([128, HQ], F32, tag="tmp", bufs=3,
                                       name=f"lnt{tagn}_{c}")
                        nc.vector.tensor_mul(out=tmp, in0=resid[:, c, :], in1=abc)
                        writer(c, tmp, cbc)
                    return run
                return [apply_chunk(c) for c in range(ND)]

            # --- o-projection + resid1 + LN1 stats, blocks of 2 ---
            def oproj_block(jb):
                def run():
                    ps = {j: mm_pool.tile([128, HQ], F32, tag="mm",
                                          name=f"op{tg}_{j}") for j in jb}
                    for k in range(ND // 2):
                        for j in jb:
                            nc.tensor.matmul(ps[j],
                                             lhsT=wo_sb[:, 2 * k:2 * k + 2, j * 128:(j + 1) * 128],
                                             rhs=hT8[:, 2 * k:2 * k + 2, qsl],
                                             start=(k == 0), stop=(k == ND // 2 - 1),
                                             perf_mode=PM.DoubleRow)
                    for j in jb:
                        nc.vector.scalar_tensor_tensor(
                            out=resid[:, j, :], in0=ps[j], scalar=IWS2,
                            in1=xqf_sb[:, j, qsl], op0=OP.mult, op1=OP.add)
                        ln_chunk(st1, f"a{tg}", j, mm_pool, lnsb)
                return run
            for jb in _blocks(ND, 2):
                tasks.append(oproj_block(jb))

            def ln1_writer(c, tmp, cbc):
                nc.vector.tensor_add(out=h1f[:, c, :], in0=tmp, in1=cbc)
                nc.vector.tensor_copy(out=h1bf[:, c, :], in_=h1f[:, c, :])

            holder = {}

            def ln1_fin():
                holder["ap1"] = ln_finish(st1, g1_sb, be1_sb, ln1_writer, f"a{tg}",
                                          mm_pool, lnsb)
            tasks.append(ln1_fin)

            def ln1_apply(i):
                def run():
                    holder["ap1"][2 * i]()
                    holder["ap1"][2 * i + 1]()
                return run
            for i in range(ND // 2):
                tasks.append(ln1_apply(i))

            # --- fc1 + gelu, one out-chunk per task; w1 streamed 2 ahead ---
            w1tiles = {}

            def w1_prefetch(jb):
                def run():
                    w1tiles[jb] = wst.tile([128, ND, 512], BF16, tag="wst",
                                           name=f"w1t{tg}_{jb}")
                    nc.gpsimd.dma_start(out=w1tiles[jb], in_=w1_d[:, jb, :, :])
                return run

            def fc1_chunk(jg):
                def run():
                    jb, j = jg // 4, jg % 4
                    w1t = w1tiles[jb]
                    ps = fc_pool.tile([128, HQ], F32, tag="mm", name=f"f1{tg}_{jg}")
                    for k in range(ND):
                        nc.tensor.matmul(ps, lhsT=w1t[:, k, j * 128:(j + 1) * 128],
                                         rhs=h1bf[:, k, :],
                                         start=(k == 0), stop=(k == ND - 1))
                    if defer_gelu:
                        # store z + bf1; gelu applied in-place later so the
                        # ACT table never leaves the exp set during era B
                        nc.vector.tensor_scalar(out=gTl[:, jg, :], in0=ps,
                                                scalar1=1.0,
                                                scalar2=bf1_sb[:, jg:jg + 1],
                                                op0=OP.mult, op1=OP.add)
                    else:
                        nc.scalar.activation(gTl[:, jg, :], ps, AF.Gelu,
                                             bias=bf1_sb[:, jg:jg + 1], scale=1.0)
                    if j == 3:
                        del w1tiles[jb]
                return run
            tasks.append(w1_prefetch(0))
            tasks.append(w1_prefetch(1))
            for jg in range(NF):
                tasks.append(fc1_chunk(jg))
                if jg % 4 == 0 and jg // 4 + 2 < 8:
                    tasks.append(w1_prefetch(jg // 4 + 2))

            post = []
            if defer_gelu:
                def gelu_chunk(jg):
                    def run():
                        nc.scalar.activation(gTl[:, jg, :], gTl[:, jg, :],
                                             AF.Gelu, scale=1.0)
                    return run
                for jg in range(NF):
                    post.append(gelu_chunk(jg))
            tasks, pre = post, tasks

            # --- fc2 + resid2 + LN2 stats; w2 streamed as per-chunk-pair
            # blocks so each task fully drains its psums (2-slot ring safe) ---
            w2tiles = {}

            def w2_prefetch(jp):
                def run():
                    for kh in (0, 1):
                        w2tiles[(jp, kh)] = pb["wst"].tile([128, 16, HQ], BF16,
                                                           tag="wst",
                                                           name=f"w2t{tg}_{jp}_{kh}")
                        nc.gpsimd.dma_start(out=w2tiles[(jp, kh)],
                                            in_=w2_d[:, jp, kh, :, :])
                return run

            def fc2_pair(jp):
                def run():
                    if jp + 1 < 4:
                        w2_prefetch(jp + 1)()
                    ps = {j: pb["mm"].tile([128, HQ], F32, tag="mm",
                                           name=f"f2{tg}_{jp}_{j}")
                          for j in (0, 1)}
                    for kh in (0, 1):
                        w2t = w2tiles.pop((jp, kh))
                        for k16 in range(16):
                            kk = kh * 16 + k16
                            for j in (0, 1):
                                nc.tensor.matmul(
                                    ps[j], lhsT=w2t[:, k16, j * 128:(j + 1) * 128],
                                    rhs=gTl[:, kk, :],
                                    start=(kk == 0), stop=(kk == NF - 1))
                    for j in (0, 1):
                        jg = 2 * jp + j
                        if use_bf2:
                            tmp2 = pb["lnsb"].tile([128, HQ], F32, tag="tmp", bufs=3,
                                                   name=f"f2t{tg}_{jg}")
                            nc.vector.tensor_scalar(out=tmp2, in0=ps[j], scalar1=1.0,
                                                    scalar2=bf2_sb[:, jg:jg + 1],
                                                    op0=OP.mult, op1=OP.add)
                            nc.vector.tensor_add(out=resid[:, jg, :], in0=tmp2,
                                                 in1=h1f[:, jg, :])
                        else:
                            nc.vector.scalar_tensor_tensor(
                                out=resid[:, jg, :], in0=ps[j], scalar=1.0,
                                in1=h1f[:, jg, :], op0=OP.mult, op1=OP.add)
                        ln_chunk(st2, f"b{tg}", jg, pb["mm"], pb["lnsb"])
                return run

            tasks.append(w2_prefetch(0))
            for jp in range(4):
                tasks.append(fc2_pair(jp))

            def ln2_writer(c, tmp, cbc):
                och = pb["lnsb"].tile([128, HQ], F32, tag="out", bufs=2,
                                      name=f"och{tg}_{c}")
                nc.vector.tensor_add(out=och, in0=tmp, in1=cbc)
                nc.sync.dma_start(out=yT_d[c * 128:(c + 1) * 128, qsl], in_=och)

            def ln2_fin():
                holder["ap2"] = ln_finish(st2, g2_sb, be2_sb, ln2_writer, f"b{tg}",
                                          pb["mm"], pb["lnsb"])
            tasks.append(ln2_fin)

            def ln2_apply(i):
                def run():
                    holder["ap2"][2 * i]()
                    holder["ap2"][2 * i + 1]()
                return run
            for i in range(ND // 2):
                tasks.append(ln2_apply(i))
            return pre, tasks

        # LN gamma/beta as f32r operands
        g1_sb = ffx.tile([1, D], F32R, name="g1_sb")
        be1_sb = ffx.tile([1, D], F32R, name="be1_sb")
        g2_sb = ffx.tile([1, D], F32R, name="g2_sb")
        be2_sb = ffx.tile([1, D], F32R, name="be2_sb")
        for dd, dst in ((g1_d, g1_sb), (be1_d, be1_sb), (g2_d, g2_sb),
                        (be2_d, be2_sb)):
            gbe_stage = ffx.tile([1, D], F32, tag="gbes", bufs=1, name="gbe_stage")
            nc.sync.dma_start(out=gbe_stage, in_=dd.rearrange("(a d) -> a d", a=1))
            nc.vector.tensor_copy(out=dst, in_=gbe_stage)

        residA = ffx.tile([128, ND, HQ], F32R, name="residA")
        h1fA = ffx.tile([128, ND, HQ], F32, name="h1fA")
        h1bfA = ffx.tile([128, ND, HQ], BF16, name="h1bfA")
        gTA = ffx.tile([128, NF, HQ], BF16, name="gTA")

        # ---------------- era A: attention half 0 + K/V production ----------
        # bucket tasks per pair (kt for pair p+2, v-half-1 chunks for pairs<4)
        # so every tile is emitted before its consuming pair
        for t in (0, 1):
            for sc4 in range(4):
                task_kt(t, sc4)()
        for c in range(NK):
            task_v(0, c)()
        kv_sched = []
        for p in range(NP):
            ts = []
            if p < 6:
                for sc4 in range(4):
                    ts.append(task_kt(p + 2, sc4))
            if p < 4:
                for c in range(4 * p, 4 * p + 4):
                    ts.append(task_v(1, c))
            kv_sched.append(ts)
        attention_half(0, kv_sched, bucketed=True)
        xkp_cm.close()

        # ---------------- era B: attention half 1 + FFN half 0 --------------
        pbA = {}
        ffnA_pre, ffnA_post = make_ffn_tasks(0, mmp, mmp, lnsbA, wstA, residA,
                                             h1fA, h1bfA, gTA, pbA, defer_gelu=True)
        attention_half(1, ffnA_pre)

        dump("KT", KT)
        dump("V3", V3)
        dump("QT", QT)
        dump("hT8", hT8)
        dump("h1fA", h1fA)
        dump("gTA", gTA)
        dump("residA", residA)
        # ---------------- era C: FFN half 1 ----------------
        att2.close()
        attbig_cm.close()
        att_ps.close()
        ffn_cm = ExitStack()
        ffn2 = ffn_cm.enter_context(tc.tile_pool(name="ffn2", bufs=1))
        pmm = ffn_cm.enter_context(tc.tile_pool(name="pmm", bufs=4, space="PSUM"))
        pmm2 = ffn_cm.enter_context(tc.tile_pool(name="pmm2", bufs=4, space="PSUM"))
        lnsbB = ffn_cm.enter_context(tc.tile_pool(name="lnsbB", bufs=2))
        wstB = ffn_cm.enter_context(tc.tile_pool(name="wstB", bufs=2))
        residB = ffn2.tile([128, ND, HQ], F32R, name="residB")
        h1fB = ffn2.tile([128, ND, HQ], F32, name="h1fB")
        h1bfB = ffn2.tile([128, ND, HQ], BF16, name="h1bfB")
        gTB = ffn2.tile([128, NF, HQ], BF16, name="gTB")
        pbA["mm"] = pmm
        pbA["lnsb"] = lnsbB
        pbA["wst"] = wstB
        pbB = {"mm": pmm2, "lnsb": lnsbB, "wst": wstB}
        ffnB_pre, ffnB_post = make_ffn_tasks(1, pmm2, pmm2, lnsbB, wstB, residB,
                                             h1fB, h1bfB, gTB, pbB)
        ia = ib = 0
        while ia < len(ffnA_post) or ib < len(ffnB_pre):
            if ia < len(ffnA_post):
                ffnA_post[ia]()
                ia += 1
            if ib < len(ffnB_pre) and (ib * len(ffnA_post) <= ia * len(ffnB_pre)
                                       or ia >= len(ffnA_post)):
                ffnB_pre[ib]()
                ib += 1
        for task in ffnB_post:
            task()
        dump("h1fB", h1fB)
        dump("gTB", gTB)
        ffn_cm.close()

    nc.compile()
    return nc


_CACHE = {}


def _get_built(use_mask, use_bv, use_bf2=False, dbg=False):
    key = (use_mask, use_bv, use_bf2, dbg)
    if key not in _CACHE:
        _CACHE[key] = _build(use_mask, use_bv, use_bf2, dbg)
    return _CACHE[key]


def kernel(x, mask, wq, bq, wk, bk, wv, bv, wo, bo, g1, be1, w1, bf1, w2, bf2, g2, be2):
    bf = ml_dtypes.bfloat16
    e4 = ml_dtypes.float8_e4m3
    f4 = np.float32
    x = np.asarray(x, f4)
    madd_full = (-10000.0 * (1.0 - np.asarray(mask).astype(f4)))  # [B, S]
    use_mask = bool((madd_full != 0.0).any())
    use_bv = bool(np.any(np.asarray(bv) != 0))
    use_bf2 = bool(np.any(np.asarray(bf2) != 0))
    nc = _get_built(use_mask, use_bv, use_bf2)

    def tile_w(w, dt, scale=1.0):
        # [D, N] -> [128, D/128, N]
        w = (np.asarray(w, f4) * scale).astype(dt)
        return np.ascontiguousarray(w.reshape(-1, 128, w.shape[1]).transpose(1, 0, 2))

    w1h = np.asarray(w1, f4).astype(bf).reshape(ND, 128, 8, 512).transpose(1, 2, 0, 3)
    w2h = np.asarray(w2, f4).astype(bf).reshape(2, 16, 128, 4, 256).transpose(2, 3, 0, 1, 4)
    shared = {
        "wq": tile_w(wq, e4, WS),
        "wk": tile_w(wk, e4, WS),
        "wv": tile_w(wv, e4, WS),
        "wo": tile_w(wo, e4, WS),
        "w1": np.ascontiguousarray(w1h),
        "w2": np.ascontiguousarray(w2h),
        "bq": np.asarray(bq, f4), "bk": np.asarray(bk, f4),
        "bf1": np.asarray(bf1, f4),
        "g1": np.asarray(g1, f4),
        "be1": np.asarray(be1, f4), "g2": np.asarray(g2, f4),
        "be2": np.asarray(be2, f4),
    }
    if use_bv:
        shared["bv"] = (np.asarray(bv, f4) * WS).astype(bf)
    if use_bf2:
        shared["bf2"] = np.asarray(bf2, f4)

    # [D, S] -> [128, ND, S] pre-tiled transposes
    bo_f = np.asarray(bo, f4)
    xTt = {b: np.ascontiguousarray(
        x[b].T.reshape(ND, 128, S).transpose(1, 0, 2)) for b in range(B)}
    xTt_8 = {b: xTt[b].astype(e4) for b in range(B)}
    bo_t = bo_f.reshape(ND, 128).T[:, :, None]          # [128, ND, 1]
    in_maps = []
    for c in range(NCORES):
        b, q0 = c // 4, (c % 4) * QS
        m = dict(shared)
        m["xkT"] = xTt_8[b]
        m["xqT"] = np.ascontiguousarray(xTt_8[b][:, :, q0:q0 + QS])
        m["xqTf"] = np.ascontiguousarray(xTt[b][:, :, q0:q0 + QS] + bo_t).astype(bf)
        if use_mask:
            m["madd"] = np.ascontiguousarray(madd_full[b])
        in_maps.append(m)

    res = run_bass_kernel_spmd(nc, in_maps, core_ids=list(range(NCORES)))
    kernel.last_result = res
    if res.exec_time_ns is not None:
        print(f"HW exec time: {res.exec_time_ns} ns")

    y = np.empty((B, S, D), np.float32)
    for c in range(NCORES):
        b, q0 = c // 4, (c % 4) * QS
        y[b, q0:q0 + QS, :] = np.asarray(res.results[c]["yT"], np.float32).T
    return y


# revision 32
# speedup vs baseline: 1.1883x; 1.0197x over previous
"""Trainium2 Bass/Tile kernel for a dense transformer block.

B=2, S=2048, D=1024, H=16 heads (dh=64), FF=4096, f32 IO.

Sharding: 8 cores = (2 batches) x (4 query-slices of 512 tokens), zero
cross-core communication (K/V recomputed per core).

v3: fp8 attention + split-query software pipeline.  All attention GEMMs
(Q/K/V/O projections, AV) are fp8e4m3 DoubleRow matmuls (0.5 PE
cycles/row, weights host-scaled by 64); scores run on fp8 Q/K (the
1/sqrt(dh) folds into the softmax exp scale); exp writes fp8 directly.
The FFN stays bf16 (fp8 FFN breaches the 2e-2 gate).  LayerNorm stats /
broadcast matmuls use float32r operands (1 cycle/row vs 4 for fp32).

The queries are processed in two 256-wide halves: era A runs attention
for half 0 (plus all K/V production), era B runs attention for half 1
with the ENTIRE half-0 FFN interleaved into its pair loop (softmax exp
keeps ACT busy while the FFN matmuls fill the PE), era C finishes with
the half-1 FFN.  LN statistics accumulate via DVE adds into SBUF so the
interleaved-FFN matmuls can share a single 2-slot PSUM ring with the
reciprocal broadcasts.
"""

import os
from contextlib import ExitStack

import numpy as np
import ml_dtypes

import concourse.bass as bass
import concourse.tile as tile
from concourse import bacc, mybir
from concourse.bass_utils import run_bass_kernel_spmd

BF16 = mybir.dt.bfloat16
F32 = mybir.dt.float32
F32R = mybir.dt.float32r
F8E4 = mybir.dt.float8e4
AF = mybir.ActivationFunctionType
OP = mybir.AluOpType
PM = mybir.MatmulPerfMode

B, S, D, H, FF = 2, 2048, 1024, 16, 4096
DH = D // H            # 64
NCORES = 8
QS = S // 4            # 512 queries per core
HQ = QS // 2           # 256-query pipeline half
NK = S // 128          # 16 key chunks
ND = D // 128          # 8 feature chunks
NF = FF // 128         # 32 ff chunks
NP = H // 2            # 8 head pairs
VW = DH + 1            # 65 = head width + ones column
EPS = 1e-12
WS = 64.0              # host-side fp8 weight scale
IWS = 1.0 / WS
IWS2 = IWS * IWS


def _blocks(n, w):
    return [list(range(i, min(i + w, n))) for i in range(0, n, w)]


def _build(use_mask, use_bv, use_bf2, dbg=False):
    nc = bacc.Bacc("TRN2", target_bir_lowering=False, debug=False)

    def din(name, shape, dtype):
        return nc.dram_tensor(name, shape, dtype, kind="ExternalInput").ap()

    def dump(name, tl):
        if not dbg:
            return
        dd = nc.dram_tensor(f"dbg_{name}", list(tl.shape), tl.dtype,
                            kind="ExternalOutput").ap()
        nc.sync.dma_start(out=dd, in_=tl)

    xkT_d = din("xkT", [128, ND, S], F8E4)
    xqT_d = din("xqT", [128, ND, QS], F8E4)
    xqTf_d = din("xqTf", [128, ND, QS], BF16)     # x slice, +bo folded in
    wq_d = din("wq", [128, ND, D], F8E4)          # pre-scaled by WS on host
    wk_d = din("wk", [128, ND, D], F8E4)
    wv_d = din("wv", [128, ND, D], F8E4)
    wo_d = din("wo", [128, ND, D], F8E4)
    w1_d = din("w1", [128, 8, ND, 512], BF16)     # [p, jb, k, n]
    w2_d = din("w2", [128, 4, 2, 16, 256], BF16)  # [p, jpair, kh, k16, n]
    bq_d = din("bq", [D], F32)
    bk_d = din("bk", [D], F32)
    bf1_d = din("bf1", [FF], F32)
    g1_d = din("g1", [D], F32)
    be1_d = din("be1", [D], F32)
    g2_d = din("g2", [D], F32)
    be2_d = din("be2", [D], F32)
    bf2_d = din("bf2", [D], F32) if use_bf2 else None
    bv_d = din("bv", [D], BF16) if use_bv else None     # pre-scaled by WS
    madd_d = din("madd", [S], F32) if use_mask else None
    yT_d = nc.dram_tensor("yT", [D, QS], F32, kind="ExternalOutput").ap()

    with tile.TileContext(nc) as tc, ExitStack() as glob:
        const = glob.enter_context(tc.tile_pool(name="const", bufs=1))
        gx = glob.enter_context(tc.tile_pool(name="gx", bufs=1))
        ffx = glob.enter_context(tc.tile_pool(name="ffx", bufs=1))
        att_ps = ExitStack()
        avp = att_ps.enter_context(tc.tile_pool(name="avp", bufs=2, space="PSUM"))
        mmp = att_ps.enter_context(tc.tile_pool(name="mmp", bufs=2, space="PSUM"))
        scp = att_ps.enter_context(tc.tile_pool(name="scp", bufs=2, space="PSUM"))

        # ---------------- big tiles + front-loaded DMAs ----------------
        hT8 = gx.tile([128, ND, QS], F8E4, name="hT8")          # 64*h, attn out
        wo_sb = gx.tile([128, ND, D], F8E4, name="wo_sb")
        xqf_sb = gx.tile([128, ND, QS], BF16, name="xqf_sb")
        attbig_cm = ExitStack()
        attbig = attbig_cm.enter_context(tc.tile_pool(name="attbig", bufs=1))
        KT = attbig.tile([128, ND, S], F8E4, name="KT")         # fp8(k + bk)
        V3 = attbig.tile([128, NK, H * VW], F8E4, name="V3")    # V + ones cols
        QT = attbig.tile([128, ND, QS], F8E4, name="QT")
        att2 = ExitStack()
        recipp = att2.enter_context(tc.tile_pool(name="recipp", bufs=2))
        expp = att2.enter_context(tc.tile_pool(name="expp", bufs=4))
        lnsbA = att2.enter_context(tc.tile_pool(name="lnsbA", bufs=2))
        wstA = att2.enter_context(tc.tile_pool(name="wstA", bufs=2))
        xkp_cm = ExitStack()
        xkp = xkp_cm.enter_context(tc.tile_pool(name="xkp", bufs=1))
        xk_sb = xkp.tile([128, ND, S], F8E4, name="xk_sb")
        wk_sb = xkp.tile([128, ND, D], F8E4, name="wk_sb")
        wv_sb = xkp.tile([128, ND, D], F8E4, name="wv_sb")
        p0 = ExitStack()
        p0pool = p0.enter_context(tc.tile_pool(name="p0pool", bufs=1))
        wq_sb = p0pool.tile([128, ND, D], F8E4, name="wq_sb")
        xq_sb = p0pool.tile([128, ND, QS], F8E4, name="xq_sb")
        nc.sync.dma_start(out=xq_sb, in_=xqT_d[:])
        nc.sync.dma_start(out=wq_sb[:, :, 0:512], in_=wq_d[:, :, 0:512])
        nc.sync.dma_start(out=wq_sb[:, :, 512:D], in_=wq_d[:, :, 512:D])
        # ---------------- constants & small params ----------------
        bq_sb = const.tile([128, ND], F32, name="bq_sb")
        nc.sync.dma_start(out=bq_sb, in_=bq_d.rearrange("(c p) -> p c", p=128))
        bk_sb = const.tile([128, ND], F32, name="bk_sb")
        nc.sync.dma_start(out=bk_sb, in_=bk_d.rearrange("(c p) -> p c", p=128))
        bf1_sb = const.tile([128, NF], F32, name="bf1_sb")
        nc.sync.dma_start(out=bf1_sb, in_=bf1_d.rearrange("(c p) -> p c", p=128))
        if use_bf2:
            bf2_sb = const.tile([128, ND], F32, name="bf2_sb")
            nc.sync.dma_start(out=bf2_sb, in_=bf2_d.rearrange("(c p) -> p c", p=128))
        if use_mask:
            madd_sb = const.tile([128, NK], F32, name="madd_sb")
            nc.sync.dma_start(out=madd_sb, in_=madd_d.rearrange("(c p) -> p c", p=128))
        if use_bv:
            bv_sb = const.tile([1, D], BF16, name="bv_sb")
            nc.sync.dma_start(out=bv_sb, in_=bv_d.rearrange("(a d) -> a d", a=1))
            ones1b = const.tile([1, 128], BF16, name="ones1b")
            nc.vector.memset(ones1b, 1.0)
        # reciprocal broadcast selectors carry the 64x for the fp8 hT scale.
        # (memset can't write f32r directly; stage in f32 and DVE-round.)
        stg = const.tile([128, 4], F32, name="stg")
        nc.vector.memset(stg[:, 2:3], 1.0 / D)
        indstg = const.tile([1, 128 + HQ], F32, name="indstg")
        nc.vector.memset(indstg, 0.0)
        nc.vector.memset(indstg[0:1, 0:64], WS)
        indE = const.tile([1, 128], F32R, name="indE")
        nc.vector.tensor_copy(out=indE, in_=indstg[0:1, 0:128])
        nc.vector.memset(indstg[0:1, 0:64], 0.0)
        nc.vector.memset(indstg[0:1, 64:128], WS)
        indO = const.tile([1, 128], F32R, name="indO")
        nc.vector.tensor_copy(out=indO, in_=indstg[0:1, 0:128])
        invD128 = const.tile([128, 1], F32R, name="invD128")
        nc.vector.tensor_copy(out=invD128, in_=stg[:, 2:3])
        ones256 = const.tile([1, HQ], F32R, name="ones256")
        nc.vector.memset(indstg[0:1, 128:128 + HQ], 1.0)
        nc.vector.tensor_copy(out=ones256, in_=indstg[0:1, 128:128 + HQ])
        eps_sb = const.tile([1, 1], F32, name="eps_sb")
        nc.vector.memset(eps_sb, EPS)
        actwarm = const.tile([1, 1], F32, name="actwarm")
        nc.scalar.activation(actwarm, eps_sb, AF.Exp)
        # startup queue: phase-A operands first, w1 stream behind
        nc.sync.dma_start(out=wk_sb[:, :, 0:256], in_=wk_d[:, :, 0:256])
        nc.sync.dma_start(out=xk_sb, in_=xkT_d[:])
        nc.sync.dma_start(out=wv_sb[:, :, 0:512], in_=wv_d[:, :, 0:512])
        nc.sync.dma_start(out=wk_sb[:, :, 256:D], in_=wk_d[:, :, 256:D])
        nc.sync.dma_start(out=wv_sb[:, :, 512:D], in_=wv_d[:, :, 512:D])
        nc.sync.dma_start(out=wo_sb, in_=wo_d[:])
        nc.sync.dma_start(out=xqf_sb, in_=xqTf_d[:])

        nc.vector.memset(
            V3.rearrange("p c (h w) -> p (c h) w", w=VW)[:, :, DH:DH + 1], 1.0)

        # ---------------- phase 0: Q projection (fp8 DoubleRow) ----------------
        for tb in _blocks(ND, 2):
            ps = {}
            for t in tb:
                ps[t] = mmp.tile([128, QS], F32, tag="mm", name=f"qtps{t}")
            for k in range(ND // 2):
                for t in tb:
                    nc.tensor.matmul(ps[t],
                                     lhsT=wq_sb[:, 2 * k:2 * k + 2, t * 128:(t + 1) * 128],
                                     rhs=xq_sb[:, 2 * k:2 * k + 2, :],
                                     start=(k == 0), stop=(k == ND // 2 - 1),
                                     perf_mode=PM.DoubleRow)
            for t in tb:
                nc.vector.tensor_scalar(out=QT[:, t, :], in0=ps[t], scalar1=IWS,
                                        scalar2=bq_sb[:, t:t + 1],
                                        op0=OP.mult, op1=OP.add)
        p0.close()

        # ---------------- attention-era task builders ----------------
        def task_kt(t, sc4):
            def run():
                ps = mmp.tile([128, 512], F32, tag="mm", name=f"ktps{t}_{sc4}")
                for k in range(ND // 2):
                    nc.tensor.matmul(ps,
                                     lhsT=wk_sb[:, 2 * k:2 * k + 2, t * 128:(t + 1) * 128],
                                     rhs=xk_sb[:, 2 * k:2 * k + 2, sc4 * 512:(sc4 + 1) * 512],
                                     start=(k == 0), stop=(k == ND // 2 - 1),
                                     perf_mode=PM.DoubleRow)
                nc.vector.tensor_scalar(
                    out=KT[:, t, sc4 * 512:(sc4 + 1) * 512], in0=ps,
                    scalar1=IWS, scalar2=bk_sb[:, t:t + 1], op0=OP.mult, op1=OP.add)
            return run

        def task_v(nh, c):
            def run():
                ps = mmp.tile([128, 512], F32, tag="mm", name=f"vps{nh}_{c}")
                if use_bv:
                    nc.tensor.matmul(ps, lhsT=ones1b,
                                     rhs=bv_sb[:, nh * 512:(nh + 1) * 512],
                                     start=True, stop=False)
                for k in range(ND // 2):
                    nc.tensor.matmul(ps,
                                     lhsT=xk_sb[:, 2 * k:2 * k + 2, c * 128:(c + 1) * 128],
                                     rhs=wv_sb[:, 2 * k:2 * k + 2, nh * 512:(nh + 1) * 512],
                                     start=(k == 0 and not use_bv),
                                     stop=(k == ND // 2 - 1),
                                     perf_mode=PM.DoubleRow)
                out_ap = V3[:, c, :].rearrange("p (h w) -> p h w", w=VW)[:, 8 * nh:8 * nh + 8, 0:DH]
                nc.vector.tensor_scalar_mul(
                    out=out_ap, in0=ps.rearrange("p (h w) -> p h w", w=DH), scalar1=IWS)
            return run

        # ---------------- attention half (pairs over one query half) ---------
        def attention_half(half, tasks, horizon=64, bucketed=False):
            q0 = half * HQ
            qsl = slice(q0, q0 + HQ)
            tailB_pending = []

            def emit_tailB(p, recE, recO, hTu_t):
                rbc = mmp.tile([128, HQ], F32, tag="mm", name=f"rbc{half}_{p}")
                nc.tensor.matmul(rbc, lhsT=indE, rhs=recE, start=True, stop=False)
                nc.tensor.matmul(rbc, lhsT=indO, rhs=recO, start=False, stop=True)
                nc.vector.tensor_mul(out=hT8[:, p, qsl], in0=hTu_t, in1=rbc)

            ntasks = 0 if bucketed else len(tasks)
            ti = 0
            for p in range(NP):
                if bucketed:
                    ptasks = tasks[p]
                    nt = len(ptasks)
                t = p
                av = {}
                av[0] = avp.tile([VW, HQ], F32, tag="av", name=f"av{half}_{p}e")
                av[1] = avp.tile([VW, HQ], F32, tag="av", name=f"av{half}_{p}o")
                pend = []

                def emit_av(i, et):
                    for parity in (0, 1):
                        h = 2 * p + parity
                        nc.tensor.matmul(av[parity],
                                         lhsT=V3[:, 2 * i:2 * i + 2, h * VW:(h + 1) * VW],
                                         rhs=et[:, parity, :, :],
                                         start=(i == 0), stop=(i == 7),
                                         perf_mode=PM.DoubleRow)

                for i in range(8):
                    sct = scp.tile([128, 2, 2, HQ], F32, tag="sc",
                                   name=f"sc{half}_{p}_{i}")
                    for cc in (0, 1):
                        ch = 2 * i + cc
                        for parity in (0, 1):
                            base = 64 * parity
                            nc.tensor.matmul(sct[:, parity, cc, :],
                                             lhsT=KT[base:base + 64, t, ch * 128:(ch + 1) * 128],
                                             rhs=QT[base:base + 64, t, qsl],
                                             start=True, stop=True)
                    et = expp.tile([128, 2, 2, HQ], F8E4, tag="exp",
                                   name=f"ex{half}_{p}_{i}")
                    if use_mask:
                        for cc in (0, 1):
                            ch = 2 * i + cc
                            nc.scalar.activation(et[:, :, cc, :], sct[:, :, cc, :],
                                                 AF.Exp, bias=madd_sb[:, ch:ch + 1],
                                                 scale=0.125)
                    else:
                        nc.scalar.activation(et, sct, AF.Exp, scale=0.125)
                    pend.append((i, et))
                    if len(pend) > 1:
                        emit_av(*pend.pop(0))
                    if i == 4 and tailB_pending:
                        emit_tailB(*tailB_pending.pop(0))
                    if bucketed:
                        for task in ptasks[(nt * i) // 8:(nt * (i + 1)) // 8]:
                            task()
                    else:
                        hi = (ntasks * min(p * 8 + i + 1, horizon)) // horizon
                        while ti < hi:
                            tasks[ti]()
                            ti += 1
                while pend:
                    emit_av(*pend.pop(0))

                denE = recipp.tile([1, HQ], F32, tag="den", bufs=2, name=f"denE{half}_{p}")
                denO = recipp.tile([1, HQ], F32, tag="den", bufs=2, name=f"denO{half}_{p}")
                recEf = recipp.tile([1, HQ], F32, tag="recf", bufs=2, name=f"recEf{half}_{p}")
                recOf = recipp.tile([1, HQ], F32, tag="recf", bufs=2, name=f"recOf{half}_{p}")
                recE = recipp.tile([1, HQ], F32R, tag="rec", bufs=2, name=f"recE{half}_{p}")
                recO = recipp.tile([1, HQ], F32R, tag="rec", bufs=2, name=f"recO{half}_{p}")
                hTu_t = recipp.tile([128, HQ], BF16, tag="htu", bufs=3,
                                    name=f"hTu{half}_{p}")
                if p == NP - 1:
                    nc.scalar.copy(denE, av[0][64:65, :])
                    nc.scalar.copy(denO, av[1][64:65, :])
                    nc.scalar.copy(hTu_t[0:64, :], av[0][0:64, :])
                    nc.scalar.copy(hTu_t[64:128, :], av[1][0:64, :])
                else:
                    nc.vector.tensor_copy(out=hTu_t[0:64, :], in_=av[0][0:64, :])
                    nc.vector.tensor_copy(out=hTu_t[64:128, :], in_=av[1][0:64, :])
                    nc.vector.tensor_copy(out=denE, in_=av[0][64:65, :])
                    nc.vector.tensor_copy(out=denO, in_=av[1][64:65, :])
                nc.vector.reciprocal_approx_fast(out=recEf, in_=denE)
                nc.vector.reciprocal_approx_fast(out=recOf, in_=denO)
                nc.vector.tensor_copy(out=recE, in_=recEf)
                nc.vector.tensor_copy(out=recO, in_=recOf)
                tailB_pending.append((p, recE, recO, hTu_t))
            while tailB_pending:
                emit_tailB(*tailB_pending.pop(0))
            if not bucketed:
                while ti < ntasks:
                    tasks[ti]()
                    ti += 1

        # ---------------- FFN emission (per query half) ----------------
        # Matmul psums flow through the mm ring of the given pool; LN stats
        # accumulate on DVE into SBUF (no long-lived psum).
        def make_ffn_tasks(half, mm_pool, fc_pool, lnsb, wst, resid, h1f, h1bf, gTl,
                           pb, defer_gelu=False):
            # pb: late-bound pools for the post phase (may be filled after
            # this builder runs): pb["mm"], pb["lnsb"], pb["wst"]
            q0 = half * HQ
            qsl = slice(q0, q0 + HQ)
            tg = f"h{half}"
            tasks = []
            st1 = {}
            st2 = {}

            def ln_accum(st, tagn, c, src_f32r, src_name, mmP, lnP):
                stp = mmP.tile([1, HQ], F32, tag="mm",
                               name=f"st{tagn}_{src_name}_{c}")
                nc.tensor.matmul(stp, lhsT=invD128, rhs=src_f32r,
                                 start=True, stop=True)
                if c == 0:
                    acc = lnP.tile([1, HQ], F32, tag=f"acc_{src_name}", bufs=1,
                                   name=f"acc{tagn}_{src_name}")
                    st[src_name] = acc
                    nc.vector.tensor_copy(out=acc, in_=stp)
                else:
                    nc.vector.tensor_add(out=st[src_name], in0=st[src_name], in1=stp)

            def ln_chunk(st, tagn, c, mmP, lnP):
                sq = lnP.tile([128, HQ], F32R, tag="sq", bufs=2, name=f"sq{tagn}_{c}")
                nc.vector.tensor_mul(out=sq, in0=resid[:, c, :], in1=resid[:, c, :])
                ln_accum(st, tagn, c, resid[:, c, :], "s", mmP, lnP)
                ln_accum(st, tagn, c, sq, "q", mmP, lnP)

            def ln_finish(st, g_sb, be_sb, writer, tagn, mmP, lnP):
                u = st["s"]
                var = lnP.tile([1, HQ], F32, tag="var", bufs=1, name=f"var_{tagn}")
                std = lnP.tile([1, HQ], F32, tag="std", bufs=1, name=f"std_{tagn}")
                avecf = lnP.tile([1, HQ], F32, tag="avecf", bufs=1, name=f"avecf_{tagn}")
                avec = lnP.tile([1, HQ], F32R, tag="avec", bufs=1, name=f"avec_{tagn}")
                cvec = lnP.tile([1, HQ], F32R, tag="cvec", bufs=1, name=f"cvec_{tagn}")
                nc.vector.scalar_tensor_tensor(out=var, in0=u, scalar=-1.0, in1=u,
                                               op0=OP.mult, op1=OP.mult)
                nc.vector.tensor_add(out=var, in0=st["q"], in1=var)
                nc.scalar.activation(std, var, AF.Sqrt, bias=eps_sb, scale=1.0)
                nc.vector.reciprocal_approx_fast(out=avecf, in_=std)
                nc.vector.tensor_copy(out=avec, in_=avecf)
                nc.vector.scalar_tensor_tensor(out=cvec, in0=u, scalar=-1.0,
                                               in1=avecf, op0=OP.mult, op1=OP.mult)

                def apply_chunk(c):
                    def run():
                        abc = mmP.tile([128, HQ], F32, tag="mm",
                                       name=f"abc{tagn}_{c}")
                        nc.tensor.matmul(abc, lhsT=g_sb[0:1, c * 128:(c + 1) * 128],
                                         rhs=avec, start=True, stop=True)
                        cbc = mmP.tile([128, HQ], F32, tag="mm",
                                       name=f"cbc{tagn}_{c}")
                        nc.tensor.matmul(cbc, lhsT=g_sb[0:1, c * 128:(c + 1) * 128],
                                         rhs=cvec, start=True, stop=False)
                        nc.tensor.matmul(cbc, lhsT=be_sb[0:1, c * 128:(c + 1) * 128],
                                         rhs=ones256, start=False, stop=True)
                        tmp = lnP.tile([128, HQ], F32, tag="tmp", bufs=2,
                                       name=f"lnt{tagn}_{c}")
                        nc.vector.tensor_mul(out=tmp, in0=resid[:, c, :], in1=abc)
                        writer(c, tmp, cbc)
                    return run
                return [apply_chunk(c) for c in range(ND)]

            # --- o-projection + resid1 + LN1 stats, blocks of 2 ---
            def oproj_block(jb):
                def run():
                    ps = {j: mm_pool.tile([128, HQ], F32, tag="mm",
                                          name=f"op{tg}_{j}") for j in jb}
                    for k in range(ND // 2):
                        for j in jb:
                            nc.tensor.matmul(ps[j],
                                             lhsT=wo_sb[:, 2 * k:2 * k + 2, j * 128:(j + 1) * 128],
                                             rhs=hT8[:, 2 * k:2 * k + 2, qsl],
                                             start=(k == 0), stop=(k == ND // 2 - 1),
                                             perf_mode=PM.DoubleRow)
                    for j in jb:
                        nc.vector.scalar_tensor_tensor(
                            out=resid[:, j, :], in0=ps[j], scalar=IWS2,
                            in1=xqf_sb[:, j, qsl], op0=OP.mult, op1=OP.add)
                        ln_chunk(st1, f"a{tg}", j, mm_pool, lnsb)
                return run
            for jb in _blocks(ND, 2):
                tasks.append(oproj_block(jb))

            def ln1_writer(c, tmp, cbc):
                nc.vector.tensor_add(out=h1f[:, c, :], in0=tmp, in1=cbc)
                nc.vector.tensor_copy(out=h1bf[:, c, :], in_=h1f[:, c, :])

            holder = {}

            def ln1_fin():
                holder["ap1"] = ln_finish(st1, g1_sb, be1_sb, ln1_writer, f"a{tg}",
                                          mm_pool, lnsb)
            tasks.append(ln1_fin)

            def ln1_apply(i):
                def run():
                    holder["ap1"][2 * i]()
                    holder["ap1"][2 * i + 1]()
                return run
            for i in range(ND // 2):
                tasks.append(ln1_apply(i))

            # --- fc1, one out-chunk per task; w1 streamed 2 blocks ahead;
            # z+bf1 stored, gelu deferred for ACT table-set locality ---
            w1tiles = {}

            def w1_prefetch(jb):
                def run():
                    w1tiles[jb] = wst.tile([128, ND, 512], BF16, tag="wst",
                                           name=f"w1t{tg}_{jb}")
                    nc.gpsimd.dma_start(out=w1tiles[jb], in_=w1_d[:, jb, :, :])
                return run

            def fc1_chunk(jg):
                def run():
                    jb, j = jg // 4, jg % 4
                    w1t = w1tiles[jb]
                    ps = fc_pool.tile([128, HQ], F32, tag="mm", name=f"f1{tg}_{jg}")
                    for k in range(ND):
                        nc.tensor.matmul(ps, lhsT=w1t[:, k, j * 128:(j + 1) * 128],
                                         rhs=h1bf[:, k, :],
                                         start=(k == 0), stop=(k == ND - 1))
                    nc.vector.tensor_scalar(out=gTl[:, jg, :], in0=ps,
                                            scalar1=1.0,
                                            scalar2=bf1_sb[:, jg:jg + 1],
                                            op0=OP.mult, op1=OP.add)
                    if j == 3:
                        del w1tiles[jb]
                return run
            fc1_tasks = [w1_prefetch(0), w1_prefetch(1)]
            for jg in range(NF):
                fc1_tasks.append(fc1_chunk(jg))
                if jg % 4 == 1 and jg // 4 + 2 < 8:
                    fc1_tasks.append(w1_prefetch(jg // 4 + 2))

            def gelu_chunk(jg):
                def run():
                    nc.scalar.activation(gTl[:, jg, :], gTl[:, jg, :],
                                         AF.Gelu, scale=1.0)
                return run
            gelu_tasks = [gelu_chunk(jg) for jg in range(NF)]
            pre = tasks + fc1_tasks
            tasks = []

            # --- fc2 + resid2 + LN2 stats; w2 streamed as per-chunk-pair
            # blocks so each task fully drains its psums (2-slot ring safe) ---
            w2tiles = {}

            def w2_prefetch(jp):
                def run():
                    for kh in (0, 1):
                        w2tiles[(jp, kh)] = pb["wst"].tile([128, 16, HQ], BF16,
                                                           tag="wst",
                                                           name=f"w2t{tg}_{jp}_{kh}")
                        nc.gpsimd.dma_start(out=w2tiles[(jp, kh)],
                                            in_=w2_d[:, jp, kh, :, :])
                return run

            def fc2_pair(jp):
                def run():
                    if jp + 1 < 4:
                        w2_prefetch(jp + 1)()
                    ps = {j: pb["mm"].tile([128, HQ], F32, tag="mm",
                                           name=f"f2{tg}_{jp}_{j}")
                          for j in (0, 1)}
                    for kh in (0, 1):
                        w2t = w2tiles.pop((jp, kh))
                        for k16 in range(16):
                            kk = kh * 16 + k16
                            for j in (0, 1):
                                nc.tensor.matmul(
                                    ps[j], lhsT=w2t[:, k16, j * 128:(j + 1) * 128],
                                    rhs=gTl[:, kk, :],
                                    start=(kk == 0), stop=(kk == NF - 1))
                    for j in (0, 1):
                        jg = 2 * jp + j
                        if use_bf2:
                            tmp2 = pb["lnsb"].tile([128, HQ], F32, tag="tmp", bufs=3,
                                                   name=f"f2t{tg}_{jg}")
                            nc.vector.tensor_scalar(out=tmp2, in0=ps[j], scalar1=1.0,
                                                    scalar2=bf2_sb[:, jg:jg + 1],
                                                    op0=OP.mult, op1=OP.add)
                            nc.vector.tensor_add(out=resid[:, jg, :], in0=tmp2,
                                                 in1=h1f[:, jg, :])
                        else:
                            nc.vector.scalar_tensor_tensor(
                                out=resid[:, jg, :], in0=ps[j], scalar=1.0,
                                in1=h1f[:, jg, :], op0=OP.mult, op1=OP.add)
                        ln_chunk(st2, f"b{tg}", jg, pb["mm"], pb["lnsb"])
                return run

            tasks.append(w2_prefetch(0))
            for jp in range(4):
                tasks.append(fc2_pair(jp))

            def ln2_writer(c, tmp, cbc):
                och = pb["lnsb"].tile([128, HQ], F32, tag="out", bufs=2,
                                      name=f"och{tg}_{c}")
                nc.vector.tensor_add(out=och, in0=tmp, in1=cbc)
                nc.sync.dma_start(out=yT_d[c * 128:(c + 1) * 128, qsl], in_=och)

            def ln2_fin():
                holder["ap2"] = ln_finish(st2, g2_sb, be2_sb, ln2_writer, f"b{tg}",
                                          pb["mm"], pb["lnsb"])
            tasks.append(ln2_fin)

            def ln2_apply(i):
                def run():
                    holder["ap2"][2 * i]()
                    holder["ap2"][2 * i + 1]()
                return run
            for i in range(ND // 2):
                tasks.append(ln2_apply(i))
            return {"pre": pre, "gelu": gelu_tasks, "post": tasks}

        # LN gamma/beta as f32r operands
        g1_sb = ffx.tile([1, D], F32R, name="g1_sb")
        be1_sb = ffx.tile([1, D], F32R, name="be1_sb")
        g2_sb = ffx.tile([1, D], F32R, name="g2_sb")
        be2_sb = ffx.tile([1, D], F32R, name="be2_sb")
        for dd, dst in ((g1_d, g1_sb), (be1_d, be1_sb), (g2_d, g2_sb),
                        (be2_d, be2_sb)):
            gbe_stage = ffx.tile([1, D], F32, tag="gbes", bufs=1, name="gbe_stage")
            nc.sync.dma_start(out=gbe_stage, in_=dd.rearrange("(a d) -> a d", a=1))
            nc.vector.tensor_copy(out=dst, in_=gbe_stage)

        residA = ffx.tile([128, ND, HQ], F32R, name="residA")
        h1fA = ffx.tile([128, ND, HQ], F32, name="h1fA")
        h1bfA = ffx.tile([128, ND, HQ], BF16, name="h1bfA")
        gTA = ffx.tile([128, NF, HQ], BF16, name="gTA")

        # ---------------- era A: attention half 0 + K/V production ----------
        # bucket tasks per pair (kt for pair p+2, v-half-1 chunks for pairs<4)
        # so every tile is emitted before its consuming pair
        for t in (0, 1):
            for sc4 in range(4):
                task_kt(t, sc4)()
        for c in range(NK):
            task_v(0, c)()
        kv_sched = []
        for p in range(NP):
            ts = []
            if p < 6:
                for sc4 in range(4):
                    ts.append(task_kt(p + 2, sc4))
            if p < 4:
                for c in range(4 * p, 4 * p + 4):
                    ts.append(task_v(1, c))
            kv_sched.append(ts)
        attention_half(0, kv_sched, bucketed=True)
        xkp_cm.close()

        # ---------------- era B: attention half 1 + FFN half 0 --------------
        pbA = {}
        ffnA = make_ffn_tasks(0, mmp, mmp, lnsbA, wstA, residA,
                              h1fA, h1bfA, gTA, pbA, defer_gelu=True)
        attention_half(1, ffnA["pre"])

        dump("KT", KT)
        dump("V3", V3)
        dump("QT", QT)
        dump("hT8", hT8)
        dump("h1fA", h1fA)
        dump("gTA", gTA)
        dump("residA", residA)
        # ---------------- era C: FFN half 1 ----------------
        att2.close()
        attbig_cm.close()
        att_ps.close()
        ffn_cm = ExitStack()
        ffn2 = ffn_cm.enter_context(tc.tile_pool(name="ffn2", bufs=1))
        pmm = ffn_cm.enter_context(tc.tile_pool(name="pmm", bufs=4, space="PSUM"))
        pmm2 = ffn_cm.enter_context(tc.tile_pool(name="pmm2", bufs=4, space="PSUM"))
        lnsbB = ffn_cm.enter_context(tc.tile_pool(name="lnsbB", bufs=2))
        wstB = ffn_cm.enter_context(tc.tile_pool(name="wstB", bufs=4))
        residB = ffn2.tile([128, ND, HQ], F32R, name="residB")
        h1fB = ffn2.tile([128, ND, HQ], F32, name="h1fB")
        h1bfB = ffn2.tile([128, ND, HQ], BF16, name="h1bfB")
        gTB = ffn2.tile([128, NF, HQ], BF16, name="gTB")
        pbA["mm"] = pmm
        pbA["lnsb"] = lnsbB
        pbA["wst"] = wstB
        pbB = {"mm": pmm2, "lnsb": lnsbB, "wst": wstB}
        ffnB = make_ffn_tasks(1, pmm2, pmm2, lnsbB, wstB, residB,
                              h1fB, h1bfB, gTB, pbB)

        def zip_run(a, b):
            ia = ib = 0
            na, nb = len(a), len(b)
            while ia < na or ib < nb:
                if ia < na and (ib >= nb or ia * nb <= ib * na):
                    a[ia]()
                    ia += 1
                elif ib < nb:
                    b[ib]()
                    ib += 1

        # phase 1: half-0 gelus on ACT while the PE runs half-1 oproj/LN1/fc1
        zip_run(ffnA["gelu"], ffnB["pre"])
        # phase 2: half-1 gelus on ACT while the PE runs half-0 fc2 (+LN2 stats)
        zip_run(ffnB["gelu"], ffnA["post"])
        # phase 3: half-1 fc2 + LN2 + output
        for task in ffnB["post"]:
            task()
        dump("h1fB", h1fB)
        dump("gTB", gTB)
        ffn_cm.close()

    nc.compile()
    return nc


_CACHE = {}


def _get_built(use_mask, use_bv, use_bf2=False, dbg=False):
    key = (use_mask, use_bv, use_bf2, dbg)
    if key not in _CACHE:
        _CACHE[key] = _build(use_mask, use_bv, use_bf2, dbg)
    return _CACHE[key]


def kernel(x, mask, wq, bq, wk, bk, wv, bv, wo, bo, g1, be1, w1, bf1, w2, bf2, g2, be2):
    bf = ml_dtypes.bfloat16
    e4 = ml_dtypes.float8_e4m3
    f4 = np.float32
    x = np.asarray(x, f4)
    madd_full = (-10000.0 * (1.0 - np.asarray(mask).astype(f4)))  # [B, S]
    use_mask = bool((madd_full != 0.0).any())
    use_bv = bool(np.any(np.asarray(bv) != 0))
    use_bf2 = bool(np.any(np.asarray(bf2) != 0))
    nc = _get_built(use_mask, use_bv, use_bf2)

    def tile_w(w, dt, scale=1.0):
        # [D, N] -> [128, D/128, N]
        w = (np.asarray(w, f4) * scale).astype(dt)
        return np.ascontiguousarray(w.reshape(-1, 128, w.shape[1]).transpose(1, 0, 2))

    w1h = np.asarray(w1, f4).astype(bf).reshape(ND, 128, 8, 512).transpose(1, 2, 0, 3)
    w2h = np.asarray(w2, f4).astype(bf).reshape(2, 16, 128, 4, 256).transpose(2, 3, 0, 1, 4)
    shared = {
        "wq": tile_w(wq, e4, WS),
        "wk": tile_w(wk, e4, WS),
        "wv": tile_w(wv, e4, WS),
        "wo": tile_w(wo, e4, WS),
        "w1": np.ascontiguousarray(w1h),
        "w2": np.ascontiguousarray(w2h),
        "bq": np.asarray(bq, f4), "bk": np.asarray(bk, f4),
        "bf1": np.asarray(bf1, f4),
        "g1": np.asarray(g1, f4),
        "be1": np.asarray(be1, f4), "g2": np.asarray(g2, f4),
        "be2": np.asarray(be2, f4),
    }
    if use_bv:
        shared["bv"] = (np.asarray(bv, f4) * WS).astype(bf)
    if use_bf2:
        shared["bf2"] = np.asarray(bf2, f4)

    # [D, S] -> [128, ND, S] pre-tiled transposes
    bo_f = np.asarray(bo, f4)
    xTt = {b: np.ascontiguousarray(
        x[b].T.reshape(ND, 128, S).transpose(1, 0, 2)) for b in range(B)}
    xTt_8 = {b: xTt[b].astype(e4) for b in range(B)}
    bo_t = bo_f.reshape(ND, 128).T[:, :, None]          # [128, ND, 1]
    in_maps = []
    for c in range(NCORES):
        b, q0 = c // 4, (c % 4) * QS
        m = dict(shared)
        m["xkT"] = xTt_8[b]
        m["xqT"] = np.ascontiguousarray(xTt_8[b][:, :, q0:q0 + QS])
        m["xqTf"] = np.ascontiguousarray(xTt[b][:, :, q0:q0 + QS] + bo_t).astype(bf)
        if use_mask:
            m["madd"] = np.ascontiguousarray(madd_full[b])
        in_maps.append(m)

    res = run_bass_kernel_spmd(nc, in_maps, core_ids=list(range(NCORES)))
    kernel.last_result = res
    if res.exec_time_ns is not None:
        print(f"HW exec time: {res.exec_time_ns} ns")

    y = np.empty((B, S, D), np.float32)
    for c in range(NCORES):
        b, q0 = c // 4, (c % 4) * QS
        y[b, q0:q0 + QS, :] = np.asarray(res.results[c]["yT"], np.float32).T
    return y


# revision 34
# speedup vs baseline: 1.1897x; 1.0012x over previous
"""Trainium2 Bass/Tile kernel for a dense transformer block.

B=2, S=2048, D=1024, H=16 heads (dh=64), FF=4096, f32 IO.

Sharding: 8 cores = (2 batches) x (4 query-slices of 512 tokens), zero
cross-core communication (K/V recomputed per core).

v3: fp8 attention + split-query software pipeline.  All attention GEMMs
(Q/K/V/O projections, AV) are fp8e4m3 DoubleRow matmuls (0.5 PE
cycles/row, weights host-scaled by 64); scores run on fp8 Q/K (the
1/sqrt(dh) folds into the softmax exp scale); exp writes fp8 directly.
The FFN stays bf16 (fp8 FFN breaches the 2e-2 gate).  LayerNorm stats /
broadcast matmuls use float32r operands (1 cycle/row vs 4 for fp32).

The queries are processed in two 256-wide halves: era A runs attention
for half 0 (plus all K/V production), era B runs attention for half 1
with the ENTIRE half-0 FFN interleaved into its pair loop (softmax exp
keeps ACT busy while the FFN matmuls fill the PE), era C finishes with
the half-1 FFN.  LN statistics accumulate via DVE adds into SBUF so the
interleaved-FFN matmuls can share a single 2-slot PSUM ring with the
reciprocal broadcasts.
"""

import os
from contextlib import ExitStack

import numpy as np
import ml_dtypes

import concourse.bass as bass
import concourse.tile as tile
from concourse import bacc, mybir
from concourse.bass_utils import run_bass_kernel_spmd

BF16 = mybir.dt.bfloat16
F32 = mybir.dt.float32
F32R = mybir.dt.float32r
F8E4 = mybir.dt.float8e4
AF = mybir.ActivationFunctionType
OP = mybir.AluOpType
PM = mybir.MatmulPerfMode

B, S, D, H, FF = 2, 2048, 1024, 16, 4096
DH = D // H            # 64
NCORES = 8
QS = S // 4            # 512 queries per core
HQ = QS // 2           # 256-query pipeline half
NK = S // 128          # 16 key chunks
ND = D // 128          # 8 feature chunks
NF = FF // 128         # 32 ff chunks
NP = H // 2            # 8 head pairs
VW = DH + 1            # 65 = head width + ones column
EPS = 1e-12
WS = 64.0              # host-side fp8 weight scale
IWS = 1.0 / WS
IWS2 = IWS * IWS


def _blocks(n, w):
    return [list(range(i, min(i + w, n))) for i in range(0, n, w)]


def _build(use_mask, use_bv, use_bf2, dbg=False):
    nc = bacc.Bacc("TRN2", target_bir_lowering=False, debug=False)

    def din(name, shape, dtype):
        return nc.dram_tensor(name, shape, dtype, kind="ExternalInput").ap()

    def dump(name, tl):
        if not dbg:
            return
        dd = nc.dram_tensor(f"dbg_{name}", list(tl.shape), tl.dtype,
                            kind="ExternalOutput").ap()
        nc.sync.dma_start(out=dd, in_=tl)

    xkT_d = din("xkT", [128, ND, S], F8E4)
    xqT_d = din("xqT", [128, ND, QS], F8E4)
    xqTf_d = din("xqTf", [128, ND, QS], BF16)     # x slice, +bo folded in
    wq_d = din("wq", [128, ND, D], F8E4)          # pre-scaled by WS on host
    wk_d = din("wk", [128, ND, D], F8E4)
    wv_d = din("wv", [128, ND, D], F8E4)
    wo_d = din("wo", [128, ND, D], F8E4)
    w1_d = din("w1", [128, 8, ND, 512], BF16)     # [p, jb, k, n]
    w2_d = din("w2", [128, 4, 2, 16, 256], BF16)  # [p, jpair, kh, k16, n]
    bq_d = din("bq", [D], F32)
    bk_d = din("bk", [D], F32)
    bf1_d = din("bf1", [FF], F32)
    g1_d = din("g1", [D], F32)
    be1_d = din("be1", [D], F32)
    g2_d = din("g2", [D], F32)
    be2_d = din("be2", [D], F32)
    bf2_d = din("bf2", [D], F32) if use_bf2 else None
    bv_d = din("bv", [D], BF16) if use_bv else None     # pre-scaled by WS
    madd_d = din("madd", [S], F32) if use_mask else None
    yT_d = nc.dram_tensor("yT", [D, QS], F32, kind="ExternalOutput").ap()

    with tile.TileContext(nc) as tc, ExitStack() as glob:
        const = glob.enter_context(tc.tile_pool(name="const", bufs=1))
        gx = glob.enter_context(tc.tile_pool(name="gx", bufs=1))
        ffx = glob.enter_context(tc.tile_pool(name="ffx", bufs=1))
        att_ps = ExitStack()
        avp = att_ps.enter_context(tc.tile_pool(name="avp", bufs=2, space="PSUM"))
        mmp = att_ps.enter_context(tc.tile_pool(name="mmp", bufs=2, space="PSUM"))
        scp = att_ps.enter_context(tc.tile_pool(name="scp", bufs=2, space="PSUM"))

        # ---------------- big tiles + front-loaded DMAs ----------------
        hT8 = gx.tile([128, ND, QS], F8E4, name="hT8")          # 64*h, attn out
        wo_sb = gx.tile([128, ND, D], F8E4, name="wo_sb")
        xqf_sb = gx.tile([128, ND, QS], BF16, name="xqf_sb")
        attbig_cm = ExitStack()
        attbig = attbig_cm.enter_context(tc.tile_pool(name="attbig", bufs=1))
        KT = attbig.tile([128, ND, S], F8E4, name="KT")         # fp8(k + bk)
        V3 = attbig.tile([128, NK, H * VW], F8E4, name="V3")    # V + ones cols
        QT = attbig.tile([128, ND, QS], F8E4, name="QT")
        att2 = ExitStack()
        recipp = att2.enter_context(tc.tile_pool(name="recipp", bufs=2))
        expp = att2.enter_context(tc.tile_pool(name="expp", bufs=4))
        lnsbA = att2.enter_context(tc.tile_pool(name="lnsbA", bufs=2))
        wstA = att2.enter_context(tc.tile_pool(name="wstA", bufs=2))
        xkp_cm = ExitStack()
        xkp = xkp_cm.enter_context(tc.tile_pool(name="xkp", bufs=1))
        xk_sb = xkp.tile([128, ND, S], F8E4, name="xk_sb")
        wk_sb = xkp.tile([128, ND, D], F8E4, name="wk_sb")
        wv_sb = xkp.tile([128, ND, D], F8E4, name="wv_sb")
        p0 = ExitStack()
        p0pool = p0.enter_context(tc.tile_pool(name="p0pool", bufs=1))
        wq_sb = p0pool.tile([128, ND, D], F8E4, name="wq_sb")
        xq_sb = p0pool.tile([128, ND, QS], F8E4, name="xq_sb")
        nc.sync.dma_start(out=xq_sb, in_=xqT_d[:])
        nc.sync.dma_start(out=wq_sb[:, :, 0:512], in_=wq_d[:, :, 0:512])
        nc.sync.dma_start(out=wq_sb[:, :, 512:D], in_=wq_d[:, :, 512:D])
        # ---------------- constants & small params ----------------
        bq_sb = const.tile([128, ND], F32, name="bq_sb")
        nc.sync.dma_start(out=bq_sb, in_=bq_d.rearrange("(c p) -> p c", p=128))
        bk_sb = const.tile([128, ND], F32, name="bk_sb")
        nc.sync.dma_start(out=bk_sb, in_=bk_d.rearrange("(c p) -> p c", p=128))
        bf1_sb = const.tile([128, NF], F32, name="bf1_sb")
        nc.sync.dma_start(out=bf1_sb, in_=bf1_d.rearrange("(c p) -> p c", p=128))
        if use_bf2:
            bf2_sb = const.tile([128, ND], F32, name="bf2_sb")
            nc.sync.dma_start(out=bf2_sb, in_=bf2_d.rearrange("(c p) -> p c", p=128))
        if use_mask:
            madd_sb = const.tile([128, NK], F32, name="madd_sb")
            nc.sync.dma_start(out=madd_sb, in_=madd_d.rearrange("(c p) -> p c", p=128))
        if use_bv:
            bv_sb = const.tile([1, D], BF16, name="bv_sb")
            nc.sync.dma_start(out=bv_sb, in_=bv_d.rearrange("(a d) -> a d", a=1))
            ones1b = const.tile([1, 128], BF16, name="ones1b")
            nc.vector.memset(ones1b, 1.0)
        # reciprocal broadcast selectors carry the 64x for the fp8 hT scale.
        # (memset can't write f32r directly; stage in f32 and DVE-round.)
        stg = const.tile([128, 4], F32, name="stg")
        nc.vector.memset(stg[:, 2:3], 1.0 / D)
        indstg = const.tile([1, 128 + HQ], F32, name="indstg")
        nc.vector.memset(indstg, 0.0)
        nc.vector.memset(indstg[0:1, 0:64], WS)
        indE = const.tile([1, 128], F32R, name="indE")
        nc.vector.tensor_copy(out=indE, in_=indstg[0:1, 0:128])
        nc.vector.memset(indstg[0:1, 0:64], 0.0)
        nc.vector.memset(indstg[0:1, 64:128], WS)
        indO = const.tile([1, 128], F32R, name="indO")
        nc.vector.tensor_copy(out=indO, in_=indstg[0:1, 0:128])
        invD128 = const.tile([128, 1], F32R, name="invD128")
        nc.vector.tensor_copy(out=invD128, in_=stg[:, 2:3])
        ones256 = const.tile([1, HQ], F32R, name="ones256")
        nc.vector.memset(indstg[0:1, 128:128 + HQ], 1.0)
        nc.vector.tensor_copy(out=ones256, in_=indstg[0:1, 128:128 + HQ])
        eps_sb = const.tile([1, 1], F32, name="eps_sb")
        nc.vector.memset(eps_sb, EPS)
        actwarm = const.tile([1, 1], F32, name="actwarm")
        nc.scalar.activation(actwarm, eps_sb, AF.Exp)
        # startup queue: phase-A operands first, w1 stream behind
        nc.sync.dma_start(out=wk_sb[:, :, 0:256], in_=wk_d[:, :, 0:256])
        nc.sync.dma_start(out=xk_sb, in_=xkT_d[:])
        nc.sync.dma_start(out=wv_sb[:, :, 0:512], in_=wv_d[:, :, 0:512])
        nc.sync.dma_start(out=wk_sb[:, :, 256:D], in_=wk_d[:, :, 256:D])
        nc.sync.dma_start(out=wv_sb[:, :, 512:D], in_=wv_d[:, :, 512:D])
        nc.sync.dma_start(out=wo_sb, in_=wo_d[:])
        nc.sync.dma_start(out=xqf_sb, in_=xqTf_d[:])

        nc.vector.memset(
            V3.rearrange("p c (h w) -> p (c h) w", w=VW)[:, :, DH:DH + 1], 1.0)

        # ---------------- phase 0: Q projection (fp8 DoubleRow) ----------------
        for tb in _blocks(ND, 2):
            ps = {}
            for t in tb:
                ps[t] = mmp.tile([128, QS], F32, tag="mm", name=f"qtps{t}")
            for k in range(ND // 2):
                for t in tb:
                    nc.tensor.matmul(ps[t],
                                     lhsT=wq_sb[:, 2 * k:2 * k + 2, t * 128:(t + 1) * 128],
                                     rhs=xq_sb[:, 2 * k:2 * k + 2, :],
                                     start=(k == 0), stop=(k == ND // 2 - 1),
                                     perf_mode=PM.DoubleRow)
            for t in tb:
                nc.vector.tensor_scalar(out=QT[:, t, :], in0=ps[t], scalar1=IWS,
                                        scalar2=bq_sb[:, t:t + 1],
                                        op0=OP.mult, op1=OP.add)
        p0.close()

        # ---------------- attention-era task builders ----------------
        def task_kt(t, sc4):
            def run():
                ps = mmp.tile([128, 512], F32, tag="mm", name=f"ktps{t}_{sc4}")
                for k in range(ND // 2):
                    nc.tensor.matmul(ps,
                                     lhsT=wk_sb[:, 2 * k:2 * k + 2, t * 128:(t + 1) * 128],
                                     rhs=xk_sb[:, 2 * k:2 * k + 2, sc4 * 512:(sc4 + 1) * 512],
                                     start=(k == 0), stop=(k == ND // 2 - 1),
                                     perf_mode=PM.DoubleRow)
                nc.vector.tensor_scalar(
                    out=KT[:, t, sc4 * 512:(sc4 + 1) * 512], in0=ps,
                    scalar1=IWS, scalar2=bk_sb[:, t:t + 1], op0=OP.mult, op1=OP.add)
            return run

        def task_v(nh, c):
            def run():
                ps = mmp.tile([128, 512], F32, tag="mm", name=f"vps{nh}_{c}")
                if use_bv:
                    nc.tensor.matmul(ps, lhsT=ones1b,
                                     rhs=bv_sb[:, nh * 512:(nh + 1) * 512],
                                     start=True, stop=False)
                for k in range(ND // 2):
                    nc.tensor.matmul(ps,
                                     lhsT=xk_sb[:, 2 * k:2 * k + 2, c * 128:(c + 1) * 128],
                                     rhs=wv_sb[:, 2 * k:2 * k + 2, nh * 512:(nh + 1) * 512],
                                     start=(k == 0 and not use_bv),
                                     stop=(k == ND // 2 - 1),
                                     perf_mode=PM.DoubleRow)
                out_ap = V3[:, c, :].rearrange("p (h w) -> p h w", w=VW)[:, 8 * nh:8 * nh + 8, 0:DH]
                nc.vector.tensor_scalar_mul(
                    out=out_ap, in0=ps.rearrange("p (h w) -> p h w", w=DH), scalar1=IWS)
            return run

        # ---------------- attention half (pairs over one query half) ---------
        def attention_half(half, tasks, horizon=64, bucketed=False):
            q0 = half * HQ
            qsl = slice(q0, q0 + HQ)
            tailB_pending = []

            def emit_tailB(p, recE, recO, hTu_t):
                rbc = mmp.tile([128, HQ], F32, tag="mm", name=f"rbc{half}_{p}")
                nc.tensor.matmul(rbc, lhsT=indE, rhs=recE, start=True, stop=False)
                nc.tensor.matmul(rbc, lhsT=indO, rhs=recO, start=False, stop=True)
                nc.vector.tensor_mul(out=hT8[:, p, qsl], in0=hTu_t, in1=rbc)

            ntasks = 0 if bucketed else len(tasks)
            ti = 0
            for p in range(NP):
                if bucketed:
                    ptasks = tasks[p]
                    nt = len(ptasks)
                t = p
                av = {}
                av[0] = avp.tile([VW, HQ], F32, tag="av", name=f"av{half}_{p}e")
                av[1] = avp.tile([VW, HQ], F32, tag="av", name=f"av{half}_{p}o")
                pend = []

                def emit_av(i, et):
                    for parity in (0, 1):
                        h = 2 * p + parity
                        nc.tensor.matmul(av[parity],
                                         lhsT=V3[:, 2 * i:2 * i + 2, h * VW:(h + 1) * VW],
                                         rhs=et[:, parity, :, :],
                                         start=(i == 0), stop=(i == 7),
                                         perf_mode=PM.DoubleRow)

                for i in range(8):
                    sct = scp.tile([128, 2, 2, HQ], F32, tag="sc",
                                   name=f"sc{half}_{p}_{i}")
                    for cc in (0, 1):
                        ch = 2 * i + cc
                        for parity in (0, 1):
                            base = 64 * parity
                            nc.tensor.matmul(sct[:, parity, cc, :],
                                             lhsT=KT[base:base + 64, t, ch * 128:(ch + 1) * 128],
                                             rhs=QT[base:base + 64, t, qsl],
                                             start=True, stop=True)
                    et = expp.tile([128, 2, 2, HQ], F8E4, tag="exp",
                                   name=f"ex{half}_{p}_{i}")
                    if use_mask:
                        for cc in (0, 1):
                            ch = 2 * i + cc
                            nc.scalar.activation(et[:, :, cc, :], sct[:, :, cc, :],
                                                 AF.Exp, bias=madd_sb[:, ch:ch + 1],
                                                 scale=0.125)
                    else:
                        nc.scalar.activation(et, sct, AF.Exp, scale=0.125)
                    pend.append((i, et))
                    if len(pend) > 1:
                        emit_av(*pend.pop(0))
                    if i == 4 and tailB_pending:
                        emit_tailB(*tailB_pending.pop(0))
                    if bucketed:
                        for task in ptasks[(nt * i) // 8:(nt * (i + 1)) // 8]:
                            task()
                    else:
                        hi = (ntasks * min(p * 8 + i + 1, horizon)) // horizon
                        while ti < hi:
                            tasks[ti]()
                            ti += 1
                while pend:
                    emit_av(*pend.pop(0))

                denE = recipp.tile([1, HQ], F32, tag="den", bufs=2, name=f"denE{half}_{p}")
                denO = recipp.tile([1, HQ], F32, tag="den", bufs=2, name=f"denO{half}_{p}")
                recEf = recipp.tile([1, HQ], F32, tag="recf", bufs=2, name=f"recEf{half}_{p}")
                recOf = recipp.tile([1, HQ], F32, tag="recf", bufs=2, name=f"recOf{half}_{p}")
                recE = recipp.tile([1, HQ], F32R, tag="rec", bufs=2, name=f"recE{half}_{p}")
                recO = recipp.tile([1, HQ], F32R, tag="rec", bufs=2, name=f"recO{half}_{p}")
                hTu_t = recipp.tile([128, HQ], BF16, tag="htu", bufs=3,
                                    name=f"hTu{half}_{p}")
                if p == NP - 1:
                    nc.scalar.copy(denE, av[0][64:65, :])
                    nc.scalar.copy(denO, av[1][64:65, :])
                    nc.scalar.copy(hTu_t[0:64, :], av[0][0:64, :])
                    nc.scalar.copy(hTu_t[64:128, :], av[1][0:64, :])
                else:
                    nc.vector.tensor_copy(out=hTu_t[0:64, :], in_=av[0][0:64, :])
                    nc.vector.tensor_copy(out=hTu_t[64:128, :], in_=av[1][0:64, :])
                    nc.vector.tensor_copy(out=denE, in_=av[0][64:65, :])
                    nc.vector.tensor_copy(out=denO, in_=av[1][64:65, :])
                nc.vector.reciprocal_approx_fast(out=recEf, in_=denE)
                nc.vector.reciprocal_approx_fast(out=recOf, in_=denO)
                nc.vector.tensor_copy(out=recE, in_=recEf)
                nc.vector.tensor_copy(out=recO, in_=recOf)
                tailB_pending.append((p, recE, recO, hTu_t))
            while tailB_pending:
                emit_tailB(*tailB_pending.pop(0))
            if not bucketed:
                while ti < ntasks:
                    tasks[ti]()
                    ti += 1

        # ---------------- FFN emission (per query half) ----------------
        # Matmul psums flow through the mm ring of the given pool; LN stats
        # accumulate on DVE into SBUF (no long-lived psum).
        def make_ffn_tasks(half, mm_pool, fc_pool, lnsb, wst, resid, h1f, gTl,
                           pb, defer_gelu=False):
            # pb: late-bound pools for the post phase (may be filled after
            # this builder runs): pb["mm"], pb["lnsb"], pb["wst"]
            q0 = half * HQ
            qsl = slice(q0, q0 + HQ)
            tg = f"h{half}"
            tasks = []
            st1 = {}
            st2 = {}

            def ln_accum(st, tagn, c, src_f32r, src_name, mmP, lnP):
                stp = mmP.tile([1, HQ], F32, tag="mm",
                               name=f"st{tagn}_{src_name}_{c}")
                nc.tensor.matmul(stp, lhsT=invD128, rhs=src_f32r,
                                 start=True, stop=True)
                if c == 0:
                    acc = lnP.tile([1, HQ], F32, tag=f"acc_{src_name}", bufs=1,
                                   name=f"acc{tagn}_{src_name}")
                    st[src_name] = acc
                    nc.vector.tensor_copy(out=acc, in_=stp)
                else:
                    nc.vector.tensor_add(out=st[src_name], in0=st[src_name], in1=stp)

            def ln_chunk(st, tagn, c, mmP, lnP):
                sq = lnP.tile([128, HQ], F32R, tag="sq", bufs=2, name=f"sq{tagn}_{c}")
                nc.vector.tensor_mul(out=sq, in0=resid[:, c, :], in1=resid[:, c, :])
                ln_accum(st, tagn, c, resid[:, c, :], "s", mmP, lnP)
                ln_accum(st, tagn, c, sq, "q", mmP, lnP)

            def ln_finish(st, g_sb, be_sb, writer, tagn, mmP, lnP):
                u = st["s"]
                var = lnP.tile([1, HQ], F32, tag="var", bufs=1, name=f"var_{tagn}")
                std = lnP.tile([1, HQ], F32, tag="std", bufs=1, name=f"std_{tagn}")
                avecf = lnP.tile([1, HQ], F32, tag="avecf", bufs=1, name=f"avecf_{tagn}")
                avec = lnP.tile([1, HQ], F32R, tag="avec", bufs=1, name=f"avec_{tagn}")
                cvec = lnP.tile([1, HQ], F32R, tag="cvec", bufs=1, name=f"cvec_{tagn}")
                nc.vector.scalar_tensor_tensor(out=var, in0=u, scalar=-1.0, in1=u,
                                               op0=OP.mult, op1=OP.mult)
                nc.vector.tensor_add(out=var, in0=st["q"], in1=var)
                nc.scalar.activation(std, var, AF.Sqrt, bias=eps_sb, scale=1.0)
                nc.vector.reciprocal_approx_fast(out=avecf, in_=std)
                nc.vector.tensor_copy(out=avec, in_=avecf)
                nc.vector.scalar_tensor_tensor(out=cvec, in0=u, scalar=-1.0,
                                               in1=avecf, op0=OP.mult, op1=OP.mult)

                def apply_chunk(c):
                    def run():
                        abc = mmP.tile([128, HQ], F32, tag="mm",
                                       name=f"abc{tagn}_{c}")
                        nc.tensor.matmul(abc, lhsT=g_sb[0:1, c * 128:(c + 1) * 128],
                                         rhs=avec, start=True, stop=True)
                        cbc = mmP.tile([128, HQ], F32, tag="mm",
                                       name=f"cbc{tagn}_{c}")
                        nc.tensor.matmul(cbc, lhsT=g_sb[0:1, c * 128:(c + 1) * 128],
                                         rhs=cvec, start=True, stop=False)
                        nc.tensor.matmul(cbc, lhsT=be_sb[0:1, c * 128:(c + 1) * 128],
                                         rhs=ones256, start=False, stop=True)
                        tmp = lnP.tile([128, HQ], F32, tag="tmp", bufs=2,
                                       name=f"lnt{tagn}_{c}")
                        nc.vector.tensor_mul(out=tmp, in0=resid[:, c, :], in1=abc)
                        writer(c, tmp, cbc)
                    return run
                return [apply_chunk(c) for c in range(ND)]

            # --- o-projection + resid1 + LN1 stats, blocks of 2 ---
            def oproj_block(jb):
                def run():
                    ps = {j: mm_pool.tile([128, HQ], F32, tag="mm",
                                          name=f"op{tg}_{j}") for j in jb}
                    for k in range(ND // 2):
                        for j in jb:
                            nc.tensor.matmul(ps[j],
                                             lhsT=wo_sb[:, 2 * k:2 * k + 2, j * 128:(j + 1) * 128],
                                             rhs=hT8[:, 2 * k:2 * k + 2, qsl],
                                             start=(k == 0), stop=(k == ND // 2 - 1),
                                             perf_mode=PM.DoubleRow)
                    for j in jb:
                        nc.vector.scalar_tensor_tensor(
                            out=resid[:, j, :], in0=ps[j], scalar=IWS2,
                            in1=xqf_sb[:, j, qsl], op0=OP.mult, op1=OP.add)
                        ln_chunk(st1, f"a{tg}", j, mm_pool, lnsb)
                return run
            for jb in _blocks(ND, 2):
                tasks.append(oproj_block(jb))

            def ln1_writer(c, tmp, cbc):
                nc.vector.tensor_add(out=h1f[:, c, :], in0=tmp, in1=cbc)

            holder = {}

            def ln1_fin():
                holder["ap1"] = ln_finish(st1, g1_sb, be1_sb, ln1_writer, f"a{tg}",
                                          mm_pool, lnsb)
            tasks.append(ln1_fin)

            def ln1_apply(i):
                def run():
                    holder["ap1"][2 * i]()
                    holder["ap1"][2 * i + 1]()
                return run
            for i in range(ND // 2):
                tasks.append(ln1_apply(i))

            # --- fc1, one out-chunk per task; w1 streamed 2 blocks ahead;
            # z+bf1 stored, gelu deferred for ACT table-set locality ---
            w1tiles = {}

            def w1_prefetch(jb):
                def run():
                    w1tiles[jb] = wst.tile([128, ND, 512], BF16, tag="wst",
                                           name=f"w1t{tg}_{jb}")
                    nc.gpsimd.dma_start(out=w1tiles[jb], in_=w1_d[:, jb, :, :])
                return run

            def fc1_chunk(jg):
                def run():
                    jb, j = jg // 4, jg % 4
                    w1t = w1tiles[jb]
                    ps = fc_pool.tile([128, HQ], F32, tag="mm", name=f"f1{tg}_{jg}")
                    for k in range(ND):
                        nc.tensor.matmul(ps, lhsT=w1t[:, k, j * 128:(j + 1) * 128],
                                         rhs=h1f[:, k, :],
                                         start=(k == 0), stop=(k == ND - 1))
                    if defer_gelu:
                        nc.vector.tensor_scalar(out=gTl[:, jg, :], in0=ps,
                                                scalar1=1.0,
                                                scalar2=bf1_sb[:, jg:jg + 1],
                                                op0=OP.mult, op1=OP.add)
                    else:
                        nc.scalar.activation(gTl[:, jg, :], ps, AF.Gelu,
                                             bias=bf1_sb[:, jg:jg + 1], scale=1.0)
                    if j == 3:
                        del w1tiles[jb]
                return run
            fc1_tasks = [w1_prefetch(0), w1_prefetch(1)]
            for jg in range(NF):
                fc1_tasks.append(fc1_chunk(jg))
                if jg % 4 == 1 and jg // 4 + 2 < 8:
                    fc1_tasks.append(w1_prefetch(jg // 4 + 2))

            def gelu_chunk(jg):
                def run():
                    nc.scalar.activation(gTl[:, jg, :], gTl[:, jg, :],
                                         AF.Gelu, scale=1.0)
                return run
            gelu_tasks = [gelu_chunk(jg) for jg in range(NF)] if defer_gelu else []
            pre = tasks + fc1_tasks
            tasks = []

            # --- fc2 + resid2 + LN2 stats; w2 streamed as per-chunk-pair
            # blocks so each task fully drains its psums (2-slot ring safe) ---
            w2tiles = {}

            def w2_prefetch(jp):
                def run():
                    for kh in (0, 1):
                        w2tiles[(jp, kh)] = pb["wst"].tile([128, 16, HQ], BF16,
                                                           tag="wst",
                                                           name=f"w2t{tg}_{jp}_{kh}")
                        nc.gpsimd.dma_start(out=w2tiles[(jp, kh)],
                                            in_=w2_d[:, jp, kh, :, :])
                return run

            def fc2_pair(jp):
                def run():
                    if jp + 1 < 4:
                        w2_prefetch(jp + 1)()
                    ps = {j: pb["mm"].tile([128, HQ], F32, tag="mm",
                                           name=f"f2{tg}_{jp}_{j}")
                          for j in (0, 1)}
                    for kh in (0, 1):
                        w2t = w2tiles.pop((jp, kh))
                        for k16 in range(16):
                            kk = kh * 16 + k16
                            for j in (0, 1):
                                nc.tensor.matmul(
                                    ps[j], lhsT=w2t[:, k16, j * 128:(j + 1) * 128],
                                    rhs=gTl[:, kk, :],
                                    start=(kk == 0), stop=(kk == NF - 1))
                    for j in (0, 1):
                        jg = 2 * jp + j
                        if use_bf2:
                            tmp2 = pb["lnsb"].tile([128, HQ], F32, tag="tmp", bufs=3,
                                                   name=f"f2t{tg}_{jg}")
                            nc.vector.tensor_scalar(out=tmp2, in0=ps[j], scalar1=1.0,
                                                    scalar2=bf2_sb[:, jg:jg + 1],
                                                    op0=OP.mult, op1=OP.add)
                            nc.vector.tensor_add(out=resid[:, jg, :], in0=tmp2,
                                                 in1=h1f[:, jg, :])
                        else:
                            nc.vector.scalar_tensor_tensor(
                                out=resid[:, jg, :], in0=ps[j], scalar=1.0,
                                in1=h1f[:, jg, :], op0=OP.mult, op1=OP.add)
                        ln_chunk(st2, f"b{tg}", jg, pb["mm"], pb["lnsb"])
                return run

            tasks.append(w2_prefetch(0))
            for jp in range(4):
                tasks.append(fc2_pair(jp))

            def ln2_writer(c, tmp, cbc):
                och = pb["lnsb"].tile([128, HQ], F32, tag="out", bufs=2,
                                      name=f"och{tg}_{c}")
                nc.vector.tensor_add(out=och, in0=tmp, in1=cbc)
                nc.sync.dma_start(out=yT_d[c * 128:(c + 1) * 128, qsl], in_=och)

            def ln2_fin():
                holder["ap2"] = ln_finish(st2, g2_sb, be2_sb, ln2_writer, f"b{tg}",
                                          pb["mm"], pb["lnsb"])
            tasks.append(ln2_fin)

            def ln2_apply(i):
                def run():
                    holder["ap2"][2 * i]()
                    holder["ap2"][2 * i + 1]()
                return run
            for i in range(ND // 2):
                tasks.append(ln2_apply(i))
            return {"pre": pre, "gelu": gelu_tasks, "post": tasks}

        # LN gamma/beta as f32r operands
        g1_sb = ffx.tile([1, D], F32R, name="g1_sb")
        be1_sb = ffx.tile([1, D], F32R, name="be1_sb")
        g2_sb = ffx.tile([1, D], F32R, name="g2_sb")
        be2_sb = ffx.tile([1, D], F32R, name="be2_sb")
        for dd, dst in ((g1_d, g1_sb), (be1_d, be1_sb), (g2_d, g2_sb),
                        (be2_d, be2_sb)):
            gbe_stage = ffx.tile([1, D], F32, tag="gbes", bufs=1, name="gbe_stage")
            nc.sync.dma_start(out=gbe_stage, in_=dd.rearrange("(a d) -> a d", a=1))
            nc.vector.tensor_copy(out=dst, in_=gbe_stage)

        residA = ffx.tile([128, ND, HQ], F32R, name="residA")
        h1fA = ffx.tile([128, ND, HQ], BF16, name="h1fA")
        gTA = ffx.tile([128, NF, HQ], BF16, name="gTA")

        # ---------------- era A: attention half 0 + K/V production ----------
        # bucket tasks per pair (kt for pair p+2, v-half-1 chunks for pairs<4)
        # so every tile is emitted before its consuming pair
        for t in (0, 1):
            for sc4 in range(4):
                task_kt(t, sc4)()
        for c in range(NK):
            task_v(0, c)()
        kv_sched = []
        for p in range(NP):
            ts = []
            if p < 6:
                for sc4 in range(4):
                    ts.append(task_kt(p + 2, sc4))
            if p < 4:
                for c in range(4 * p, 4 * p + 4):
                    ts.append(task_v(1, c))
            kv_sched.append(ts)
        attention_half(0, kv_sched, bucketed=True)
        xkp_cm.close()

        # ---------------- era B: attention half 1 + FFN half 0 --------------
        pbA = {}
        ffnA = make_ffn_tasks(0, mmp, mmp, lnsbA, wstA, residA,
                              h1fA, gTA, pbA, defer_gelu=True)
        attention_half(1, ffnA["pre"])

        dump("KT", KT)
        dump("V3", V3)
        dump("QT", QT)
        dump("hT8", hT8)
        dump("h1fA", h1fA)
        dump("gTA", gTA)
        dump("residA", residA)
        # ---------------- era C: FFN half 1 ----------------
        att2.close()
        attbig_cm.close()
        att_ps.close()
        ffn_cm = ExitStack()
        ffn2 = ffn_cm.enter_context(tc.tile_pool(name="ffn2", bufs=1))
        pmm = ffn_cm.enter_context(tc.tile_pool(name="pmm", bufs=4, space="PSUM"))
        pmm2 = ffn_cm.enter_context(tc.tile_pool(name="pmm2", bufs=4, space="PSUM"))
        lnsbB = ffn_cm.enter_context(tc.tile_pool(name="lnsbB", bufs=2))
        wstB = ffn_cm.enter_context(tc.tile_pool(name="wstB", bufs=4))
        residB = ffn2.tile([128, ND, HQ], F32R, name="residB")
        h1fB = ffn2.tile([128, ND, HQ], BF16, name="h1fB")
        gTB = ffn2.tile([128, NF, HQ], BF16, name="gTB")
        pbA["mm"] = pmm
        pbA["lnsb"] = lnsbB
        pbA["wst"] = wstB
        pbB = {"mm": pmm2, "lnsb": lnsbB, "wst": wstB}
        ffnB = make_ffn_tasks(1, pmm2, pmm2, lnsbB, wstB, residB,
                              h1fB, gTB, pbB, defer_gelu=False)

        def zip_run(a, b):
            ia = ib = 0
            na, nb = len(a), len(b)
            while ia < na or ib < nb:
                if ia < na and (ib >= nb or ia * nb <= ib * na):
                    a[ia]()
                    ia += 1
                elif ib < nb:
                    b[ib]()
                    ib += 1

        # phase 1: half-0 gelus on ACT while the PE runs half-1 oproj/LN1/fc1
        zip_run(ffnA["gelu"], ffnB["pre"])
        # phase 2: half-1 gelus on ACT while the PE runs half-0 fc2 (+LN2 stats)
        zip_run(ffnB["gelu"], ffnA["post"])
        # phase 3: half-1 fc2 + LN2 + output
        for task in ffnB["post"]:
            task()
        dump("h1fB", h1fB)
        dump("gTB", gTB)
        ffn_cm.close()

    nc.compile()
    return nc


_CACHE = {}


def _get_built(use_mask, use_bv, use_bf2=False, dbg=False):
    key = (use_mask, use_bv, use_bf2, dbg)
    if key not in _CACHE:
        _CACHE[key] = _build(use_mask, use_bv, use_bf2, dbg)
    return _CACHE[key]


def kernel(x, mask, wq, bq, wk, bk, wv, bv, wo, bo, g1, be1, w1, bf1, w2, bf2, g2, be2):
    bf = ml_dtypes.bfloat16
    e4 = ml_dtypes.float8_e4m3
    f4 = np.float32
    x = np.asarray(x, f4)
    madd_full = (-10000.0 * (1.0 - np.asarray(mask).astype(f4)))  # [B, S]
    use_mask = bool((madd_full != 0.0).any())
    use_bv = bool(np.any(np.asarray(bv) != 0))
    use_bf2 = bool(np.any(np.asarray(bf2) != 0))
    nc = _get_built(use_mask, use_bv, use_bf2)

    def tile_w(w, dt, scale=1.0):
        # [D, N] -> [128, D/128, N]
        w = (np.asarray(w, f4) * scale).astype(dt)
        return np.ascontiguousarray(w.reshape(-1, 128, w.shape[1]).transpose(1, 0, 2))

    w1h = np.asarray(w1, f4).astype(bf).reshape(ND, 128, 8, 512).transpose(1, 2, 0, 3)
    w2h = np.asarray(w2, f4).astype(bf).reshape(2, 16, 128, 4, 256).transpose(2, 3, 0, 1, 4)
    shared = {
        "wq": tile_w(wq, e4, WS),
        "wk": tile_w(wk, e4, WS),
        "wv": tile_w(wv, e4, WS),
        "wo": tile_w(wo, e4, WS),
        "w1": np.ascontiguousarray(w1h),
        "w2": np.ascontiguousarray(w2h),
        "bq": np.asarray(bq, f4), "bk": np.asarray(bk, f4),
        "bf1": np.asarray(bf1, f4),
        "g1": np.asarray(g1, f4),
        "be1": np.asarray(be1, f4), "g2": np.asarray(g2, f4),
        "be2": np.asarray(be2, f4),
    }
    if use_bv:
        shared["bv"] = (np.asarray(bv, f4) * WS).astype(bf)
    if use_bf2:
        shared["bf2"] = np.asarray(bf2, f4)

    # [D, S] -> [128, ND, S] pre-tiled transposes
    bo_f = np.asarray(bo, f4)
    xTt = {b: np.ascontiguousarray(
        x[b].T.reshape(ND, 128, S).transpose(1, 0, 2)) for b in range(B)}
    xTt_8 = {b: xTt[b].astype(e4) for b in range(B)}
    bo_t = bo_f.reshape(ND, 128).T[:, :, None]          # [128, ND, 1]
    in_maps = []
    for c in range(NCORES):
        b, q0 = c // 4, (c % 4) * QS
        m = dict(shared)
        m["xkT"] = xTt_8[b]
        m["xqT"] = np.ascontiguousarray(xTt_8[b][:, :, q0:q0 + QS])
        m["xqTf"] = np.ascontiguousarray(xTt[b][:, :, q0:q0 + QS] + bo_t).astype(bf)
        if use_mask:
            m["madd"] = np.ascontiguousarray(madd_full[b])
        in_maps.append(m)

    res = run_bass_kernel_spmd(nc, in_maps, core_ids=list(range(NCORES)))
    kernel.last_result = res
    if res.exec_time_ns is not None:
        print(f"HW exec time: {res.exec_time_ns} ns")

    y = np.empty((B, S, D), np.float32)
    for c in range(NCORES):
        b, q0 = c // 4, (c % 4) * QS
        y[b, q0:q0 + QS, :] = np.asarray(res.results[c]["yT"], np.float32).T
    return y


# revision 39
# speedup vs baseline: 1.2229x; 1.0279x over previous
"""Trainium2 Bass/Tile kernel for a dense transformer block.

B=2, S=2048, D=1024, H=16 heads (dh=64), FF=4096, f32 IO.

Sharding: 8 cores = (2 batches) x (4 query-slices of 512 tokens), zero
cross-core communication (K/V recomputed per core).

v2: fp8 attention.  All attention GEMMs (Q/K/V/O projections, AV) run as
fp8e4m3 DoubleRow matmuls (0.5 PE cycles/row); weights are pre-scaled by
64 on the host so values clear the fp8 subnormal range, and the 1/64 (or
1/4096) compensation folds into the existing psum-drain ops.  Scores stay
K=64 matmuls but with fp8 operands; softmax exp runs at scale=1/8 (the
1/sqrt(dh)) and writes fp8 directly, which feeds the DoubleRow AV.  The
FFN stays bf16 for accuracy (fp8 FFN breaches the 2e-2 gate).  LayerNorm
stats/broadcast matmuls and the reciprocal broadcasts use float32r moving
operands (1 cycle/row vs 4 for fp32): every producer writes the f32r tile
directly so the BIR verifier sees rounded inputs.
"""

import os
from contextlib import ExitStack

import numpy as np
import ml_dtypes

import concourse.bass as bass
import concourse.tile as tile
from concourse import bacc, mybir
from concourse.bass_utils import run_bass_kernel_spmd

BF16 = mybir.dt.bfloat16
F32 = mybir.dt.float32
F32R = mybir.dt.float32r
F8E4 = mybir.dt.float8e4
AF = mybir.ActivationFunctionType
OP = mybir.AluOpType
PM = mybir.MatmulPerfMode

B, S, D, H, FF = 2, 2048, 1024, 16, 4096
DH = D // H            # 64
NCORES = 8
QS = S // 4            # 512 queries per core
NK = S // 128          # 16 key chunks
ND = D // 128          # 8 feature chunks
NF = FF // 128         # 32 ff chunks
NP = H // 2            # 8 head pairs
VW = DH + 1            # 65 = head width + ones column
EPS = 1e-12
WS = 64.0              # host-side fp8 weight scale
IWS = 1.0 / WS
IWS2 = IWS * IWS


def _blocks(n, w):
    return [list(range(i, min(i + w, n))) for i in range(0, n, w)]


def _build(use_mask, use_bv, use_bf2, dbg=False):
    nc = bacc.Bacc("TRN2", target_bir_lowering=False, debug=False)

    def din(name, shape, dtype):
        return nc.dram_tensor(name, shape, dtype, kind="ExternalInput").ap()

    dbg_outs = {}

    def dump(name, tl):
        if not dbg:
            return
        dt = tl.dtype if hasattr(tl, "dtype") else F32
        dd = nc.dram_tensor(f"dbg_{name}", list(tl.shape), dt,
                            kind="ExternalOutput").ap()
        nc.sync.dma_start(out=dd, in_=tl)
        dbg_outs[name] = dd

    # All large operands are pre-tiled on the host to [128, ...] layouts so
    # each load is one fully-contiguous-per-partition DMA.
    xkT_d = din("xkT", [128, ND, S], F8E4)
    xqT_d = din("xqT", [128, ND, QS], F8E4)
    xqTf_d = din("xqTf", [128, ND, QS], F32)      # f32 x slice, +bo folded in
    wq_d = din("wq", [128, ND, D], F8E4)          # pre-scaled by WS on host
    wk_d = din("wk", [128, ND, D], F8E4)          # pre-scaled by WS
    wv_d = din("wv", [128, ND, D], F8E4)          # pre-scaled by WS
    wo_d = din("wo", [128, ND, D], F8E4)          # pre-scaled by WS
    w1_d = din("w1", [128, 8, ND, 512], BF16)     # [p, jb, k, n]
    w2_d = din("w2", [128, 2, 4, 8, 512], BF16)   # [p, jb, kq, k8, n]
    bq_d = din("bq", [D], F32)
    bk_d = din("bk", [D], F32)
    bf1_d = din("bf1", [FF], F32)
    g1_d = din("g1", [D], F32)
    be1_d = din("be1", [D], F32)
    g2_d = din("g2", [D], F32)
    be2_d = din("be2", [D], F32)
    bf2_d = din("bf2", [D], F32) if use_bf2 else None
    bv_d = din("bv", [D], BF16) if use_bv else None     # pre-scaled by WS
    madd_d = din("madd", [S], F32) if use_mask else None
    yT_d = nc.dram_tensor("yT", [D, QS], F32, kind="ExternalOutput").ap()

    with tile.TileContext(nc) as tc, ExitStack() as glob:
        const = glob.enter_context(tc.tile_pool(name="const", bufs=1))
        gx = glob.enter_context(tc.tile_pool(name="gx", bufs=1))
        att_ps = ExitStack()
        avp = att_ps.enter_context(tc.tile_pool(name="avp", bufs=2, space="PSUM"))
        mmp = att_ps.enter_context(tc.tile_pool(name="mmp", bufs=2, space="PSUM"))
        scp_cm = ExitStack()
        scp = scp_cm.enter_context(tc.tile_pool(name="scp", bufs=2, space="PSUM"))

        # ---------------- big tiles + front-loaded DMAs ----------------
        hT8 = gx.tile([128, ND, QS], F8E4, name="hT8")          # 64*h, attn out
        attbig_cm = ExitStack()
        attbig = attbig_cm.enter_context(tc.tile_pool(name="attbig", bufs=1))
        xk_sb = attbig.tile([128, ND, S], F8E4, name="xk_sb")   # x^T of the batch
        KT = attbig.tile([128, ND, S], F8E4, name="KT")         # K^T (k+bk)
        V3 = attbig.tile([128, NK, H * VW], F8E4, name="V3")    # V + ones cols
        QT = attbig.tile([128, ND, QS], F8E4, name="QT")
        hTu = attbig.tile([128, ND, QS], BF16, name="hTu")      # unnormalized AV
        att = ExitStack()
        wkv = att.enter_context(tc.tile_pool(name="wkv", bufs=1))
        wk_sb = wkv.tile([128, ND, D], F8E4, name="wk_sb")
        wv_sb = wkv.tile([128, ND, D], F8E4, name="wv_sb")
        p0 = ExitStack()
        p0pool = p0.enter_context(tc.tile_pool(name="p0pool", bufs=1))
        xq_sb = p0pool.tile([128, ND, QS], F8E4, name="xq_sb")
        wq_sb = p0pool.tile([128, ND, D], F8E4, name="wq_sb")
        # one consolidated DMA per tile: fewer HWDGE descriptor-gen slices
        nc.sync.dma_start(out=xq_sb, in_=xqT_d[:])
        nc.sync.dma_start(out=wq_sb[:, :, 0:512], in_=wq_d[:, :, 0:512])
        nc.sync.dma_start(out=wq_sb[:, :, 512:D], in_=wq_d[:, :, 512:D])
        # ---------------- constants & small params ----------------
        bq_sb = const.tile([128, ND], F32, name="bq_sb")
        nc.sync.dma_start(out=bq_sb, in_=bq_d.rearrange("(c p) -> p c", p=128))
        bk_sb = const.tile([128, ND], F32, name="bk_sb")
        nc.sync.dma_start(out=bk_sb, in_=bk_d.rearrange("(c p) -> p c", p=128))
        bf1_sb = const.tile([128, NF], F32, name="bf1_sb")
        nc.sync.dma_start(out=bf1_sb, in_=bf1_d.rearrange("(c p) -> p c", p=128))
        if use_bf2:
            bf2_sb = const.tile([128, ND], F32, name="bf2_sb")
            nc.sync.dma_start(out=bf2_sb, in_=bf2_d.rearrange("(c p) -> p c", p=128))
        if use_mask:
            madd_sb = const.tile([128, NK], F32, name="madd_sb")
            nc.sync.dma_start(out=madd_sb, in_=madd_d.rearrange("(c p) -> p c", p=128))
        if use_bv:
            bv_sb = const.tile([1, D], BF16, name="bv_sb")
            nc.sync.dma_start(out=bv_sb, in_=bv_d.rearrange("(a d) -> a d", a=1))
            ones1b = const.tile([1, 128], BF16, name="ones1b")
            nc.vector.memset(ones1b, 1.0)
        # reciprocal broadcast selectors carry the 64x for the fp8 hT scale.
        # (memset can't write f32r directly; stage in f32 and DVE-round.)
        stg = const.tile([128, 4], F32, name="stg")
        nc.vector.memset(stg[:, 0:2], 0.0)
        nc.vector.memset(stg[0:1, 0:1], WS)    # unused marker
        nc.vector.memset(stg[:, 2:3], 1.0 / D)
        indstg = const.tile([1, 128 + QS], F32, name="indstg")
        nc.vector.memset(indstg, 0.0)
        nc.vector.memset(indstg[0:1, 0:64], WS)
        indE = const.tile([1, 128], F32R, name="indE")
        nc.vector.tensor_copy(out=indE, in_=indstg[0:1, 0:128])
        nc.vector.memset(indstg[0:1, 0:64], 0.0)
        nc.vector.memset(indstg[0:1, 64:128], WS)
        indO = const.tile([1, 128], F32R, name="indO")
        nc.vector.tensor_copy(out=indO, in_=indstg[0:1, 0:128])
        # stats matmul lhsT carries the 1/D so PSUM sums land as mean/moment2
        invD128 = const.tile([128, 1], F32R, name="invD128")
        nc.vector.tensor_copy(out=invD128, in_=stg[:, 2:3])
        ones512 = const.tile([1, QS], F32R, name="ones512")
        nc.vector.memset(indstg[0:1, 128:128 + QS], 1.0)
        nc.vector.tensor_copy(out=ones512, in_=indstg[0:1, 128:128 + QS])
        eps_sb = const.tile([1, 1], F32, name="eps_sb")
        nc.vector.memset(eps_sb, EPS)
        actwarm = const.tile([1, 1], F32, name="actwarm")
        # prewarm the exp ACT table set during the initial DMA wait
        nc.scalar.activation(actwarm, eps_sb, AF.Exp)
        # only the columns phase-A needs block the startup queue
        nc.sync.dma_start(out=wk_sb[:, :, 0:256], in_=wk_d[:, :, 0:256])
        nc.sync.dma_start(out=xk_sb, in_=xkT_d[:])
        nc.sync.dma_start(out=wv_sb[:, :, 0:512], in_=wv_d[:, :, 0:512])
        nc.sync.dma_start(out=wk_sb[:, :, 256:D], in_=wk_d[:, :, 256:D])
        nc.sync.dma_start(out=wv_sb[:, :, 512:D], in_=wv_d[:, :, 512:D])
        wo_sb = gx.tile([128, ND, D], F8E4, name="wo_sb")
        nc.sync.dma_start(out=wo_sb, in_=wo_d[:])
        xqf_sb = gx.tile([128, ND, QS], F32, name="xqf_sb")
        nc.sync.dma_start(out=xqf_sb, in_=xqTf_d[:])

        nc.vector.memset(
            V3.rearrange("p c (h w) -> p (c h) w", w=VW)[:, :, DH:DH + 1], 1.0)

        # ---------------- phase 0: Q projection (fp8 DoubleRow) ----------------
        for tb in _blocks(ND, 2):
            ps = {}
            for t in tb:
                ps[t] = mmp.tile([128, QS], F32, tag="mm", name=f"qtps{t}")
            for k in range(ND // 2):
                for t in tb:
                    nc.tensor.matmul(ps[t],
                                     lhsT=wq_sb[:, 2 * k:2 * k + 2, t * 128:(t + 1) * 128],
                                     rhs=xq_sb[:, 2 * k:2 * k + 2, :],
                                     start=(k == 0), stop=(k == ND // 2 - 1),
                                     perf_mode=PM.DoubleRow)
            for t in tb:
                # QT = fp8(q + bq) = psum/64 + bq
                nc.vector.tensor_scalar(out=QT[:, t, :], in0=ps[t], scalar1=IWS,
                                        scalar2=bq_sb[:, t:t + 1],
                                        op0=OP.mult, op1=OP.add)
        p0.close()

        # ---------------- attention-era pools ----------------
        att2 = ExitStack()
        recipp = att2.enter_context(tc.tile_pool(name="recipp", bufs=2))
        expp = att2.enter_context(tc.tile_pool(name="expp", bufs=6))

        def task_kt(t, sc4):
            def run():
                ps = mmp.tile([128, 512], F32, tag="mm", name=f"ktps{t}_{sc4}")
                for k in range(ND // 2):
                    nc.tensor.matmul(ps,
                                     lhsT=wk_sb[:, 2 * k:2 * k + 2, t * 128:(t + 1) * 128],
                                     rhs=xk_sb[:, 2 * k:2 * k + 2, sc4 * 512:(sc4 + 1) * 512],
                                     start=(k == 0), stop=(k == ND // 2 - 1),
                                     perf_mode=PM.DoubleRow)
                # KT = fp8(k + bk) = psum/64 + bk
                nc.vector.tensor_scalar(
                    out=KT[:, t, sc4 * 512:(sc4 + 1) * 512], in0=ps,
                    scalar1=IWS, scalar2=bk_sb[:, t:t + 1], op0=OP.mult, op1=OP.add)
            return run

        def task_v(nh, c):
            def run():
                ps = mmp.tile([128, 512], F32, tag="mm", name=f"vps{nh}_{c}")
                if use_bv:
                    nc.tensor.matmul(ps, lhsT=ones1b,
                                     rhs=bv_sb[:, nh * 512:(nh + 1) * 512],
                                     start=True, stop=False)
                for k in range(ND // 2):
                    nc.tensor.matmul(ps,
                                     lhsT=xk_sb[:, 2 * k:2 * k + 2, c * 128:(c + 1) * 128],
                                     rhs=wv_sb[:, 2 * k:2 * k + 2, nh * 512:(nh + 1) * 512],
                                     start=(k == 0 and not use_bv),
                                     stop=(k == ND // 2 - 1),
                                     perf_mode=PM.DoubleRow)
                out_ap = V3[:, c, :].rearrange("p (h w) -> p h w", w=VW)[:, 8 * nh:8 * nh + 8, 0:DH]
                # V3 = fp8(v) = psum/64
                nc.vector.tensor_scalar_mul(
                    out=out_ap, in0=ps.rearrange("p (h w) -> p h w", w=DH), scalar1=IWS)
            return run

        # up-front: K^T tiles 0,1 and V heads 0..7 (needed by pairs 0..3)
        for t in (0, 1):
            for sc4 in range(4):
                task_kt(t, sc4)()
        for c in range(NK):
            task_v(0, c)()

        def tasks_for_pair(p):
            ts = []
            if p < 6:
                for sc4 in range(4):
                    ts.append(task_kt(p + 2, sc4))
            if p < 4:
                for c in range(4 * p, 4 * p + 4):
                    ts.append(task_v(1, c))
            return ts

        # ---------------- attention pairs ----------------
        tailB_pending = []

        def emit_tailB(p, recE, recO, pool=None):
            rbc = (pool or mmp).tile([128, QS], F32, tag="mm", name=f"rbc{p}")
            nc.tensor.matmul(rbc, lhsT=indE, rhs=recE, start=True, stop=False)
            nc.tensor.matmul(rbc, lhsT=indO, rhs=recO, start=False, stop=True)
            # hT8 = fp8(hTu * 64*rec) = fp8(64*h)
            nc.vector.tensor_mul(out=hT8[:, p, :], in0=hTu[:, p, :], in1=rbc)

        for p in range(NP):
            t = p
            av = {}
            av[0] = avp.tile([VW, QS], F32, tag="av", name=f"av{p}e")
            av[1] = avp.tile([VW, QS], F32, tag="av", name=f"av{p}o")
            pend = []   # exp tiles awaiting their AV matmul, one step behind
            tasks = tasks_for_pair(p)
            ntasks = len(tasks)

            def emit_av(i, ets):
                for parity in (0, 1):
                    h = 2 * p + parity
                    nc.tensor.matmul(av[parity],
                                     lhsT=V3[:, 2 * i:2 * i + 2, h * VW:(h + 1) * VW],
                                     rhs=ets[parity],
                                     start=(i == 0), stop=(i == 7),
                                     perf_mode=PM.DoubleRow)

            for i in range(8):
                # scores for key chunks (2i, 2i+1), per head parity (the two
                # parities sit in different PE row groups).
                sct = {}
                for parity in (0, 1):
                    sct[parity] = scp.tile([128, 2, 512], F32, tag="sc",
                                           name=f"sc{p}_{i}_{parity}")
                for cc in (0, 1):
                    ch = 2 * i + cc
                    for parity in (0, 1):
                        base = 64 * parity
                        nc.tensor.matmul(sct[parity][:, cc, :],
                                         lhsT=KT[base:base + 64, t, ch * 128:(ch + 1) * 128],
                                         rhs=QT[base:base + 64, t, :],
                                         start=True, stop=True)
                ets = {}
                for parity in (0, 1):
                    et = expp.tile([128, 2, 512], F8E4, tag="exp",
                                   name=f"ex{p}_{i}_{parity}")
                    if use_mask:
                        for cc in (0, 1):
                            ch = 2 * i + cc
                            nc.scalar.activation(et[:, cc, :], sct[parity][:, cc, :],
                                                 AF.Exp, bias=madd_sb[:, ch:ch + 1],
                                                 scale=0.125)
                    else:
                        nc.scalar.activation(et, sct[parity], AF.Exp, scale=0.125)
                    ets[parity] = et
                pend.append((i, ets))
                if len(pend) > 1:
                    emit_av(*pend.pop(0))
                if i == 4 and tailB_pending:
                    emit_tailB(*tailB_pending.pop(0))
                lo = (ntasks * i) // 8
                hi = (ntasks * (i + 1)) // 8
                for task in tasks[lo:hi]:
                    task()
            while pend:
                emit_av(*pend.pop(0))

            # tail A: drain AV psum, compute reciprocals of the denominators.
            # reciprocal_approx_* needs f32 SBUF input (from psum it reads
            # garbage on hw) and f32 output, so: copy den to SBUF, recip,
            # round into the f32r matmul operand.
            denE = recipp.tile([1, QS], F32, tag="den", bufs=2, name=f"denE{p}")
            denO = recipp.tile([1, QS], F32, tag="den", bufs=2, name=f"denO{p}")
            recEf = recipp.tile([1, QS], F32, tag="recf", bufs=2, name=f"recEf{p}")
            recOf = recipp.tile([1, QS], F32, tag="recf", bufs=2, name=f"recOf{p}")
            recE = recipp.tile([1, QS], F32R, tag="rec", bufs=4, name=f"recE{p}")
            recO = recipp.tile([1, QS], F32R, tag="rec", bufs=4, name=f"recO{p}")
            if p == NP - 1:
                nc.scalar.copy(denE, av[0][64:65, :])
                nc.scalar.copy(denO, av[1][64:65, :])
                nc.vector.reciprocal_approx_fast(out=recEf, in_=denE)
                nc.vector.reciprocal_approx_fast(out=recOf, in_=denO)
                nc.vector.tensor_copy(out=recE, in_=recEf)
                nc.vector.tensor_copy(out=recO, in_=recOf)
                nc.scalar.copy(hTu[0:64, p, :], av[0][0:64, :])
                nc.scalar.copy(hTu[64:128, p, :], av[1][0:64, :])
            else:
                nc.vector.tensor_copy(out=hTu[0:64, p, :], in_=av[0][0:64, :])
                nc.vector.tensor_copy(out=hTu[64:128, p, :], in_=av[1][0:64, :])
                nc.vector.tensor_copy(out=denE, in_=av[0][64:65, :])
                nc.vector.tensor_copy(out=denO, in_=av[1][64:65, :])
                nc.vector.reciprocal_approx_fast(out=recEf, in_=denE)
                nc.vector.reciprocal_approx_fast(out=recOf, in_=denO)
                nc.vector.tensor_copy(out=recE, in_=recEf)
                nc.vector.tensor_copy(out=recO, in_=recOf)
            tailB_pending.append((p, recE, recO))

        dump("QT", QT)
        dump("KT", KT)
        dump("V3", V3)
        dump("hTu", hTu)
        scp_cm.close()
        att_ps.close()
        ffn_ps = ExitStack()
        pmmA = ffn_ps.enter_context(tc.tile_pool(name="pmmA", bufs=4, space="PSUM"))
        pmmB = ffn_ps.enter_context(tc.tile_pool(name="pmmB", bufs=2, space="PSUM"))
        while tailB_pending:
            emit_tailB(*tailB_pending.pop(0), pool=pmmA)
        # prewarm the sqrt table set while PE runs o-proj (LN1 needs it)
        nc.scalar.activation(actwarm, eps_sb, AF.Sqrt)

        dump("hT8", hT8)
        att2.close()
        att.close()
        attbig_cm.close()

        # ---------------- FFN-era pools ----------------
        ffn = ExitStack()
        fx = ffn.enter_context(tc.tile_pool(name="fx", bufs=1))
        wst = ffn.enter_context(tc.tile_pool(name="wst", bufs=2))
        sqp = ffn.enter_context(tc.tile_pool(name="sqp", bufs=2))
        smp = ffn.enter_context(tc.tile_pool(name="smp", bufs=1))
        tmpp = ffn.enter_context(tc.tile_pool(name="tmpp", bufs=3))
        outp = ffn.enter_context(tc.tile_pool(name="outp", bufs=2))

        g1_sb = fx.tile([1, D], F32R, name="g1_sb")
        be1_sb = fx.tile([1, D], F32R, name="be1_sb")
        g2_sb = fx.tile([1, D], F32R, name="g2_sb")
        be2_sb = fx.tile([1, D], F32R, name="be2_sb")
        gbe_stage = fx.tile([1, 4, D], F32, name="gbe_stage")
        resid = fx.tile([128, ND, QS], F32R, name="resid")  # resid1 then resid2
        h1bf = fx.tile([128, ND, QS], BF16, name="h1bf")
        h1f = fx.tile([128, ND, QS], F32, name="h1f")
        gT = fx.tile([128, NF, QS], BF16, name="gT")
        for idx, dd in enumerate((g1_d, be1_d, g2_d, be2_d)):
            nc.sync.dma_start(out=gbe_stage[:, idx, :],
                              in_=dd.rearrange("(a d) -> a d", a=1))
        # round the LN vectors into f32r tiles (matmul operands must be
        # produced as f32r for the BIR verifier)
        nc.vector.tensor_copy(out=g1_sb, in_=gbe_stage[:, 0, :])
        nc.vector.tensor_copy(out=be1_sb, in_=gbe_stage[:, 1, :])
        nc.vector.tensor_copy(out=g2_sb, in_=gbe_stage[:, 2, :])
        nc.vector.tensor_copy(out=be2_sb, in_=gbe_stage[:, 3, :])

        def ln_stats_start(tagn):
            st_s = pmmB.tile([1, QS], F32, tag="stat", bufs=2, name=f"sts_{tagn}")
            st_q = pmmB.tile([1, QS], F32, tag="stat", bufs=2, name=f"stq_{tagn}")
            return st_s, st_q

        def ln_stats_chunk(st, resid, c, tagn):
            st_s, st_q = st
            sq = sqp.tile([128, QS], F32R, tag="sq", name=f"sq{tagn}_{c}")
            nc.vector.tensor_mul(out=sq, in0=resid[:, c, :], in1=resid[:, c, :])
            nc.tensor.matmul(st_s, lhsT=invD128, rhs=resid[:, c, :],
                             start=(c == 0), stop=(c == ND - 1))
            nc.tensor.matmul(st_q, lhsT=invD128, rhs=sq,
                             start=(c == 0), stop=(c == ND - 1))

        # o-projection (fp8 DR) + residual 1, LN1 stats interleaved per chunk
        st1 = ln_stats_start("a")
        for jb in _blocks(ND, 4):
            ps = {j: pmmA.tile([128, QS], F32, tag="mm", name=f"ops{j}") for j in jb}
            for k in range(ND // 2):
                for j in jb:
                    nc.tensor.matmul(ps[j],
                                     lhsT=wo_sb[:, 2 * k:2 * k + 2, j * 128:(j + 1) * 128],
                                     rhs=hT8[:, 2 * k:2 * k + 2, :],
                                     start=(k == 0), stop=(k == ND // 2 - 1),
                                     perf_mode=PM.DoubleRow)
            for j in jb:
                # resid1 = psum/4096 + (x + bo);  psum = 64h @ 64wo
                nc.vector.scalar_tensor_tensor(
                    out=resid[:, j, :], in0=ps[j], scalar=IWS2,
                    in1=xqf_sb[:, j, :], op0=OP.mult, op1=OP.add)
                ln_stats_chunk(st1, resid, j, "a")

        def ln_finish(st, resid, g_sb, be_sb, writer, tagn):
            st_s, st_q = st
            u = smp.tile([1, QS], F32, tag="u", name=f"u_{tagn}")
            var0 = smp.tile([1, QS], F32, tag="var0", name=f"var0_{tagn}")
            var = smp.tile([1, QS], F32, tag="var", name=f"var_{tagn}")
            std = smp.tile([1, QS], F32, tag="std", name=f"std_{tagn}")
            avecf = smp.tile([1, QS], F32, tag="avecf", name=f"avecf_{tagn}")
            avec = smp.tile([1, QS], F32R, tag="avec", name=f"avec_{tagn}")
            cvec = smp.tile([1, QS], F32R, tag="cvec", name=f"cvec_{tagn}")
            nc.vector.tensor_scalar_mul(out=u, in0=st_s, scalar1=1.0)  # mean, to SBUF
            nc.vector.scalar_tensor_tensor(out=var0, in0=u, scalar=-1.0, in1=u,
                                           op0=OP.mult, op1=OP.mult)
            nc.vector.scalar_tensor_tensor(out=var, in0=st_q, scalar=1.0, in1=var0,
                                           op0=OP.mult, op1=OP.add)
            nc.scalar.activation(std, var, AF.Sqrt, bias=eps_sb, scale=1.0)
            nc.vector.reciprocal_approx_fast(out=avecf, in_=std)
            nc.vector.tensor_copy(out=avec, in_=avecf)
            nc.vector.scalar_tensor_tensor(out=cvec, in0=u, scalar=-1.0,
                                           in1=avecf, op0=OP.mult, op1=OP.mult)
            for c in range(ND):
                abc = pmmB.tile([128, QS], F32, tag="bc", bufs=2, name=f"abc{tagn}_{c}")
                nc.tensor.matmul(abc, lhsT=g_sb[0:1, c * 128:(c + 1) * 128], rhs=avec,
                                 start=True, stop=True)
                cbc = pmmB.tile([128, QS], F32, tag="bc", bufs=2, name=f"cbc{tagn}_{c}")
                nc.tensor.matmul(cbc, lhsT=g_sb[0:1, c * 128:(c + 1) * 128], rhs=cvec,
                                 start=True, stop=False)
                nc.tensor.matmul(cbc, lhsT=be_sb[0:1, c * 128:(c + 1) * 128], rhs=ones512,
                                 start=False, stop=True)
                tmp = tmpp.tile([128, QS], F32, tag="tmp", name=f"lnt{tagn}_{c}")
                nc.vector.tensor_mul(out=tmp, in0=resid[:, c, :], in1=abc)
                writer(c, tmp, cbc)

        def ln1_writer(c, tmp, cbc):
            nc.vector.tensor_add(out=h1f[:, c, :], in0=tmp, in1=cbc)
            nc.vector.tensor_copy(out=h1bf[:, c, :], in_=h1f[:, c, :])

        dump("resid1", resid)
        ln_finish(st1, resid, g1_sb, be1_sb, ln1_writer, "a")
        dump("h1f", h1f)

        # FFN fc1 + gelu (w1 streamed via SWDGE, prefetched one block ahead)
        w1tiles = {}
        for jb in range(2):
            w1tiles[jb] = wst.tile([128, ND, 512], BF16, tag="wst", name=f"w1t{jb}")
            nc.gpsimd.dma_start(out=w1tiles[jb], in_=w1_d[:, jb, :, :])
        for jb in range(8):
            w1t = w1tiles.pop(jb)
            if jb + 2 < 8:
                w1tiles[jb + 2] = wst.tile([128, ND, 512], BF16, tag="wst",
                                           name=f"w1t{jb + 2}")
                nc.gpsimd.dma_start(out=w1tiles[jb + 2], in_=w1_d[:, jb + 2, :, :])
            ps = {j: pmmA.tile([128, QS], F32, tag="mm", name=f"f1ps{jb}_{j}")
                  for j in range(4)}
            for k in range(ND):
                for j in range(4):
                    nc.tensor.matmul(ps[j], lhsT=w1t[:, k, j * 128:(j + 1) * 128],
                                     rhs=h1bf[:, k, :], start=(k == 0), stop=(k == ND - 1))
            for j in range(4):
                jg = jb * 4 + j
                nc.scalar.activation(gT[:, jg, :], ps[j], AF.Gelu,
                                     bias=bf1_sb[:, jg:jg + 1], scale=1.0)
        # prewarm the sqrt table set while PE runs fc2 (LN2 needs it)
        nc.scalar.activation(actwarm, eps_sb, AF.Sqrt)

        # FFN fc2 + residual 2 (w2 streamed via SWDGE in pre-tiled blocks, bf16)
        w2tiles = {}

        def w2_prefetch(n):
            jb, kq = n // 4, n % 4
            w2tiles[n] = wst.tile([128, 8, 512], BF16, tag="wst",
                                  name=f"w2t{jb}_{kq}")
            nc.gpsimd.dma_start(out=w2tiles[n], in_=w2_d[:, jb, kq, :, :])

        w2_prefetch(0)
        st2 = ln_stats_start("b")
        for jb in range(2):
            ps = {j: pmmA.tile([128, QS], F32, tag="mm", name=f"f2ps{jb}_{j}")
                  for j in range(4)}
            for kq in range(4):
                n = jb * 4 + kq
                if n + 1 < 8:
                    w2_prefetch(n + 1)
                w2t = w2tiles.pop(n)
                for k8 in range(8):
                    for j in range(4):
                        nc.tensor.matmul(ps[j], lhsT=w2t[:, k8, j * 128:(j + 1) * 128],
                                         rhs=gT[:, kq * 8 + k8, :],
                                         start=(kq == 0 and k8 == 0),
                                         stop=(kq == 3 and k8 == 7))
            for j in range(4):
                jg = jb * 4 + j
                if use_bf2:
                    tmp2 = tmpp.tile([128, QS], F32, tag="tmp", name=f"f2t{jg}")
                    nc.vector.tensor_scalar(out=tmp2, in0=ps[j], scalar1=1.0,
                                            scalar2=bf2_sb[:, jg:jg + 1],
                                            op0=OP.mult, op1=OP.add)
                    nc.vector.tensor_add(out=resid[:, jg, :], in0=tmp2,
                                         in1=h1f[:, jg, :])
                else:
                    nc.vector.scalar_tensor_tensor(
                        out=resid[:, jg, :], in0=ps[j], scalar=1.0,
                        in1=h1f[:, jg, :], op0=OP.mult, op1=OP.add)
                ln_stats_chunk(st2, resid, jg, "b")

        dump("gT", gT)
        dump("resid2", resid)

        def ln2_writer(c, tmp, cbc):
            och = outp.tile([128, QS], F32, tag="out", name=f"och{c}")
            nc.vector.tensor_add(out=och, in0=tmp, in1=cbc)
            nc.sync.dma_start(out=yT_d[c * 128:(c + 1) * 128, :], in_=och)

        ln_finish(st2, resid, g2_sb, be2_sb, ln2_writer, "b")
        ffn.close()
        ffn_ps.close()

    nc.compile()
    return nc


_CACHE = {}


def _get_built(use_mask, use_bv, use_bf2=False, dbg=False):
    key = (use_mask, use_bv, use_bf2, dbg)
    if key not in _CACHE:
        _CACHE[key] = _build(use_mask, use_bv, use_bf2, dbg)
    return _CACHE[key]


def kernel(x, mask, wq, bq, wk, bk, wv, bv, wo, bo, g1, be1, w1, bf1, w2, bf2, g2, be2):
    bf = ml_dtypes.bfloat16
    e4 = ml_dtypes.float8_e4m3
    f4 = np.float32
    x = np.asarray(x, f4)
    madd_full = (-10000.0 * (1.0 - np.asarray(mask).astype(f4)))  # [B, S]
    use_mask = bool((madd_full != 0.0).any())
    use_bv = bool(np.any(np.asarray(bv) != 0))
    use_bf2 = bool(np.any(np.asarray(bf2) != 0))
    nc = _get_built(use_mask, use_bv, use_bf2)

    def tile_w(w, dt, scale=1.0):
        # [D, N] -> [128, D/128, N]
        w = (np.asarray(w, f4) * scale).astype(dt)
        return np.ascontiguousarray(w.reshape(-1, 128, w.shape[1]).transpose(1, 0, 2))

    w1h = np.asarray(w1, f4).astype(bf).reshape(ND, 128, 8, 512).transpose(1, 2, 0, 3)
    w2h = np.asarray(w2, f4).astype(bf).reshape(4, 8, 128, 2, 512).transpose(2, 3, 0, 1, 4)
    shared = {
        "wq": tile_w(wq, e4, WS),
        "wk": tile_w(wk, e4, WS),
        "wv": tile_w(wv, e4, WS),
        "wo": tile_w(wo, e4, WS),
        "w1": np.ascontiguousarray(w1h),
        "w2": np.ascontiguousarray(w2h),
        "bq": np.asarray(bq, f4), "bk": np.asarray(bk, f4),
        "bf1": np.asarray(bf1, f4),
        "g1": np.asarray(g1, f4),
        "be1": np.asarray(be1, f4), "g2": np.asarray(g2, f4),
        "be2": np.asarray(be2, f4),
    }
    if use_bv:
        shared["bv"] = (np.asarray(bv, f4) * WS).astype(bf)
    if use_bf2:
        shared["bf2"] = np.asarray(bf2, f4)

    # [D, S] -> [128, ND, S] pre-tiled transposes
    bo_f = np.asarray(bo, f4)
    xTt = {b: np.ascontiguousarray(
        x[b].T.reshape(ND, 128, S).transpose(1, 0, 2)) for b in range(B)}
    xTt_8 = {b: xTt[b].astype(e4) for b in range(B)}
    bo_t = bo_f.reshape(ND, 128).T[:, :, None]          # [128, ND, 1]
    in_maps = []
    for c in range(NCORES):
        b, q0 = c // 4, (c % 4) * QS
        m = dict(shared)
        m["xkT"] = xTt_8[b]
        m["xqT"] = np.ascontiguousarray(xTt_8[b][:, :, q0:q0 + QS])
        m["xqTf"] = np.ascontiguousarray(xTt[b][:, :, q0:q0 + QS] + bo_t)
        if use_mask:
            m["madd"] = np.ascontiguousarray(madd_full[b])
        in_maps.append(m)

    res = run_bass_kernel_spmd(nc, in_maps, core_ids=list(range(NCORES)))
    kernel.last_result = res
    if res.exec_time_ns is not None:
        print(f"HW exec time: {res.exec_time_ns} ns")

    y = np.empty((B, S, D), np.float32)
    for c in range(NCORES):
        b, q0 = c // 4, (c % 4) * QS
        y[b, q0:q0 + QS, :] = np.asarray(res.results[c]["yT"], np.float32).T
    return y
